# revision 1
# baseline (speedup 1.0000x reference)
"""MLA-style attention kernel for 8 TRN2 NeuronCores.

Sharding: core c -> batch b = c//4, heads r*4..r*4+3 where r = c%4.
Each core computes its batch's latent projections (duplicated within the
4-core group), its 4 heads' attention, and a partial output projection.
Partial outputs (transposed, [C, T]) are summed per batch on the host.

All activations on-chip use a transposed [feature, T] layout so the whole
matmul chain needs no inter-layer transposes; x and the weights are
transposed once on-chip via the PE array.  Matmuls run as float32r
(4x fp32 rate).  RoPE halves are kept planar (re rows 0:32, im rows
32:64, same permutation for q and k) which leaves dot products invariant;
the cos/sin tables are stored duplicated on both partition halves so
every DVE operand pair shares a base partition.  Scores are computed
pre-transposed (S^T tiles [k, q]) so exp writes P^T directly and the PV
matmul needs no on-chip transposes.  Causal softmax skips upper-triangle
512-blocks; diagonal blocks get an additive -1e30 mask before exp.  Softmax denominators
are accumulated with a ones-column matmul on the transposed probability
tiles and applied via a rank-1 broadcast matmul + DVE multiply.
"""
import math
import numpy as np

import concourse.bass as bass
import concourse.bacc as bacc
import concourse.mybir as mybir
import concourse.tile as tile
from concourse.bass_utils import run_bass_kernel_spmd

F32 = mybir.dt.float32
F32R = mybir.dt.float32r
BF16 = mybir.dt.bfloat16
Exp = mybir.ActivationFunctionType.Exp
Copy = mybir.ActivationFunctionType.Copy

B, T, C = 2, 2048, 2048
H = 16
HS = 128
NL = 512
RHD = 64
HLOC = 4              # heads per core
P = 128
NNL = NL // P         # 4
TCH = 512
NCH = T // TCH        # 4 chunks of T
NCS = C // TCH        # 4 c-strips for the down projection
SCALE = 1.0 / math.sqrt(HS + RHD)
NEG = -1.0e30

_NC_CACHE = {}


def _r(ap):
    return ap.bitcast(F32R)


def _deint(ap2d):
    # [p, 2d] -> (evens [p, d], odds [p, d]) along the free dim
    rr = ap2d.rearrange("p (d two) -> p two d", two=2)
    return rr[:, 0, :], rr[:, 1, :]


def build():
    nc = bacc.Bacc("TRN2", target_bir_lowering=False, debug=False, num_devices=8)

    x_ext = nc.dram_tensor("x", [TCH, C], F32R, kind="ExternalInput")
    wdq_ext = nc.dram_tensor("wdq", [NL, C], F32R, kind="ExternalInput")
    wdkv_ext = nc.dram_tensor("wdkv", [NL, C], F32R, kind="ExternalInput")
    wkr_ext = nc.dram_tensor("wkr", [RHD, C], F32R, kind="ExternalInput")
    wuq_ext = nc.dram_tensor("wuq", [HLOC * HS, NL], F32R, kind="ExternalInput")
    wuk_ext = nc.dram_tensor("wuk", [HLOC * HS, NL], F32R, kind="ExternalInput")
    wuv_ext = nc.dram_tensor("wuv", [HLOC * HS, NL], F32R, kind="ExternalInput")
    wqr_ext = nc.dram_tensor("wqr", [HLOC * RHD, NL], F32R, kind="ExternalInput")
    wo_ext = nc.dram_tensor("wo", [C, HLOC * HS], F32R, kind="ExternalInput")
    cos_ext = nc.dram_tensor("cos", [T, RHD // 2], F32R, kind="ExternalInput")
    sin_ext = nc.dram_tensor("sin", [T, RHD // 2], F32R, kind="ExternalInput")
    out_ext = nc.dram_tensor("out", [C, T], F32, kind="ExternalOutput")

    ident_dram = nc.inline_tensor(np.eye(P, dtype=np.float32), name="identc")
    ones_dram = nc.inline_tensor(np.ones((P, P), dtype=np.float32), name="onesc")
    # transposed sliding causal mask for S^T tiles [k-sub, q-chunk]:
    # m[jj, u] = 0 if u >= 384 + jj else -1e30.  For k-subtile ks the
    # diagonal-block mask is m[:, 384-128*ks : 384-128*ks+512], which allows
    # q-col qq >= ks*128 + jj.
    m = np.full((P, 896), NEG, dtype=np.float32)
    for jj in range(P):
        m[jj, 384 + jj:] = 0.0
    masks_dram = nc.inline_tensor(m, name="maskc")

    ahT_dram = nc.dram_tensor("ahT", [HLOC, HS, T], BF16)
    agin_dram = nc.dram_tensor("agin", [NL + NL + RHD, TCH], BF16)
    agout_dram = nc.dram_tensor("agout", [4, NL + NL + RHD, TCH], BF16)
    woT_dram = nc.dram_tensor("woT", [HLOC, P, C], BF16)

    with tile.TileContext(nc) as tc:
        with (
            tc.tile_pool(name="pers", bufs=1) as pers,
            tc.tile_pool(name="pmm", bufs=4, space="PSUM") as pmm,
            tc.tile_pool(name="ptp", bufs=2, space="PSUM") as ptp,
            tc.tile_pool(name="pou", bufs=1, space="PSUM") as pou,
        ):
            ident = pers.tile([P, P], F32R, tag="ident", name="ident")
            nc.sync.dma_start(ident[:], ident_dram.ap().bitcast(F32R))
            onesb = pers.tile([P, P], F32R, tag="onesb", name="onesb")
            nc.sync.dma_start(onesb[:], ones_dram.ap().bitcast(F32R))
            maskbuf = pers.tile([P, 896], BF16, tag="maskbuf", name="maskbuf")
            nc.gpsimd.dma_start(out=maskbuf[:], in_=masks_dram.ap())

            cqT = [pers.tile([P, T], BF16, tag=f"cqT{i}", name=f"cqT{i}")
                   for i in range(NNL)]
            ckvT = [pers.tile([P, T], BF16, tag=f"ckvT{i}", name=f"ckvT{i}")
                    for i in range(NNL)]
            kr = pers.tile([RHD, T], F32R, tag="kr", name="kr")
            ca = pers.tile([RHD, T], BF16, tag="ca", name="ca")
            sa = pers.tile([RHD, T], BF16, tag="sa", name="sa")

            def transpose_into(dst_ap, src_ap, eng="dve"):
                """PE-transpose src [p, w<=128] -> psum [w, p] -> copy to dst."""
                tp = ptp.tile([P, P], src_ap.dtype, tag="tp", name="tp")
                kdim = src_ap.shape[0]
                nc.tensor.transpose(
                    tp[: src_ap.shape[1], :kdim], src_ap, ident[:kdim, :kdim]
                )
                cp = nc.scalar.copy if eng == "act" else nc.vector.tensor_copy
                cp(dst_ap, tp[: src_ap.shape[1], :kdim])

            def transpose_pair_into(dst_ap, srcA, srcB, eng="dve"):
                """Two PE transposes into one psum tile, one 256-wide copy."""
                tp2 = ptp.tile([P, 2 * P], srcA.dtype, tag="tp", name="tp")
                nc.tensor.transpose(tp2[:, 0:P], srcA, ident[:])
                nc.tensor.transpose(tp2[:, P:2 * P], srcB, ident[:])
                cp = nc.scalar.copy if eng == "act" else nc.vector.tensor_copy
                cp(dst_ap, tp2[:])

            def rope(dst, dst_sl, raw, tmp, sl):
                """dst[:, dst_sl] = rope(raw) with planar re/im halves.

                raw may be PSUM or SBUF; all operand pairs share a base
                partition (tables are duplicated on both halves).
                """
                nc.vector.tensor_mul(tmp[0:32, :], raw[32:64, :], sa[32:64, sl])
                nc.vector.tensor_mul(tmp[32:64, :], raw[32:64, :], ca[32:64, sl])
                nc.vector.tensor_mul(dst[0:32, dst_sl], raw[0:32, :], ca[0:32, sl])
                nc.vector.tensor_mul(dst[32:64, dst_sl], raw[0:32, :], sa[0:32, sl])
                nc.vector.tensor_sub(
                    dst[0:32, dst_sl], dst[0:32, dst_sl], tmp[0:32, :]
                )
                nc.vector.tensor_add(
                    dst[32:64, dst_sl], dst[32:64, dst_sl], tmp[32:64, :]
                )

            # ---------------- phase B/C: up-projections + attention ---------
            with (
                tc.tile_pool(name="pw2", bufs=1) as pw2,
                tc.tile_pool(name="ph", bufs=1) as ph,
                tc.tile_pool(name="pat", bufs=1) as pat,
            ):
                # ---------------- phase A: cos/sin, x^T + down-proj by c-strip --
                with (
                    tc.tile_pool(name="pa", bufs=1) as pa,
                    tc.tile_pool(name="pw", bufs=1) as pw,
                ):
                    # ca/sa = [cos; cos], [sin; sin] transposed to [64, T]
                    for s in range(T // P):
                        for ext, dst, tg in ((cos_ext, ca, "cstrip"),
                                             (sin_ext, sa, "sstrip")):
                            strip = pa.tile([P, RHD // 2], F32R, tag=tg, bufs=2,
                                            name=tg)
                            nc.sync.dma_start(strip[:], ext.ap()[s * P:(s + 1) * P, :])
                            tp = ptp.tile([P, P], F32R, tag="tp", name="tp")
                            nc.tensor.transpose(tp[: RHD // 2, :], strip[:], ident[:])
                            nc.vector.tensor_copy(dst[0:32, s * P:(s + 1) * P],
                                                  tp[:32, :])
                            nc.vector.tensor_copy(dst[32:64, s * P:(s + 1) * P],
                                                  tp[:32, :])

                    kr_raw = pa.tile([RHD, TCH], F32, tag="kr_raw",
                                     name="kr_raw")
                    cq_part = [pa.tile([P, TCH], F32, tag=f"cqp{i}",
                                       name=f"cqp{i}") for i in range(NNL)]
                    ckv_part = [pa.tile([P, TCH], F32, tag=f"ckvp{i}",
                                        name=f"ckvp{i}") for i in range(NNL)]

                    for co in range(NCS):        # 512-wide strip of C
                        c0 = co * TCH
                        # transposed weight strips for this c-strip
                        wdqTs = [pw.tile([P, NL], F32R, tag=f"wdqT{i}",
                                         name=f"wdqT{i}") for i in range(4)]
                        wdkvTs = [pw.tile([P, NL], F32R, tag=f"wdkvT{i}",
                                          name=f"wdkvT{i}") for i in range(4)]
                        for w_ext, wTs in ((wdq_ext, wdqTs), (wdkv_ext, wdkvTs)):
                            for rp in range(NL // P // 2):
                                stripA = pw.tile([P, TCH], F32R, tag="wstripA",
                                                 bufs=2, name="wstripA")
                                stripB = pw.tile([P, TCH], F32R, tag="wstripB",
                                                 bufs=2, name="wstripB")
                                nc.sync.dma_start(
                                    stripA[:],
                                    w_ext.ap()[2 * rp * P:(2 * rp + 1) * P, c0:c0 + TCH],
                                )
                                nc.sync.dma_start(
                                    stripB[:],
                                    w_ext.ap()[(2 * rp + 1) * P:(2 * rp + 2) * P, c0:c0 + TCH],
                                )
                                for ci in range(4):
                                    transpose_pair_into(
                                        wTs[ci][:, 2 * rp * P:(2 * rp + 2) * P],
                                        stripA[:, ci * P:(ci + 1) * P],
                                        stripB[:, ci * P:(ci + 1) * P],
                                        eng="act",
                                    )
                        wkrTs = [pw.tile([P, RHD], F32R, tag=f"wkrT{i}",
                                         name=f"wkrT{i}") for i in range(4)]
                        kstrip = pw.tile([RHD, TCH], F32R, tag="kstrip",
                                         name="kstrip")
                        nc.sync.dma_start(kstrip[:], wkr_ext.ap()[:, c0:c0 + TCH])
                        for ci in range(4):
                            tp = ptp.tile([P, P], F32R, tag="tp", name="tp")
                            nc.tensor.transpose(
                                tp[:, :RHD], kstrip[:, ci * P:(ci + 1) * P],
                                ident[:RHD, :RHD],
                            )
                            ev, od = _deint(tp[:, :RHD])
                            nc.scalar.copy(wkrTs[ci][:, 0:32], ev)
                            nc.scalar.copy(wkrTs[ci][:, 32:64], od)

                        # x^T for this c-strip (this core's 512-row T-chunk only)
                        xTs = [pa.tile([P, TCH], F32R, tag=f"xt{i}",
                                       name=f"xt{i}") for i in range(4)]
                        for tp_ in range(TCH // P // 2):
                            xnA = pa.tile([P, TCH], F32R, tag="xnA", bufs=2,
                                          name="xnA")
                            xnB = pa.tile([P, TCH], F32R, tag="xnB", bufs=2,
                                          name="xnB")
                            nc.sync.dma_start(
                                xnA[:],
                                x_ext.ap()[2 * tp_ * P:(2 * tp_ + 1) * P, c0:c0 + TCH],
                            )
                            nc.sync.dma_start(
                                xnB[:],
                                x_ext.ap()[(2 * tp_ + 1) * P:(2 * tp_ + 2) * P, c0:c0 + TCH],
                            )
                            for ci in range(4):
                                transpose_pair_into(
                                    xTs[ci][:, 2 * tp_ * P:(2 * tp_ + 2) * P],
                                    xnA[:, ci * P:(ci + 1) * P],
                                    xnB[:, ci * P:(ci + 1) * P],
                                )

                        # partial down projections, accumulated across c-strips
                        for wTs, dstP in ((wdqTs, cq_part), (wdkvTs, ckv_part)):
                            for nl in range(NNL):
                                acc = pmm.tile([P, TCH], F32, tag="mm", name="mm")
                                for ci in range(4):
                                    nc.tensor.matmul(
                                        acc[:],
                                        wTs[ci][:, nl * P:(nl + 1) * P],
                                        xTs[ci][:],
                                        start=(ci == 0),
                                        stop=(ci == 3),
                                    )
                                if co == 0:
                                    nc.vector.tensor_copy(dstP[nl][:], acc[:])
                                else:
                                    nc.vector.tensor_add(
                                        dstP[nl][:], dstP[nl][:], acc[:]
                                    )
                        acc = pmm.tile([RHD, TCH], F32, tag="mm", name="mm")
                        for ci in range(4):
                            nc.tensor.matmul(
                                acc[:],
                                wkrTs[ci][:],
                                xTs[ci][:],
                                start=(ci == 0),
                                stop=(ci == 3),
                            )
                        if co == 0:
                            nc.vector.tensor_copy(kr_raw[:], acc[:])
                        else:
                            nc.vector.tensor_add(kr_raw[:], kr_raw[:], acc[:])

                    # ship partials: [cq(512); ckv(512); kr(64)] x TCH
                    for nl in range(NNL):
                        nc.gpsimd.dma_start(
                            out=agin_dram.ap()[nl * P:(nl + 1) * P, :],
                            in_=cq_part[nl][:],
                        )
                        nc.gpsimd.dma_start(
                            out=agin_dram.ap()[NL + nl * P:NL + (nl + 1) * P, :],
                            in_=ckv_part[nl][:],
                        )
                    nc.gpsimd.dma_start(out=agin_dram.ap()[2 * NL:2 * NL + RHD, :],
                                        in_=kr_raw[:])
                    nc.gpsimd.collective_compute(
                        "AllGather",
                        mybir.AluOpType.bypass,
                        replica_groups=[[0, 1, 2, 3], [4, 5, 6, 7]],
                        ins=[agin_dram.ap().opt()],
                        outs=[agout_dram.ap().opt()],
                    )
                    wuqT = [pw2.tile([P, HLOC * HS], BF16, tag=f"wuqT{i}",
                                     name=f"wuqT{i}") for i in range(NNL)]
                    wukT = [pw2.tile([P, HLOC * HS], BF16, tag=f"wukT{i}",
                                     name=f"wukT{i}") for i in range(NNL)]
                    wuvT = [pw2.tile([P, HLOC * HS], BF16, tag=f"wuvT{i}",
                                     name=f"wuvT{i}") for i in range(NNL)]
                    for w_ext, wT in ((wuq_ext, wuqT), (wuk_ext, wukT),
                                      (wuv_ext, wuvT)):
                        for rp in range(HLOC * HS // P // 2):
                            stripA = pw2.tile([P, NL], F32R, tag="usA",
                                              bufs=2, name="usA")
                            stripB = pw2.tile([P, NL], F32R, tag="usB",
                                              bufs=2, name="usB")
                            nc.sync.dma_start(
                                stripA[:],
                                w_ext.ap()[2 * rp * P:(2 * rp + 1) * P, :],
                            )
                            nc.sync.dma_start(
                                stripB[:],
                                w_ext.ap()[(2 * rp + 1) * P:(2 * rp + 2) * P, :],
                            )
                            for cs in range(NNL):
                                transpose_pair_into(
                                    wT[cs][:, 2 * rp * P:(2 * rp + 2) * P],
                                    stripA[:, cs * P:(cs + 1) * P],
                                    stripB[:, cs * P:(cs + 1) * P],
                                    eng="act",
                                )
                    wqrT = [pw2.tile([P, HLOC * RHD], BF16, tag=f"wqrT{i}",
                                     name=f"wqrT{i}") for i in range(NNL)]
                    for rs in range(HLOC * RHD // P):
                        strip = pw2.tile([P, NL], F32R, tag="ustrip", bufs=2,
                                         name="ustrip")
                        nc.sync.dma_start(strip[:], wqr_ext.ap()[rs * P:(rs + 1) * P, :])
                        for cs in range(NNL):
                            tp = ptp.tile([P, P], F32R, tag="tp", name="tp")
                            nc.tensor.transpose(
                                tp[:], strip[:, cs * P:(cs + 1) * P], ident[:]
                            )
                            for hh in range(2):
                                hloc = rs * 2 + hh
                                ev, od = _deint(tp[:, hh * RHD:(hh + 1) * RHD])
                                base = hloc * RHD
                                nc.scalar.copy(
                                    wqrT[cs][:, base:base + 32], ev
                                )
                                nc.scalar.copy(
                                    wqrT[cs][:, base + 32:base + 64], od
                                )

                    # transpose W_o during the collective window, staged
                    # to DRAM for phase D
                    for sp in range(C // P // 2):
                        osA = pw.tile([P, HLOC * HS], F32R, tag="osA",
                                      bufs=1, name="osA")
                        osB = pw.tile([P, HLOC * HS], F32R, tag="osB",
                                      bufs=1, name="osB")
                        nc.sync.dma_start(
                            osA[:],
                            wo_ext.ap()[2 * sp * P:(2 * sp + 1) * P, :],
                        )
                        nc.sync.dma_start(
                            osB[:],
                            wo_ext.ap()[(2 * sp + 1) * P:(2 * sp + 2) * P, :],
                        )
                        for fs in range(HLOC):
                            tp2 = ptp.tile([P, 2 * P], F32R, tag="tp",
                                           name="tp")
                            nc.tensor.transpose(
                                tp2[:, 0:P], osA[:, fs * P:(fs + 1) * P],
                                ident[:],
                            )
                            nc.tensor.transpose(
                                tp2[:, P:2 * P], osB[:, fs * P:(fs + 1) * P],
                                ident[:],
                            )
                            wob = pw.tile([P, 2 * P], BF16, tag="wob",
                                          bufs=2, name="wob")
                            nc.scalar.copy(wob[:], tp2[:])
                            nc.sync.dma_start(
                                woT_dram.ap()[fs, :,
                                              2 * sp * P:(2 * sp + 2) * P],
                                wob[:],
                            )

                    # unpack gathered latents into [feat, T] layout
                    for ch in range(NCH):
                        sl = slice(ch * TCH, (ch + 1) * TCH)
                        for nl in range(NNL):
                            nc.sync.dma_start(
                                cqT[nl][:, sl],
                                agout_dram.ap()[ch, nl * P:(nl + 1) * P, :],
                            )
                            nc.sync.dma_start(
                                ckvT[nl][:, sl],
                                agout_dram.ap()[ch, NL + nl * P:NL + (nl + 1) * P, :],
                            )
                        krg = pa.tile([RHD, TCH], BF16, tag="krg", bufs=2,
                                      name="krg")
                        nc.sync.dma_start(
                            krg[:], agout_dram.ap()[ch, 2 * NL:2 * NL + RHD, :]
                        )
                        tmp = pa.tile([RHD, TCH], F32, tag="rtmp", bufs=1,
                                      name="rtmp")
                        rope(kr, sl, krg[:], tmp, sl)

                for h in range(HLOC):
                    qcT = ph.tile([P, T], F32R, tag="qcT", name="qcT")
                    kcT = ph.tile([P, T], F32R, tag="kcT", name="kcT")
                    qr = ph.tile([RHD, T], F32R, tag="qr", name="qr")
                    vv = ph.tile([P, T], F32R, tag="vv", name="vv")
                    hs = slice(h * P, (h + 1) * P)
                    for ch in range(NCH):
                        sl = slice(ch * TCH, (ch + 1) * TCH)
                        for wT, srcT, dst in (
                            (wuqT, cqT, qcT),
                            (wukT, ckvT, kcT),
                        ):
                            acc = pmm.tile([P, TCH], F32, tag="mm", name="mm")
                            for nl in range(NNL):
                                nc.tensor.matmul(
                                    acc[:],
                                    wT[nl][:, hs],
                                    srcT[nl][:, sl],
                                    start=(nl == 0),
                                    stop=(nl == NNL - 1),
                                )
                            nc.vector.tensor_copy(dst[:, sl], acc[:])
                        # q_r raw + rope
                        acc = pmm.tile([RHD, TCH], F32, tag="mm", name="mm")
                        for nl in range(NNL):
                            nc.tensor.matmul(
                                acc[:],
                                wqrT[nl][:, h * RHD:(h + 1) * RHD],
                                cqT[nl][:, sl],
                                start=(nl == 0),
                                stop=(nl == NNL - 1),
                            )
                        tmp = ph.tile([RHD, TCH], F32, tag="rtmp2", name="rtmp2")
                        rope(qr, sl, acc[:], tmp, sl)
                    # v: compute v^T [hs, t] then PE-transpose to natural
                    for ch in range(NCH):
                        sl = slice(ch * TCH, (ch + 1) * TCH)
                        acc = pmm.tile([P, TCH], F32, tag="mm", name="mm")
                        for nl in range(NNL):
                            nc.tensor.matmul(
                                acc[:],
                                wuvT[nl][:, hs],
                                ckvT[nl][:, sl],
                                start=(nl == 0),
                                stop=(nl == NNL - 1),
                            )
                        vts = ph.tile([P, TCH], F32R, tag="vts", bufs=2,
                                      name="vts")
                        nc.scalar.copy(vts[:], acc[:])
                        for sp in range(2):
                            tt = ch * 4 + 2 * sp
                            transpose_pair_into(
                                vv[:, tt * P:(tt + 2) * P],
                                vts[:, 2 * sp * P:(2 * sp + 1) * P],
                                vts[:, (2 * sp + 1) * P:(2 * sp + 2) * P],
                                eng="act",
                            )

                    # ---- causal attention for this head ----
                    for tq in range(NCH):
                        outU = pou.tile([P, TCH], F32, tag="ou", name="ou")
                        den = pou.tile([1, TCH], F32, tag="de", name="de")
                        nkc = tq + 1
                        qsl = slice(tq * TCH, (tq + 1) * TCH)
                        for kc in range(nkc):
                            for ks in range(4):
                                kt = kc * 4 + ks
                                k0 = kt * P
                                ST = pmm.tile([P, TCH], F32, tag="mm",
                                              name="mm")
                                nc.tensor.matmul(
                                    ST[:],
                                    kcT[:, k0:k0 + P],
                                    qcT[:, qsl],
                                    start=True,
                                    stop=False,
                                )
                                nc.tensor.matmul(
                                    ST[:],
                                    kr[:, k0:k0 + P],
                                    qr[:, qsl],
                                    start=False,
                                    stop=True,
                                )
                                if kc == tq:
                                    off = 384 - ks * P
                                    nc.vector.tensor_add(
                                        ST[:], ST[:],
                                        maskbuf[:, off:off + TCH],
                                    )
                                Pt = pat.tile([P, TCH], F32R, tag="pt",
                                              bufs=6, name="pt")
                                nc.scalar.activation(Pt[:], ST[:], Exp,
                                                     scale=SCALE)
                                last = kc == nkc - 1 and ks == 3
                                first = kc == 0 and ks == 0
                                nc.tensor.matmul(
                                    den[:],
                                    onesb[:, 0:1],
                                    Pt[:],
                                    start=first,
                                    stop=last,
                                    skip_group_check=True,
                                )
                                nc.tensor.matmul(
                                    outU[:],
                                    vv[:, k0:k0 + P],
                                    Pt[:],
                                    start=first,
                                    stop=last,
                                    skip_group_check=True,
                                )
                        recip = pat.tile([1, TCH], F32, tag="rc", name="rc")
                        nc.vector.reciprocal(recip[:], den[:])
                        recipr = pat.tile([1, TCH], F32R, tag="rcr", name="rcr")
                        nc.vector.tensor_copy(recipr[:], recip[:])
                        bc = pmm.tile([P, TCH], F32, tag="mm", name="mm")
                        nc.tensor.matmul(
                            bc[:], onesb[0:1, :], recipr[:],
                            start=True, stop=True,
                        )
                        bc_sb = pat.tile([P, TCH], F32, tag="bcs", bufs=2,
                                         name="bcs")
                        nc.scalar.activation(bc_sb[:], bc[:], Copy)
                        oh = pat.tile([P, TCH], BF16, tag="oh", bufs=2,
                                      name="oh")
                        nc.vector.tensor_mul(oh[:], outU[:], bc_sb[:])
                        nc.sync.dma_start(
                            ahT_dram.ap()[h, :, tq * TCH:(tq + 1) * TCH], oh[:]
                        )

            # ---------------- phase D: output projection --------------------
            with tc.tile_pool(name="pd", bufs=1) as pd:
                woT = [pd.tile([P, C], BF16, tag=f"woT{i}", name=f"woT{i}")
                       for i in range(HLOC)]
                for fs in range(HLOC):
                    nc.sync.dma_start(woT[fs][:], woT_dram.ap()[fs])
                for tq in range(NCH):
                    ah = []
                    for h in range(HLOC):
                        t = pd.tile([P, TCH], BF16, tag=f"ah{h}", bufs=2,
                                    name=f"ah{h}")
                        nc.sync.dma_start(
                            t[:], ahT_dram.ap()[h, :, tq * TCH:(tq + 1) * TCH]
                        )
                        ah.append(t)
                    for cs in range(C // P):
                        acc = pmm.tile([P, TCH], F32, tag="mm", name="mm")
                        for h in range(HLOC):
                            nc.tensor.matmul(
                                acc[:],
                                woT[h][:, cs * P:(cs + 1) * P],
                                ah[h][:],
                                start=(h == 0),
                                stop=(h == HLOC - 1),
                            )
                        ot = pd.tile([P, TCH], F32, tag="ot", bufs=3, name="ot")
                        nc.scalar.copy(ot[:], acc[:])
                        nc.sync.dma_start(
                            out_ext.ap()[cs * P:(cs + 1) * P,
                                         tq * TCH:(tq + 1) * TCH],
                            ot[:],
                        )

    nc.compile()
    return nc


def _get_nc():
    if "nc" not in _NC_CACHE:
        _NC_CACHE["nc"] = build()
    return _NC_CACHE["nc"]


def kernel(x, freqs_cos, freqs_sin, W_dq, W_uq, W_dkv, W_uk, W_uv, W_qr, W_kr,
           W_o, trace=False, **trace_kwargs):
    nc = _get_nc()
    f32 = lambda a: np.ascontiguousarray(np.asarray(a, dtype=np.float32))
    x = f32(x); W_dq = f32(W_dq); W_uq = f32(W_uq); W_dkv = f32(W_dkv)
    W_uk = f32(W_uk); W_uv = f32(W_uv); W_qr = f32(W_qr); W_kr = f32(W_kr)
    W_o = f32(W_o)
    cos = f32(freqs_cos); sin = f32(freqs_sin)

    in_maps = []
    for c in range(8):
        b, r = divmod(c, 4)
        in_maps.append({
            "x": x[b, r * TCH:(r + 1) * TCH],
            "wdq": W_dq, "wdkv": W_dkv, "wkr": W_kr,
            "wuq": W_uq[r * HLOC * HS:(r + 1) * HLOC * HS],
            "wuk": W_uk[r * HLOC * HS:(r + 1) * HLOC * HS],
            "wuv": W_uv[r * HLOC * HS:(r + 1) * HLOC * HS],
            "wqr": W_qr[r * HLOC * RHD:(r + 1) * HLOC * RHD],
            "wo": W_o[:, r * HLOC * HS:(r + 1) * HLOC * HS],
            "cos": cos, "sin": sin,
        })
    res = run_bass_kernel_spmd(nc, in_maps, core_ids=list(range(8)),
                               trace=trace, **trace_kwargs)
    out = np.zeros((B, T, C), dtype=np.float32)
    for c in range(8):
        b = c // 4
        out[b] += res.results[c]["out"].T
    kernel.last_result = res
    return out



# revision 12
# speedup vs baseline: 1.1616x; 1.1616x over previous
"""MLA-style attention kernel for 8 TRN2 NeuronCores, v2.

Sharding: core c -> batch b = c//4, heads r*4..r*4+3 where r = c%4.
The latent down-projections are REPLICATED within each 4-core batch
group (no collective, no cross-core dependency): each core computes the
full-T latents c_q/c_kv/k_r from the full x[b], then its 4 heads'
attention and a partial output projection summed on the host.

All activations stay SBUF-resident in a transposed [feature, T] layout.
Down/up-projections run in bf16 (PSUM fp32 accumulate).  Scores use
fp8e4 with DoubleRow perf mode: q/k packed as [128, 2, T] fp8 where
slot 0 holds the 128 content dims and slot 1 rows 0:64 hold the roped
rope dims (planar re/im), rows 64:128 zero.  One DoubleRow matmul per
512x128 score subtile (4x fewer PE cycles than two f32r matmuls).
Softmax denominators use N=1 ones-column matmuls accumulating into a
[128q, 4] PSUM tile (nearly free on PE), transposed + reciprocal +
rank-1 broadcast matmul for the final normalization.  The diagonal
causal blocks shrink the score matmul to the valid q-range, memset the
dead Pt columns, and apply a fixed 128-wide triangular mask.
exp() pipelines two subtiles deep so PE never waits on the Act engine.
V is produced directly in natural [t, hs] layout (lhsT = ckv^T), so no
PE transposes are needed after phase A.  Attention is processed
tq-major (all 4 heads per 512-query chunk) so the output projection
and its DMA interleave with Act-bound attention work.
"""
import math
from collections import deque
import numpy as np

import concourse.bass as bass
import concourse.bacc as bacc
import concourse.mybir as mybir
import concourse.tile as tile
from concourse.bass_utils import run_bass_kernel_spmd

F32 = mybir.dt.float32
F32R = mybir.dt.float32r
BF16 = mybir.dt.bfloat16
F8 = mybir.dt.float8e4
Exp = mybir.ActivationFunctionType.Exp
DR = mybir.MatmulPerfMode.DoubleRow

B, T, C = 2, 2048, 2048
H = 16
HS = 128
NL = 512
RHD = 64
HLOC = 4              # heads per core
P = 128
NNL = NL // P         # 4
NCT = C // P          # 16
TCH = 512
NCH = T // TCH        # 4
SCALE = 1.0 / math.sqrt(HS + RHD)
NEG = -1.0e30

_NC_CACHE = {}


def _deint(ap2d):
    # [p, 2d] -> (evens [p, d], odds [p, d]) along the free dim
    rr = ap2d.rearrange("p (d two) -> p two d", two=2)
    return rr[:, 0, :], rr[:, 1, :]


def build():
    nc = bacc.Bacc("TRN2", target_bir_lowering=False, debug=False, num_devices=8)

    x_ext = nc.dram_tensor("x", [T, C], F32R, kind="ExternalInput")
    wdq_ext = nc.dram_tensor("wdq", [NL, C], F32R, kind="ExternalInput")
    wdkv_ext = nc.dram_tensor("wdkv", [NL, C], F32R, kind="ExternalInput")
    wkr_ext = nc.dram_tensor("wkr", [RHD, C], F32R, kind="ExternalInput")
    wuq_ext = nc.dram_tensor("wuq", [HLOC * HS, NL], F32R, kind="ExternalInput")
    wuk_ext = nc.dram_tensor("wuk", [HLOC * HS, NL], F32R, kind="ExternalInput")
    wuv_ext = nc.dram_tensor("wuv", [HLOC * HS, NL], F32R, kind="ExternalInput")
    wqr_ext = nc.dram_tensor("wqr", [HLOC * RHD, NL], F32R, kind="ExternalInput")
    wo_ext = nc.dram_tensor("wo", [C, HLOC * HS], F32R, kind="ExternalInput")
    cos_ext = nc.dram_tensor("cos", [T, RHD // 2], F32R, kind="ExternalInput")
    sin_ext = nc.dram_tensor("sin", [T, RHD // 2], F32R, kind="ExternalInput")
    out_ext = nc.dram_tensor("out", [C, T], F32, kind="ExternalOutput")

    ident_dram = nc.inline_tensor(np.eye(P, dtype=np.float32), name="identc")
    ones_dram = nc.inline_tensor(np.ones((P, P), dtype=np.float32), name="onesc")
    # triangular mask for the 128-wide diagonal band of shrunk S^T tiles:
    # m[jj, u] = 0 if u >= jj else -1e30 taken from cols 384:512 of the
    # baseline 896-wide sliding mask (kept full-width for generality).
    m = np.full((P, 896), NEG, dtype=np.float32)
    for jj in range(P):
        m[jj, 384 + jj:] = 0.0
    masks_dram = nc.inline_tensor(m, name="maskc")
    # row selector for the 1/den broadcast: sel4[k, qq*128+j] = (k == qq)
    sel = np.zeros((4, 512), dtype=np.float32)
    for qq in range(4):
        sel[qq, qq * P:(qq + 1) * P] = 1.0
    sel4_dram = nc.inline_tensor(sel, name="sel4c")

    with tile.TileContext(nc) as tc:
        with (
            tc.tile_pool(name="pers", bufs=1) as pers,
            tc.tile_pool(name="ptp", bufs=2, space="PSUM") as ptp,
        ):
            ident = pers.tile([P, P], F32R, tag="ident", name="ident")
            nc.sync.dma_start(ident[:], ident_dram.ap().bitcast(F32R))
            maskbuf = pers.tile([P, 896], BF16, tag="maskbuf", name="maskbuf")
            nc.gpsimd.dma_start(out=maskbuf[:], in_=masks_dram.ap())
            onescol = pers.tile([P, 1], BF16, tag="onescol", name="onescol")
            nc.vector.memset(onescol[:], 1.0)
            sel4f = pers.tile([4, TCH], F32R, tag="sel4f", name="sel4f")
            nc.sync.dma_start(sel4f[:], sel4_dram.ap().bitcast(F32R))
            sel4 = pers.tile([4, TCH], BF16, tag="sel4", name="sel4")
            nc.vector.tensor_copy(sel4[:], sel4f[:])

            # rope tables, cos/sin duplicated on all four 32-row groups
            ca4 = pers.tile([P, T], BF16, tag="ca4", name="ca4")
            sa4 = pers.tile([P, T], BF16, tag="sa4", name="sa4")

            # full-T latents (bf16, [feat, T])
            cqT = [pers.tile([P, T], BF16, tag=f"cqT{i}", name=f"cqT{i}")
                   for i in range(NNL)]
            ckvT = [pers.tile([P, T], BF16, tag=f"ckvT{i}", name=f"ckvT{i}")
                    for i in range(NNL)]
            krraw = pers.tile([RHD, T], BF16, tag="krraw", name="krraw")

            _ecnt = [0]

            def ecopy(dst, src, pin=None):
                """PSUM->SBUF evacuation copy, alternating Act/DVE."""
                _ecnt[0] += 1
                eng = pin if pin else ("act" if _ecnt[0] % 2 else "dve")
                if eng == "act":
                    nc.scalar.copy(dst, src)
                else:
                    nc.vector.tensor_copy(dst, src)

            def transpose_pair_into(dst_ap, srcA, srcB, pin=None):
                tp2 = ptp.tile([P, 2 * P], F32R, tag="tp", name="tp")
                nc.tensor.transpose(tp2[:, 0:P], srcA, ident[:])
                nc.tensor.transpose(tp2[:, P:2 * P], srcB, ident[:])
                ecopy(dst_ap, tp2[:], pin=pin)

            # ---------------- phase A: weights prep + down-projection -------
            with (
                tc.tile_pool(name="pa", bufs=1) as pa,
                tc.tile_pool(name="pacc", bufs=1, space="PSUM") as pacc,
            ):
                # --- W_dq / W_dkv transposed to [c, nl] bf16 tiles
                wdqT = [pa.tile([P, NL], BF16, tag=f"wdqT{i}", name=f"wdqT{i}")
                        for i in range(NCT)]
                wdkvT = [pa.tile([P, NL], BF16, tag=f"wdkvT{i}", name=f"wdkvT{i}")
                         for i in range(NCT)]
                for w_ext, wTs in ((wdq_ext, wdqT), (wdkv_ext, wdkvT)):
                    for rp in range(NL // P // 2):
                        sA = pa.tile([P, C], F32R, tag="wsA", bufs=2, name="wsA")
                        sB = pa.tile([P, C], F32R, tag="wsB", bufs=2, name="wsB")
                        nc.sync.dma_start(
                            sA[:], w_ext.ap()[2 * rp * P:(2 * rp + 1) * P, :])
                        nc.sync.dma_start(
                            sB[:], w_ext.ap()[(2 * rp + 1) * P:(2 * rp + 2) * P, :])
                        for ci in range(NCT):
                            transpose_pair_into(
                                wTs[ci][:, 2 * rp * P:(2 * rp + 2) * P],
                                sA[:, ci * P:(ci + 1) * P],
                                sB[:, ci * P:(ci + 1) * P],
                            )
                # --- W_kr transposed + deinterleaved to planar re/im
                wkrT = [pa.tile([P, RHD], BF16, tag=f"wkrT{i}", name=f"wkrT{i}")
                        for i in range(NCT)]
                kstrip = pa.tile([RHD, C], F32R, tag="kstrip", name="kstrip")
                nc.sync.dma_start(kstrip[:], wkr_ext.ap())
                for ci in range(NCT):
                    tp = ptp.tile([P, 2 * P], F32R, tag="tp", name="tp")
                    nc.tensor.transpose(
                        tp[:, :RHD], kstrip[:, ci * P:(ci + 1) * P],
                        ident[:RHD, :RHD])
                    ev, od = _deint(tp[:, :RHD])
                    nc.scalar.copy(wkrT[ci][:, 0:32], ev)
                    nc.scalar.copy(wkrT[ci][:, 32:64], od)

                # --- cos/sin tables (transpose strips, then duplicate rows)
                for s in range(T // P):
                    cst = pa.tile([P, RHD // 2], F32R, tag="cst", bufs=2,
                                  name="cst")
                    sst = pa.tile([P, RHD // 2], F32R, tag="sst", bufs=2,
                                  name="sst")
                    nc.gpsimd.dma_start(out=cst[:],
                                        in_=cos_ext.ap()[s * P:(s + 1) * P, :])
                    nc.gpsimd.dma_start(out=sst[:],
                                        in_=sin_ext.ap()[s * P:(s + 1) * P, :])
                    tp = ptp.tile([P, 2 * P], F32R, tag="tp", name="tp")
                    nc.tensor.transpose(tp[:32, 0:P], cst[:], ident[:])
                    nc.tensor.transpose(tp[:32, P:2 * P], sst[:], ident[:])
                    nc.vector.tensor_copy(ca4[0:32, s * P:(s + 1) * P],
                                          tp[:32, 0:P])
                    nc.vector.tensor_copy(sa4[0:32, s * P:(s + 1) * P],
                                          tp[:32, P:2 * P])
                for d in range(1, 4):
                    nc.vector.tensor_copy(ca4[32 * d:32 * (d + 1), :], ca4[0:32, :])
                    nc.vector.tensor_copy(sa4[32 * d:32 * (d + 1), :], sa4[0:32, :])

                # --- stream x, transpose, accumulate latents per 512-chunk
                xT = [pa.tile([P, TCH], BF16, tag=f"xT{i}", name=f"xT{i}")
                      for i in range(NCT)]
                for tch in range(NCH):
                    t0 = tch * TCH
                    for sp in range(2):
                        xA = pa.tile([P, C], F32R, tag="xA", bufs=2, name="xA")
                        xB = pa.tile([P, C], F32R, tag="xB", bufs=2, name="xB")
                        nc.sync.dma_start(
                            xA[:], x_ext.ap()[t0 + 2 * sp * P:t0 + (2 * sp + 1) * P, :])
                        nc.sync.dma_start(
                            xB[:], x_ext.ap()[t0 + (2 * sp + 1) * P:t0 + (2 * sp + 2) * P, :])
                        for ci in range(NCT):
                            transpose_pair_into(
                                xT[ci][:, 2 * sp * P:(2 * sp + 2) * P],
                                xA[:, ci * P:(ci + 1) * P],
                                xB[:, ci * P:(ci + 1) * P],
                            )
                    for wTs, dstT in ((wdqT, cqT), (wdkvT, ckvT)):
                        for j in range(NNL):
                            acc = pacc.tile([P, TCH], F32, tag=f"acc{j}",
                                            name=f"acc{j}")
                            for ci in range(NCT):
                                nc.tensor.matmul(
                                    acc[:],
                                    wTs[ci][:, j * P:(j + 1) * P],
                                    xT[ci][:],
                                    start=(ci == 0),
                                    stop=(ci == NCT - 1),
                                )
                            ecopy(dstT[j][:, t0:t0 + TCH], acc[:])
                    acck = pacc.tile([RHD, TCH], F32, tag="acck", name="acck")
                    for ci in range(NCT):
                        nc.tensor.matmul(
                            acck[:],
                            wkrT[ci][:],
                            xT[ci][:],
                            start=(ci == 0),
                            stop=(ci == NCT - 1),
                        )
                    ecopy(krraw[:, t0:t0 + TCH], acck[:], pin="act")

            # ---------------- phase B: up-proj weights + per-head tiles -----
            with tc.tile_pool(name="pb", bufs=1) as pb:
                # W_uq / W_uk / W_uv transposed to [nl, 4*128] bf16
                wuqT = [pb.tile([P, HLOC * HS], BF16, tag=f"wuqT{i}",
                                name=f"wuqT{i}") for i in range(NNL)]
                wukT = [pb.tile([P, HLOC * HS], BF16, tag=f"wukT{i}",
                                name=f"wukT{i}") for i in range(NNL)]
                wuvT = [pb.tile([P, HLOC * HS], BF16, tag=f"wuvT{i}",
                                name=f"wuvT{i}") for i in range(NNL)]
                for w_ext, wT in ((wuq_ext, wuqT), (wuk_ext, wukT),
                                  (wuv_ext, wuvT)):
                    for rp in range(HLOC * HS // P // 2):
                        sA = pb.tile([P, NL], F32R, tag="usA", bufs=2, name="usA")
                        sB = pb.tile([P, NL], F32R, tag="usB", bufs=2, name="usB")
                        nc.gpsimd.dma_start(
                            out=sA[:], in_=w_ext.ap()[2 * rp * P:(2 * rp + 1) * P, :])
                        nc.gpsimd.dma_start(
                            out=sB[:], in_=w_ext.ap()[(2 * rp + 1) * P:(2 * rp + 2) * P, :])
                        for cs in range(NNL):
                            transpose_pair_into(
                                wT[cs][:, 2 * rp * P:(2 * rp + 2) * P],
                                sA[:, cs * P:(cs + 1) * P],
                                sB[:, cs * P:(cs + 1) * P],
                            )
                # W_qr per head-pair, columns reordered [Are, Bre, Aim, Bim]
                wqrT = [[pb.tile([P, P], BF16, tag=f"wqrT{g}{i}",
                                 name=f"wqrT{g}{i}") for i in range(NNL)]
                        for g in range(2)]
                for g in range(2):
                    strip = pb.tile([P, NL], F32R, tag="qrs", bufs=2, name="qrs")
                    nc.gpsimd.dma_start(
                        out=strip[:], in_=wqr_ext.ap()[g * P:(g + 1) * P, :])
                    for cs in range(NNL):
                        tp = ptp.tile([P, 2 * P], F32R, tag="tp", name="tp")
                        nc.tensor.transpose(
                            tp[:, 0:P], strip[:, cs * P:(cs + 1) * P], ident[:])
                        evA, odA = _deint(tp[:, 0:RHD])
                        evB, odB = _deint(tp[:, RHD:2 * RHD])
                        nc.scalar.copy(wqrT[g][cs][:, 0:32], evA)
                        nc.scalar.copy(wqrT[g][cs][:, 32:64], evB)
                        nc.scalar.copy(wqrT[g][cs][:, 64:96], odA)
                        nc.scalar.copy(wqrT[g][cs][:, 96:128], odB)
                # W_o transposed to [hs, C] bf16 per head
                woT = [pb.tile([P, C], BF16, tag=f"woT{i}", name=f"woT{i}")
                       for i in range(HLOC)]
                for sp in range(C // P // 2):
                    oA = pb.tile([P, HLOC * HS], F32R, tag="osA", bufs=2,
                                 name="osA")
                    oB = pb.tile([P, HLOC * HS], F32R, tag="osB", bufs=2,
                                 name="osB")
                    nc.gpsimd.dma_start(
                        out=oA[:], in_=wo_ext.ap()[2 * sp * P:(2 * sp + 1) * P, :])
                    nc.gpsimd.dma_start(
                        out=oB[:], in_=wo_ext.ap()[(2 * sp + 1) * P:(2 * sp + 2) * P, :])
                    for fs in range(HLOC):
                        transpose_pair_into(
                            woT[fs][:, 2 * sp * P:(2 * sp + 2) * P],
                            oA[:, fs * P:(fs + 1) * P],
                            oB[:, fs * P:(fs + 1) * P],
                        )

                # --- per-head fp8 q/k packs, natural-layout V, rope ---------
                q8 = [pb.tile([P, 2, T], F8, tag=f"q8{h}", name=f"q8{h}")
                      for h in range(HLOC)]
                k8 = [pb.tile([P, 2, T], F8, tag=f"k8{h}", name=f"k8{h}")
                      for h in range(HLOC)]
                vv = [pb.tile([P, T // P, P], BF16, tag=f"vv{h}", name=f"vv{h}")
                      for h in range(HLOC)]
                for h in range(HLOC):
                    nc.vector.memset(q8[h][64:128, 1, :], 0.0)
                    nc.gpsimd.memset(k8[h][64:128, 1, :], 0.0)

                with (
                    tc.tile_pool(name="pmm", bufs=3, space="PSUM") as pmm,
                    tc.tile_pool(name="pou", bufs=2, space="PSUM") as pou,
                    tc.tile_pool(name="pde", bufs=1, space="PSUM") as pde,
                    tc.tile_pool(name="pat", bufs=1) as pat,
                ):
                    # k_r rope -> krf8 (planar re/im), shared across heads
                    krf8 = pb.tile([RHD, T], F8, tag="krf8", name="krf8")
                    rtmp = pb.tile([P, T], BF16, tag="rtmp", bufs=2, name="rtmp")
                    rro = pb.tile([P, T], BF16, tag="rro", bufs=2, name="rro")
                    nc.vector.tensor_mul(rtmp[0:32, :], krraw[32:64, :], sa4[32:64, :])
                    nc.vector.tensor_mul(rtmp[32:64, :], krraw[32:64, :], ca4[32:64, :])
                    nc.vector.tensor_mul(rro[0:32, :], krraw[0:32, :], ca4[0:32, :])
                    nc.vector.tensor_mul(rro[32:64, :], krraw[0:32, :], sa4[0:32, :])
                    nc.vector.tensor_sub(krf8[0:32, :], rro[0:32, :], rtmp[0:32, :])
                    nc.vector.tensor_add(krf8[32:64, :], rro[32:64, :], rtmp[32:64, :])
                    for h in range(HLOC):
                        nc.gpsimd.dma_start(out=k8[h][0:RHD, 1, :], in_=krf8[:])

                    # up-projections, head-pair at a time
                    for g in range(2):
                        hA, hB = 2 * g, 2 * g + 1
                        qraw = pb.tile([P, T], BF16, tag="qraw", bufs=2,
                                       name="qraw")
                        for ch in range(NCH):
                            sl = slice(ch * TCH, (ch + 1) * TCH)
                            for hh in (hA, hB):
                                for wT, dst in ((wuqT, q8), (wukT, k8)):
                                    acc = pmm.tile([P, TCH], F32, tag="mm",
                                                   name="mm")
                                    for nl in range(NNL):
                                        nc.tensor.matmul(
                                            acc[:],
                                            wT[nl][:, hh * P:(hh + 1) * P],
                                            cqT[nl][:, sl] if wT is wuqT
                                            else ckvT[nl][:, sl],
                                            start=(nl == 0),
                                            stop=(nl == NNL - 1),
                                        )
                                    ecopy(dst[hh][:, 0, sl], acc[:])
                            qacc = pmm.tile([P, TCH], F32, tag="mm", name="mm")
                            for nl in range(NNL):
                                nc.tensor.matmul(
                                    qacc[:],
                                    wqrT[g][nl][:],
                                    cqT[nl][:, sl],
                                    start=(nl == 0),
                                    stop=(nl == NNL - 1),
                                )
                            ecopy(qraw[:, sl], qacc[:], pin="act")
                        # natural-layout V for both heads
                        for hh in (hA, hB):
                            for ts4 in range(T // TCH):
                                vps = pmm.tile([P, TCH], F32, tag="mm", name="mm")
                                for j in range(4):
                                    kt = ts4 * 4 + j
                                    for nl in range(NNL):
                                        nc.tensor.matmul(
                                            vps[:, j * P:(j + 1) * P],
                                            ckvT[nl][:, kt * P:(kt + 1) * P],
                                            wuvT[nl][:, hh * P:(hh + 1) * P],
                                            start=(nl == 0),
                                            stop=(nl == NNL - 1),
                                            skip_group_check=True,
                                        )
                                ecopy(
                                    vv[hh][:, ts4 * 4:(ts4 + 1) * 4, :],
                                    vps[:].rearrange("p (a b) -> p a b", a=4),
                                )
                        # rope for the pair: rows [Are, Bre, Aim, Bim]
                        roq = pb.tile([P, T], F8, tag="roq", bufs=2, name="roq")
                        nc.vector.tensor_mul(rtmp[0:64, :], qraw[64:128, :],
                                             sa4[64:128, :])
                        nc.vector.tensor_mul(rtmp[64:128, :], qraw[64:128, :],
                                             ca4[64:128, :])
                        nc.vector.tensor_mul(rro[0:64, :], qraw[0:64, :],
                                             ca4[0:64, :])
                        nc.vector.tensor_mul(rro[64:128, :], qraw[0:64, :],
                                             sa4[0:64, :])
                        nc.vector.tensor_sub(roq[0:64, :], rro[0:64, :],
                                             rtmp[0:64, :])
                        nc.vector.tensor_add(roq[64:128, :], rro[64:128, :],
                                             rtmp[64:128, :])
                        nc.gpsimd.dma_start(out=q8[hA][0:32, 1, :], in_=roq[0:32, :])
                        nc.gpsimd.dma_start(out=q8[hA][32:64, 1, :], in_=roq[64:96, :])
                        nc.gpsimd.dma_start(out=q8[hB][0:32, 1, :], in_=roq[32:64, :])
                        nc.gpsimd.dma_start(out=q8[hB][32:64, 1, :], in_=roq[96:128, :])

                    # ---------------- attention + fused output projection ---
                    for tq in range(NCH):
                        qsl = slice(tq * TCH, (tq + 1) * TCH)
                        ohs = []
                        for h in range(HLOC):
                            outU = pou.tile([P, TCH], F32, tag="ou", name="ou")
                            den = pde.tile([P, 4], F32, tag="de", name="de")
                            nsub = (tq + 1) * 4
                            pend = deque()

                            # den accumulates via start=False onto memset
                            # zeros: a start=True on one 4-byte column would
                            # re-mark the whole 2KB PSUM zero-region and wipe
                            # the sibling columns' partial sums.
                            nc.vector.memset(den[:], 0.0)

                            def flush_one():
                                Pt, kt = pend.popleft()
                                first = kt == 0
                                last = kt == nsub - 1
                                for qq in range(4):
                                    nc.tensor.matmul(
                                        den[:, qq:qq + 1],
                                        Pt[:, qq * P:(qq + 1) * P],
                                        onescol[:],
                                        start=False,
                                        stop=last,
                                        skip_group_check=True,
                                    )
                                nc.tensor.matmul(
                                    outU[:],
                                    vv[h][:, kt, :],
                                    Pt[:],
                                    start=first,
                                    stop=last,
                                    skip_group_check=True,
                                )

                            for kt in range(nsub):
                                kc, ks = kt // 4, kt % 4
                                diag = kc == tq
                                off = ks * P if diag else 0
                                npr = TCH - off
                                ST = pmm.tile([P, TCH], F32, tag="mm", name="mm")
                                nc.tensor.matmul(
                                    ST[:, 0:npr],
                                    k8[h][:, :, kt * P:(kt + 1) * P],
                                    q8[h][:, :, qsl.start + off:qsl.stop],
                                    perf_mode=DR,
                                    start=True,
                                    stop=True,
                                )
                                if diag:
                                    nc.vector.tensor_add(
                                        ST[:, 0:P], ST[:, 0:P],
                                        maskbuf[:, 384:384 + P],
                                    )
                                Pt = pat.tile([P, TCH], BF16, tag="pt", bufs=6,
                                              name="pt")
                                if off:
                                    nc.vector.memset(Pt[:, 0:off], 0.0)
                                nc.scalar.activation(Pt[:, off:TCH], ST[:, 0:npr],
                                                     Exp, scale=SCALE)
                                pend.append((Pt, kt))
                                if len(pend) > 2:
                                    flush_one()
                            while pend:
                                flush_one()

                            # normalization: 1/den broadcast over q columns
                            den_sb = pat.tile([P, 4], F32R, tag="dsb", bufs=2,
                                              name="dsb")
                            nc.vector.tensor_copy(den_sb[:], den[:])
                            tpd = ptp.tile([P, 2 * P], F32R, tag="tp", name="tp")
                            nc.tensor.transpose(tpd[0:4, 0:P], den_sb[:], ident[:])
                            rec = pat.tile([4, P], F32, tag="rec", bufs=2,
                                           name="rec")
                            nc.vector.reciprocal(rec[:], tpd[0:4, 0:P])
                            recb = pat.tile([4, P], BF16, tag="recb", bufs=2,
                                            name="recb")
                            nc.vector.tensor_copy(recb[:], rec[:])
                            bc = pmm.tile([P, TCH], F32, tag="mm", name="mm")
                            for qq in range(4):
                                nc.tensor.matmul(
                                    bc[:, qq * P:(qq + 1) * P],
                                    sel4[:, qq * P:(qq + 1) * P],
                                    recb[:],
                                    start=True, stop=True,
                                    skip_group_check=True,
                                )
                            bcs = pat.tile([P, TCH], BF16, tag="bcs", bufs=2,
                                           name="bcs")
                            nc.scalar.copy(bcs[:], bc[:])
                            oh = pat.tile([P, TCH], BF16, tag=f"oh{h}", bufs=2,
                                          name=f"oh{h}")
                            nc.vector.tensor_mul(oh[:], outU[:], bcs[:])
                            ohs.append(oh)

                        # output projection for this tq chunk
                        for cs in range(NCT):
                            acc = pmm.tile([P, TCH], F32, tag="mm", name="mm")
                            for h2 in range(HLOC):
                                nc.tensor.matmul(
                                    acc[:],
                                    woT[h2][:, cs * P:(cs + 1) * P],
                                    ohs[h2][:],
                                    start=(h2 == 0),
                                    stop=(h2 == HLOC - 1),
                                )
                            ot = pat.tile([P, TCH], F32, tag="ot", bufs=3,
                                          name="ot")
                            ecopy(ot[:], acc[:])
                            nc.sync.dma_start(
                                out_ext.ap()[cs * P:(cs + 1) * P, qsl], ot[:])

    nc.compile()
    return nc


def _get_nc():
    if "nc" not in _NC_CACHE:
        _NC_CACHE["nc"] = build()
    return _NC_CACHE["nc"]


def kernel(x, freqs_cos, freqs_sin, W_dq, W_uq, W_dkv, W_uk, W_uv, W_qr, W_kr,
           W_o, trace=False, **trace_kwargs):
    nc = _get_nc()
    f32 = lambda a: np.ascontiguousarray(np.asarray(a, dtype=np.float32))
    x = f32(x); W_dq = f32(W_dq); W_uq = f32(W_uq); W_dkv = f32(W_dkv)
    W_uk = f32(W_uk); W_uv = f32(W_uv); W_qr = f32(W_qr); W_kr = f32(W_kr)
    W_o = f32(W_o)
    cos = f32(freqs_cos); sin = f32(freqs_sin)

    in_maps = []
    for c in range(8):
        b, r = divmod(c, 4)
        in_maps.append({
            "x": x[b],
            "wdq": W_dq, "wdkv": W_dkv, "wkr": W_kr,
            "wuq": W_uq[r * HLOC * HS:(r + 1) * HLOC * HS],
            "wuk": W_uk[r * HLOC * HS:(r + 1) * HLOC * HS],
            "wuv": W_uv[r * HLOC * HS:(r + 1) * HLOC * HS],
            "wqr": W_qr[r * HLOC * RHD:(r + 1) * HLOC * RHD],
            "wo": W_o[:, r * HLOC * HS:(r + 1) * HLOC * HS],
            "cos": cos, "sin": sin,
        })
    res = run_bass_kernel_spmd(nc, in_maps, core_ids=list(range(8)),
                               trace=trace, **trace_kwargs)
    out = np.zeros((B, T, C), dtype=np.float32)
    for c in range(8):
        b = c // 4
        out[b] += res.results[c]["out"].T
    kernel.last_result = res
    return out


# revision 25
# speedup vs baseline: 1.2919x; 1.1121x over previous
"""MLA-style attention kernel for 8 TRN2 NeuronCores, v3.

Sharding: core c -> batch b = c//4, heads r*4..r*4+3 where r = c%4.
The latent down-projections are REPLICATED within each 4-core batch
group (no collective, no cross-core dependency): each core computes the
full-T latents c_q/c_kv/k_r from the full x[b], then its 4 heads'
attention and a partial output projection summed on the host.

All activations stay SBUF-resident in a transposed [feature, T] layout.
Down/up-projections run in bf16 (PSUM fp32 accumulate).  Scores use
fp8e4 with DoubleRow perf mode: q/k packed as [128, 2, T] fp8 where
slot 0 holds the 128 content dims and slot 1 rows 0:64 hold the roped
rope dims (planar re/im), rows 64:128 zero.  One DoubleRow matmul per
512x128 score subtile (4x fewer PE cycles than two f32r matmuls).
Softmax denominators use N=1 ones-column matmuls accumulating into a
[128q, 4] PSUM tile (start=False onto memset zeros -- a start=True
would wipe sibling columns through the 2KB zero-region), then
transpose + reciprocal + selector broadcast matmul for normalization.
Diagonal causal blocks shrink the score matmul to the valid q-range,
memset the dead Pt columns, and add a fixed 128-wide triangular mask.
exp() pipelines two subtiles deep; the normalization tail of head h and
the output projection of chunk tq-1 are emitted under the NEXT head's
score loop so PE never drains while Act works.  V is produced directly
in natural [t, hs] layout (lhsT = ckv^T): no transposes after phase A.
Both hardware DMA queues are used: SP for W_dq/W_dkv/x/output, Act for
cos/sin, W_u/W_qr/W_o, mask, and the SBUF-to-SBUF fp8 slot copies.
"""
import math
from collections import deque
import numpy as np

import concourse.bass as bass
import concourse.bacc as bacc
import concourse.mybir as mybir
import concourse.tile as tile
from concourse.bass_utils import run_bass_kernel_spmd

F32 = mybir.dt.float32
F32R = mybir.dt.float32r
BF16 = mybir.dt.bfloat16
F8 = mybir.dt.float8e4
Exp = mybir.ActivationFunctionType.Exp
DR = mybir.MatmulPerfMode.DoubleRow

B, T, C = 2, 2048, 2048
H = 16
HS = 128
NL = 512
RHD = 64
HLOC = 4              # heads per core
P = 128
NNL = NL // P         # 4
NCT = C // P          # 16
TCH = 512
NCH = T // TCH        # 4
SCALE = 1.0 / math.sqrt(HS + RHD)
NEG = -1.0e30

_NC_CACHE = {}


def _deint(ap2d):
    # [p, 2d] -> (evens [p, d], odds [p, d]) along the free dim
    rr = ap2d.rearrange("p (d two) -> p two d", two=2)
    return rr[:, 0, :], rr[:, 1, :]


def build():
    nc = bacc.Bacc("TRN2", target_bir_lowering=False, debug=False, num_devices=8)

    x_ext = nc.dram_tensor("x", [T, C], F32R, kind="ExternalInput")
    wdq_ext = nc.dram_tensor("wdq", [NL, C], F32R, kind="ExternalInput")
    wdkv_ext = nc.dram_tensor("wdkv", [NL, C], F32R, kind="ExternalInput")
    wkr_ext = nc.dram_tensor("wkr", [RHD, C], F32R, kind="ExternalInput")
    wuq_ext = nc.dram_tensor("wuq", [HLOC * HS, NL], F32R, kind="ExternalInput")
    wuk_ext = nc.dram_tensor("wuk", [HLOC * HS, NL], F32R, kind="ExternalInput")
    wuv_ext = nc.dram_tensor("wuv", [HLOC * HS, NL], F32R, kind="ExternalInput")
    wqr_ext = nc.dram_tensor("wqr", [HLOC * RHD, NL], F32R, kind="ExternalInput")
    wo_ext = nc.dram_tensor("wo", [C, HLOC * HS], F32R, kind="ExternalInput")
    cos_ext = nc.dram_tensor("cos", [T, RHD // 2], F32R, kind="ExternalInput")
    sin_ext = nc.dram_tensor("sin", [T, RHD // 2], F32R, kind="ExternalInput")
    out_ext = nc.dram_tensor("out", [C, T], F32, kind="ExternalOutput")

    ident_dram = nc.inline_tensor(np.eye(P, dtype=np.float32), name="identc")
    # triangular mask for the 128-wide diagonal band of shrunk S^T tiles
    m = np.full((P, 896), NEG, dtype=np.float32)
    for jj in range(P):
        m[jj, 384 + jj:] = 0.0
    masks_dram = nc.inline_tensor(m, name="maskc")
    # row selector for the 1/den broadcast: sel4[k, qq*128+j] = (k == qq)
    sel = np.zeros((4, 512), dtype=np.float32)
    for qq in range(4):
        sel[qq, qq * P:(qq + 1) * P] = 1.0
    sel4_dram = nc.inline_tensor(sel, name="sel4c")

    with tile.TileContext(nc) as tc:
        with (
            tc.tile_pool(name="pers", bufs=1) as pers,
            tc.tile_pool(name="ptp", bufs=2, space="PSUM") as ptp,
        ):
            ident = pers.tile([P, P], F32R, tag="ident", name="ident")
            nc.sync.dma_start(ident[:], ident_dram.ap().bitcast(F32R))
            maskbuf = pers.tile([P, 896], BF16, tag="maskbuf", name="maskbuf")
            nc.gpsimd.dma_start(out=maskbuf[:], in_=masks_dram.ap())
            onescol = pers.tile([P, 1], BF16, tag="onescol", name="onescol")
            nc.vector.memset(onescol[:], 1.0)
            sel4 = pers.tile([4, TCH], BF16, tag="sel4", name="sel4")
            nc.gpsimd.dma_start(out=sel4[:], in_=sel4_dram.ap())

            # rope tables, cos/sin duplicated on all four 32-row groups
            ca4 = pers.tile([P, T], BF16, tag="ca4", name="ca4")
            sa4 = pers.tile([P, T], BF16, tag="sa4", name="sa4")

            # full-T latents (bf16, [feat, T])
            cqT = [pers.tile([P, T], BF16, tag=f"cqT{i}", name=f"cqT{i}")
                   for i in range(NNL)]
            ckvT = [pers.tile([P, T], BF16, tag=f"ckvT{i}", name=f"ckvT{i}")
                    for i in range(NNL)]
            krraw = pers.tile([RHD, T], BF16, tag="krraw", name="krraw")

            _ecnt = [0]

            def ecopy(dst, src, pin=None):
                """PSUM->SBUF evacuation copy, alternating Act/DVE."""
                _ecnt[0] += 1
                eng = pin if pin else ("act" if _ecnt[0] % 2 else "dve")
                if eng == "act":
                    nc.scalar.copy(dst, src)
                else:
                    nc.vector.tensor_copy(dst, src)

            def transpose_pair_into(dst_ap, srcA, srcB, pin=None):
                tp2 = ptp.tile([P, 2 * P], F32R, tag="tp", name="tp")
                nc.tensor.transpose(tp2[:, 0:P], srcA, ident[:])
                nc.tensor.transpose(tp2[:, P:2 * P], srcB, ident[:])
                ecopy(dst_ap, tp2[:], pin=pin)

            # ============ phase A: x chunks + all weight prep, interleaved ==
            with tc.tile_pool(name="pb", bufs=1) as pb:
                # -- persistent-ish weight destinations (pb outlives phase A)
                wuqT = [pb.tile([P, HLOC * HS], BF16, tag=f"wuqT{i}",
                                name=f"wuqT{i}") for i in range(NNL)]
                wukT = [pb.tile([P, HLOC * HS], BF16, tag=f"wukT{i}",
                                name=f"wukT{i}") for i in range(NNL)]
                wuvT = [pb.tile([P, HLOC * HS], BF16, tag=f"wuvT{i}",
                                name=f"wuvT{i}") for i in range(NNL)]
                wqrT = [[pb.tile([P, P], BF16, tag=f"wqrT{g}{i}",
                                 name=f"wqrT{g}{i}") for i in range(NNL)]
                        for g in range(2)]
                woT = [pb.tile([P, C], BF16, tag=f"woT{i}", name=f"woT{i}")
                       for i in range(HLOC)]

                pa_ctx = (
                    tc.tile_pool(name="pa", bufs=1),
                    tc.tile_pool(name="pacc", bufs=1, space="PSUM"),
                )
                pa = pa_ctx[0].__enter__()
                pacc = pa_ctx[1].__enter__()

                wdqT = [pa.tile([P, NL], BF16, tag=f"wdqT{i}", name=f"wdqT{i}")
                        for i in range(NCT)]
                wdkvT = [pa.tile([P, NL], BF16, tag=f"wdkvT{i}",
                                 name=f"wdkvT{i}") for i in range(NCT)]
                wkrT = [pa.tile([P, RHD], BF16, tag=f"wkrT{i}", name=f"wkrT{i}")
                        for i in range(NCT)]
                xT = [pa.tile([P, TCH], BF16, tag=f"xT{i}", name=f"xT{i}")
                      for i in range(NCT)]

                def x_chunk_transpose(tch):
                    t0 = tch * TCH
                    for sp in range(2):
                        rA = slice(t0 + 2 * sp * P, t0 + (2 * sp + 1) * P)
                        rB = slice(t0 + (2 * sp + 1) * P, t0 + (2 * sp + 2) * P)
                        for hf in range(2):
                            cf = slice(hf * (C // 2), (hf + 1) * (C // 2))
                            xA = pa.tile([P, C // 2], F32R, tag="xA", bufs=2,
                                         name="xA")
                            xB = pa.tile([P, C // 2], F32R, tag="xB", bufs=2,
                                         name="xB")
                            nc.sync.dma_start(xA[:], x_ext.ap()[rA, cf])
                            nc.sync.dma_start(xB[:], x_ext.ap()[rB, cf])
                            for ci in range(NCT // 2):
                                transpose_pair_into(
                                    xT[hf * 8 + ci][:, 2 * sp * P:(2 * sp + 2) * P],
                                    xA[:, ci * P:(ci + 1) * P],
                                    xB[:, ci * P:(ci + 1) * P],
                                )

                def x_chunk_matmuls(tch):
                    t0 = tch * TCH
                    for wTs, dstT in ((wdqT, cqT), (wdkvT, ckvT)):
                        for j in range(NNL):
                            acc = pacc.tile([P, TCH], F32, tag=f"acc{j}",
                                            name=f"acc{j}")
                            for ci in range(NCT):
                                nc.tensor.matmul(
                                    acc[:],
                                    wTs[ci][:, j * P:(j + 1) * P],
                                    xT[ci][:],
                                    start=(ci == 0),
                                    stop=(ci == NCT - 1),
                                )
                            ecopy(dstT[j][:, t0:t0 + TCH], acc[:])
                    acck = pacc.tile([RHD, TCH], F32, tag="acck", name="acck")
                    for ci in range(NCT):
                        nc.tensor.matmul(
                            acck[:],
                            wkrT[ci][:],
                            xT[ci][:],
                            start=(ci == 0),
                            stop=(ci == NCT - 1),
                        )
                    ecopy(krraw[:, t0:t0 + TCH], acck[:], pin="act")

                def wd_prep():
                    for w_ext, wTs in ((wdq_ext, wdqT), (wdkv_ext, wdkvT)):
                        for rp in range(NL // P // 2):
                            rA = slice(2 * rp * P, (2 * rp + 1) * P)
                            rB = slice((2 * rp + 1) * P, (2 * rp + 2) * P)
                            for hf in range(2):
                                cf = slice(hf * (C // 2), (hf + 1) * (C // 2))
                                sA = pa.tile([P, C // 2], F32R, tag="wsA",
                                             bufs=2, name="wsA")
                                sB = pa.tile([P, C // 2], F32R, tag="wsB",
                                             bufs=2, name="wsB")
                                nc.sync.dma_start(sA[:], w_ext.ap()[rA, cf])
                                nc.sync.dma_start(sB[:], w_ext.ap()[rB, cf])
                                for ci in range(NCT // 2):
                                    transpose_pair_into(
                                        wTs[hf * 8 + ci][:, 2 * rp * P:(2 * rp + 2) * P],
                                        sA[:, ci * P:(ci + 1) * P],
                                        sB[:, ci * P:(ci + 1) * P],
                                    )
                    kstrip = pa.tile([RHD, C], F32R, tag="kstrip", name="kstrip")
                    nc.sync.dma_start(kstrip[:], wkr_ext.ap())
                    for ci in range(NCT):
                        tp = ptp.tile([P, 2 * P], F32R, tag="tp", name="tp")
                        nc.tensor.transpose(
                            tp[:, :RHD], kstrip[:, ci * P:(ci + 1) * P],
                            ident[:RHD, :RHD])
                        ev, od = _deint(tp[:, :RHD])
                        nc.scalar.copy(wkrT[ci][:, 0:32], ev)
                        nc.scalar.copy(wkrT[ci][:, 32:64], od)

                def table_prep():
                    for s in range(T // P):
                        cst = pa.tile([P, RHD // 2], F32R, tag="cst", bufs=2,
                                      name="cst")
                        sst = pa.tile([P, RHD // 2], F32R, tag="sst", bufs=2,
                                      name="sst")
                        nc.scalar.dma_start(cst[:],
                                            cos_ext.ap()[s * P:(s + 1) * P, :])
                        nc.scalar.dma_start(sst[:],
                                            sin_ext.ap()[s * P:(s + 1) * P, :])
                        tp = ptp.tile([P, 2 * P], F32R, tag="tp", name="tp")
                        nc.tensor.transpose(tp[:32, 0:P], cst[:], ident[:])
                        nc.tensor.transpose(tp[:32, P:2 * P], sst[:], ident[:])
                        nc.vector.tensor_copy(ca4[0:32, s * P:(s + 1) * P],
                                              tp[:32, 0:P])
                        nc.vector.tensor_copy(sa4[0:32, s * P:(s + 1) * P],
                                              tp[:32, P:2 * P])
                    for d in range(1, 4):
                        nc.vector.tensor_copy(ca4[32 * d:32 * (d + 1), :],
                                              ca4[0:32, :])
                        nc.vector.tensor_copy(sa4[32 * d:32 * (d + 1), :],
                                              sa4[0:32, :])

                def wu_prep():
                    for w_ext, wT in ((wuq_ext, wuqT), (wuk_ext, wukT),
                                      (wuv_ext, wuvT)):
                        for rp in range(HLOC * HS // P // 2):
                            sA = pa.tile([P, NL], F32R, tag="usA", bufs=2,
                                         name="usA")
                            sB = pa.tile([P, NL], F32R, tag="usB", bufs=2,
                                         name="usB")
                            nc.scalar.dma_start(
                                sA[:], w_ext.ap()[2 * rp * P:(2 * rp + 1) * P, :])
                            nc.scalar.dma_start(
                                sB[:], w_ext.ap()[(2 * rp + 1) * P:(2 * rp + 2) * P, :])
                            for cs in range(NNL):
                                transpose_pair_into(
                                    wT[cs][:, 2 * rp * P:(2 * rp + 2) * P],
                                    sA[:, cs * P:(cs + 1) * P],
                                    sB[:, cs * P:(cs + 1) * P],
                                )

                def wo_wqr_prep():
                    for g in range(2):
                        strip = pa.tile([P, NL], F32R, tag="qrs", bufs=2,
                                        name="qrs")
                        nc.scalar.dma_start(
                            strip[:], wqr_ext.ap()[g * P:(g + 1) * P, :])
                        for cs in range(NNL):
                            tp = ptp.tile([P, 2 * P], F32R, tag="tp", name="tp")
                            nc.tensor.transpose(
                                tp[:, 0:P], strip[:, cs * P:(cs + 1) * P],
                                ident[:])
                            evA, odA = _deint(tp[:, 0:RHD])
                            evB, odB = _deint(tp[:, RHD:2 * RHD])
                            nc.scalar.copy(wqrT[g][cs][:, 0:32], evA)
                            nc.scalar.copy(wqrT[g][cs][:, 32:64], evB)
                            nc.scalar.copy(wqrT[g][cs][:, 64:96], odA)
                            nc.scalar.copy(wqrT[g][cs][:, 96:128], odB)
                    for sp in range(C // P // 2):
                        oA = pa.tile([P, HLOC * HS], F32R, tag="osA", bufs=2,
                                     name="osA")
                        oB = pa.tile([P, HLOC * HS], F32R, tag="osB", bufs=2,
                                     name="osB")
                        nc.scalar.dma_start(
                            oA[:], wo_ext.ap()[2 * sp * P:(2 * sp + 1) * P, :])
                        nc.scalar.dma_start(
                            oB[:], wo_ext.ap()[(2 * sp + 1) * P:(2 * sp + 2) * P, :])
                        for fs in range(HLOC):
                            transpose_pair_into(
                                woT[fs][:, 2 * sp * P:(2 * sp + 2) * P],
                                oA[:, fs * P:(fs + 1) * P],
                                oB[:, fs * P:(fs + 1) * P],
                            )

                # interleave: x transposes first so PE starts immediately,
                # weight preps slot between chunks while x DMA streams.
                x_chunk_transpose(0)
                wd_prep()
                x_chunk_matmuls(0)
                x_chunk_transpose(1)
                table_prep()
                x_chunk_matmuls(1)
                x_chunk_transpose(2)
                wu_prep()
                x_chunk_matmuls(2)
                x_chunk_transpose(3)
                wo_wqr_prep()
                x_chunk_matmuls(3)

                pa_ctx[1].__exit__(None, None, None)
                pa_ctx[0].__exit__(None, None, None)

                # ============ phase B: rope, up-projections, fp8 packs ======
                pb2_ctx = tc.tile_pool(name="pb2", bufs=1)
                pb2 = pb2_ctx.__enter__()
                q8 = [pb2.tile([P, 2, T], F8, tag=f"q8{h}", name=f"q8{h}")
                      for h in range(HLOC)]
                k8 = [pb2.tile([P, 2, T], F8, tag=f"k8{h}", name=f"k8{h}")
                      for h in range(HLOC)]
                vv = [pb2.tile([P, T // P, P], BF16, tag=f"vv{h}", name=f"vv{h}")
                      for h in range(HLOC)]
                for h in range(HLOC):
                    nc.vector.memset(q8[h][64:128, 1, :], 0.0)
                    nc.gpsimd.memset(k8[h][64:128, 1, :], 0.0)

                with (
                    tc.tile_pool(name="pmm", bufs=2, space="PSUM") as pmm,
                    tc.tile_pool(name="pou", bufs=2, space="PSUM") as pou,
                    tc.tile_pool(name="pde", bufs=2, space="PSUM") as pde,
                    tc.tile_pool(name="pat", bufs=1) as pat,
                ):
                    # k_r rope -> krf8 (planar re/im), shared across heads
                    krf8 = pb2.tile([RHD, T], F8, tag="krf8", name="krf8")
                    rtmp = pb2.tile([P, T], BF16, tag="rtmp", name="rtmp")
                    rro = pb2.tile([P, T], BF16, tag="rro", name="rro")
                    nc.vector.tensor_mul(rtmp[0:32, :], krraw[32:64, :], sa4[32:64, :])
                    nc.vector.tensor_mul(rtmp[32:64, :], krraw[32:64, :], ca4[32:64, :])
                    nc.vector.tensor_mul(rro[0:32, :], krraw[0:32, :], ca4[0:32, :])
                    nc.vector.tensor_mul(rro[32:64, :], krraw[0:32, :], sa4[0:32, :])
                    nc.vector.tensor_sub(krf8[0:32, :], rro[0:32, :], rtmp[0:32, :])
                    nc.vector.tensor_add(krf8[32:64, :], rro[32:64, :], rtmp[32:64, :])
                    for h in range(HLOC):
                        nc.scalar.dma_start(k8[h][0:RHD, 1, :], krf8[:])

                    # up-projections, head-pair at a time
                    for g in range(2):
                        hA, hB = 2 * g, 2 * g + 1
                        qraw = pb2.tile([P, T], BF16, tag="qraw", name="qraw")
                        for ch in range(NCH):
                            sl = slice(ch * TCH, (ch + 1) * TCH)
                            for hh in (hA, hB):
                                for wT, src, dst in ((wuqT, cqT, q8),
                                                     (wukT, ckvT, k8)):
                                    acc = pmm.tile([P, TCH], F32, tag="mm",
                                                   name="mm")
                                    for nl in range(NNL):
                                        nc.tensor.matmul(
                                            acc[:],
                                            wT[nl][:, hh * P:(hh + 1) * P],
                                            src[nl][:, sl],
                                            start=(nl == 0),
                                            stop=(nl == NNL - 1),
                                        )
                                    ecopy(dst[hh][:, 0, sl], acc[:])
                            qacc = pmm.tile([P, TCH], F32, tag="mm", name="mm")
                            for nl in range(NNL):
                                nc.tensor.matmul(
                                    qacc[:],
                                    wqrT[g][nl][:],
                                    cqT[nl][:, sl],
                                    start=(nl == 0),
                                    stop=(nl == NNL - 1),
                                )
                            ecopy(qraw[:, sl], qacc[:], pin="act")
                        # natural-layout V for both heads
                        for hh in (hA, hB):
                            for ts4 in range(T // TCH):
                                vps = pmm.tile([P, TCH], F32, tag="mm", name="mm")
                                for j in range(4):
                                    kt = ts4 * 4 + j
                                    for nl in range(NNL):
                                        nc.tensor.matmul(
                                            vps[:, j * P:(j + 1) * P],
                                            ckvT[nl][:, kt * P:(kt + 1) * P],
                                            wuvT[nl][:, hh * P:(hh + 1) * P],
                                            start=(nl == 0),
                                            stop=(nl == NNL - 1),
                                            skip_group_check=True,
                                        )
                                ecopy(
                                    vv[hh][:, ts4 * 4:(ts4 + 1) * 4, :],
                                    vps[:].rearrange("p (a b) -> p a b", a=4),
                                )
                        # rope for the pair: rows [Are, Bre, Aim, Bim]
                        roq = pb2.tile([P, T], F8, tag="roq", bufs=2, name="roq")
                        nc.vector.tensor_mul(rtmp[0:64, :], qraw[64:128, :],
                                             sa4[64:128, :])
                        nc.vector.tensor_mul(rtmp[64:128, :], qraw[64:128, :],
                                             ca4[64:128, :])
                        nc.vector.tensor_mul(rro[0:64, :], qraw[0:64, :],
                                             ca4[0:64, :])
                        nc.vector.tensor_mul(rro[64:128, :], qraw[0:64, :],
                                             sa4[0:64, :])
                        nc.vector.tensor_sub(roq[0:64, :], rro[0:64, :],
                                             rtmp[0:64, :])
                        nc.vector.tensor_add(roq[64:128, :], rro[64:128, :],
                                             rtmp[64:128, :])
                        nc.scalar.dma_start(q8[hA][0:32, 1, :], roq[0:32, :])
                        nc.scalar.dma_start(q8[hA][32:64, 1, :], roq[64:96, :])
                        nc.scalar.dma_start(q8[hB][0:32, 1, :], roq[32:64, :])
                        nc.scalar.dma_start(q8[hB][32:64, 1, :], roq[96:128, :])

                    # ============ attention + deferred norm/output proj =====
                    deferred_b = deque()   # normalization tails
                    outproj_q = deque()    # (ohs, tq, cs_start) groups

                    def emit_outproj_group():
                        g_ohs, g_tq, cs0 = outproj_q.popleft()
                        g_qsl = slice(g_tq * TCH, (g_tq + 1) * TCH)
                        for cs in range(cs0, cs0 + 4):
                            acc = pmm.tile([P, TCH], F32, tag="mm", name="mm")
                            for h2 in range(HLOC):
                                nc.tensor.matmul(
                                    acc[:],
                                    woT[h2][:, cs * P:(cs + 1) * P],
                                    g_ohs[h2][:],
                                    start=(h2 == 0),
                                    stop=(h2 == HLOC - 1),
                                )
                            ot = pat.tile([P, TCH], F32, tag="ot", bufs=3,
                                          name="ot")
                            ecopy(ot[:], acc[:])
                            nc.sync.dma_start(
                                out_ext.ap()[cs * P:(cs + 1) * P, g_qsl], ot[:])

                    ohs_by_tq = {}
                    for tq in range(NCH):
                        qsl = slice(tq * TCH, (tq + 1) * TCH)
                        ohs = []
                        ohs_by_tq[tq] = ohs
                        for h in range(HLOC):
                            outU = pou.tile([P, TCH], F32, tag="ou", name="ou")
                            den = pde.tile([P, 4], F32, tag="de", name="de")
                            nc.vector.memset(den[:], 0.0)
                            nsub = (tq + 1) * 4
                            pend = deque()

                            def flush_one(outU=outU, den=den, nsub=nsub, h=h,
                                          pend=pend):
                                Pt, kt = pend.popleft()
                                for qq in range(4):
                                    nc.tensor.matmul(
                                        den[:, qq:qq + 1],
                                        Pt[:, qq * P:(qq + 1) * P],
                                        onescol[:],
                                        start=False,
                                        stop=(kt == nsub - 1),
                                        skip_group_check=True,
                                    )
                                nc.tensor.matmul(
                                    outU[:],
                                    vv[h][:, kt, :],
                                    Pt[:],
                                    start=(kt == 0),
                                    stop=(kt == nsub - 1),
                                    skip_group_check=True,
                                )

                            for kt in range(nsub):
                                kc, ks = kt // 4, kt % 4
                                diag = kc == tq
                                off = ks * P if diag else 0
                                npr = TCH - off
                                ST = pmm.tile([P, TCH], F32, tag="mm", name="mm")
                                nc.tensor.matmul(
                                    ST[:, 0:npr],
                                    k8[h][:, :, kt * P:(kt + 1) * P],
                                    q8[h][:, :, qsl.start + off:qsl.stop],
                                    perf_mode=DR,
                                    start=True,
                                    stop=True,
                                )
                                if diag:
                                    nc.vector.tensor_add(
                                        ST[:, 0:P], ST[:, 0:P],
                                        maskbuf[:, 384:384 + P],
                                    )
                                Pt = pat.tile([P, TCH], BF16, tag="pt", bufs=6,
                                              name="pt")
                                if off:
                                    nc.vector.memset(Pt[:, 0:off], 0.0)
                                nc.scalar.activation(Pt[:, off:TCH], ST[:, 0:npr],
                                                     Exp, scale=SCALE)
                                pend.append((Pt, kt))
                                if len(pend) > 2:
                                    flush_one()
                                if kt == 2:
                                    # emit previous head's deferred tail and
                                    # one output-projection group here, where
                                    # PE has score work queued to hide them
                                    if deferred_b:
                                        deferred_b.popleft()()
                                    if outproj_q:
                                        emit_outproj_group()
                            while pend:
                                flush_one()

                            # normalization head: transpose+recip now (DVE),
                            # broadcast+apply deferred under the next head
                            den_sb = pat.tile([P, 4], F32R, tag="dsb", bufs=2,
                                              name="dsb")
                            nc.vector.tensor_copy(den_sb[:], den[:])
                            tpd = ptp.tile([P, 2 * P], F32R, tag="tp", name="tp")
                            nc.tensor.transpose(tpd[0:4, 0:P], den_sb[:], ident[:])
                            rec = pat.tile([4, P], F32, tag="rec", bufs=2,
                                           name="rec")
                            nc.vector.reciprocal(rec[:], tpd[0:4, 0:P])
                            recb = pat.tile([4, P], BF16, tag="recb", bufs=2,
                                            name="recb")
                            nc.vector.tensor_copy(recb[:], rec[:])
                            oh = pat.tile([P, TCH], BF16, tag=f"oh{h}", bufs=2,
                                          name=f"oh{h}")
                            ohs.append(oh)

                            def norm_tail(recb=recb, outU=outU, oh=oh, h=h,
                                          tq=tq):
                                bc = pmm.tile([P, TCH], F32, tag="mm", name="mm")
                                for qq in range(4):
                                    nc.tensor.matmul(
                                        bc[:, qq * P:(qq + 1) * P],
                                        sel4[:, qq * P:(qq + 1) * P],
                                        recb[:],
                                        start=True, stop=True,
                                        skip_group_check=True,
                                    )
                                bcs = pat.tile([P, TCH], BF16, tag="bcs",
                                               bufs=2, name="bcs")
                                nc.scalar.copy(bcs[:], bc[:])
                                nc.vector.tensor_mul(oh[:], outU[:], bcs[:])
                                if h == HLOC - 1:
                                    for cs0 in range(0, NCT, 4):
                                        outproj_q.append(
                                            (ohs_by_tq[tq], tq, cs0))

                            deferred_b.append(norm_tail)

                    # drain deferred work
                    while deferred_b:
                        deferred_b.popleft()()
                    while outproj_q:
                        emit_outproj_group()

                pb2_ctx.__exit__(None, None, None)

    nc.compile()
    return nc


def _get_nc():
    if "nc" not in _NC_CACHE:
        _NC_CACHE["nc"] = build()
    return _NC_CACHE["nc"]


def kernel(x, freqs_cos, freqs_sin, W_dq, W_uq, W_dkv, W_uk, W_uv, W_qr, W_kr,
           W_o, trace=False, **trace_kwargs):
    nc = _get_nc()
    f32 = lambda a: np.ascontiguousarray(np.asarray(a, dtype=np.float32))
    x = f32(x); W_dq = f32(W_dq); W_uq = f32(W_uq); W_dkv = f32(W_dkv)
    W_uk = f32(W_uk); W_uv = f32(W_uv); W_qr = f32(W_qr); W_kr = f32(W_kr)
    W_o = f32(W_o)
    cos = f32(freqs_cos); sin = f32(freqs_sin)

    in_maps = []
    for c in range(8):
        b, r = divmod(c, 4)
        in_maps.append({
            "x": x[b],
            "wdq": W_dq, "wdkv": W_dkv, "wkr": W_kr,
            "wuq": W_uq[r * HLOC * HS:(r + 1) * HLOC * HS],
            "wuk": W_uk[r * HLOC * HS:(r + 1) * HLOC * HS],
            "wuv": W_uv[r * HLOC * HS:(r + 1) * HLOC * HS],
            "wqr": W_qr[r * HLOC * RHD:(r + 1) * HLOC * RHD],
            "wo": W_o[:, r * HLOC * HS:(r + 1) * HLOC * HS],
            "cos": cos, "sin": sin,
        })
    res = run_bass_kernel_spmd(nc, in_maps, core_ids=list(range(8)),
                               trace=trace, **trace_kwargs)
    out = np.zeros((B, T, C), dtype=np.float32)
    for c in range(8):
        b = c // 4
        out[b] += res.results[c]["out"].T
    kernel.last_result = res
    return out


# revision 37
# speedup vs baseline: 1.3842x; 1.0715x over previous
"""MLA-style attention kernel for 8 TRN2 NeuronCores, v3.

Sharding: core c -> batch b = c//4, heads r*4..r*4+3 where r = c%4.
The latent down-projections are REPLICATED within each 4-core batch
group (no collective, no cross-core dependency): each core computes the
full-T latents c_q/c_kv/k_r from the full x[b], then its 4 heads'
attention and a partial output projection summed on the host.

All activations stay SBUF-resident in a transposed [feature, T] layout.
Down/up-projections run in bf16 (PSUM fp32 accumulate).  Scores use
fp8e4 with DoubleRow perf mode: q/k packed as [128, 2, T] fp8 where
slot 0 holds the 128 content dims and slot 1 rows 0:64 hold the roped
rope dims (planar re/im), rows 64:128 zero.  One DoubleRow matmul per
512x128 score subtile (4x fewer PE cycles than two f32r matmuls).
Softmax denominators use N=1 ones-column matmuls accumulating into a
[128q, 4] PSUM tile (start=False onto memset zeros -- a start=True
would wipe sibling columns through the 2KB zero-region), then
transpose + reciprocal + selector broadcast matmul for normalization.
Diagonal causal blocks shrink the score matmul to the valid q-range,
memset the dead Pt columns, and add a fixed 128-wide triangular mask.
exp() pipelines two subtiles deep; the normalization tail of head h and
the output projection of chunk tq-1 are emitted under the NEXT head's
score loop so PE never drains while Act works.  V is produced directly
in natural [t, hs] layout (lhsT = ckv^T): no transposes after phase A.
Both hardware DMA queues are used: SP for W_dq/W_dkv/x/output, Act for
cos/sin, W_u/W_qr/W_o, mask, and the SBUF-to-SBUF fp8 slot copies.
"""
import math
from collections import deque
import numpy as np

import concourse.bass as bass
import concourse.bacc as bacc
import concourse.mybir as mybir
import concourse.tile as tile
from concourse.bass_utils import run_bass_kernel_spmd

F32 = mybir.dt.float32
F32R = mybir.dt.float32r
BF16 = mybir.dt.bfloat16
F8 = mybir.dt.float8e4
Exp = mybir.ActivationFunctionType.Exp
DR = mybir.MatmulPerfMode.DoubleRow

B, T, C = 2, 2048, 2048
H = 16
HS = 128
NL = 512
RHD = 64
HLOC = 4              # heads per core
P = 128
NNL = NL // P         # 4
NCT = C // P          # 16
TCH = 512
NCH = T // TCH        # 4
SCALE = 1.0 / math.sqrt(HS + RHD)
NEG = -1.0e30

_NC_CACHE = {}


def _deint(ap2d):
    # [p, 2d] -> (evens [p, d], odds [p, d]) along the free dim
    rr = ap2d.rearrange("p (d two) -> p two d", two=2)
    return rr[:, 0, :], rr[:, 1, :]


def build():
    nc = bacc.Bacc("TRN2", target_bir_lowering=False, debug=False, num_devices=8)

    x_ext = nc.dram_tensor("x", [T, C], F32R, kind="ExternalInput")
    wdq_ext = nc.dram_tensor("wdq", [NL, C], F32R, kind="ExternalInput")
    wdkv_ext = nc.dram_tensor("wdkv", [NL, C], F32R, kind="ExternalInput")
    wkr_ext = nc.dram_tensor("wkr", [RHD, C], F32R, kind="ExternalInput")
    wuq_ext = nc.dram_tensor("wuq", [HLOC * HS, NL], F32R, kind="ExternalInput")
    wuk_ext = nc.dram_tensor("wuk", [HLOC * HS, NL], F32R, kind="ExternalInput")
    wuv_ext = nc.dram_tensor("wuv", [HLOC * HS, NL], F32R, kind="ExternalInput")
    wqr_ext = nc.dram_tensor("wqr", [HLOC * RHD, NL], F32R, kind="ExternalInput")
    wo_ext = nc.dram_tensor("wo", [C, HLOC * HS], F32R, kind="ExternalInput")
    cos_ext = nc.dram_tensor("cos", [T, RHD // 2], F32R, kind="ExternalInput")
    sin_ext = nc.dram_tensor("sin", [T, RHD // 2], F32R, kind="ExternalInput")
    out_ext = nc.dram_tensor("out", [C, T], F32, kind="ExternalOutput")

    ident_dram = nc.inline_tensor(np.eye(P, dtype=np.float32), name="identc")
    # triangular mask for the 128-wide diagonal band of shrunk S^T tiles
    m = np.full((P, 896), NEG, dtype=np.float32)
    for jj in range(P):
        m[jj, 384 + jj:] = 0.0
    masks_dram = nc.inline_tensor(m, name="maskc")
    # row selector for the 1/den broadcast: sel4[k, qq*128+j] = (k == qq)
    sel = np.zeros((4, 512), dtype=np.float32)
    for qq in range(4):
        sel[qq, qq * P:(qq + 1) * P] = 1.0
    sel4_dram = nc.inline_tensor(sel, name="sel4c")

    with tile.TileContext(nc) as tc:
        with (
            tc.tile_pool(name="pers", bufs=1) as pers,
            tc.tile_pool(name="ptp", bufs=2, space="PSUM") as ptp,
        ):
            ident = pers.tile([P, P], F32R, tag="ident", name="ident")
            nc.sync.dma_start(ident[:], ident_dram.ap().bitcast(F32R))
            maskbuf = pers.tile([P, 896], BF16, tag="maskbuf", name="maskbuf")
            nc.gpsimd.dma_start(out=maskbuf[:], in_=masks_dram.ap())
            onescol = pers.tile([P, 1], BF16, tag="onescol", name="onescol")
            nc.vector.memset(onescol[:], 1.0)
            sel4 = pers.tile([4, TCH], BF16, tag="sel4", name="sel4")
            nc.gpsimd.dma_start(out=sel4[:], in_=sel4_dram.ap())

            # rope tables, cos/sin duplicated on all four 32-row groups
            ca4 = pers.tile([P, T], BF16, tag="ca4", name="ca4")
            sa4 = pers.tile([P, T], BF16, tag="sa4", name="sa4")

            # full-T latents (bf16, [feat, T])
            cqT = [pers.tile([P, T], BF16, tag=f"cqT{i}", name=f"cqT{i}")
                   for i in range(NNL)]
            ckvT = [pers.tile([P, T], BF16, tag=f"ckvT{i}", name=f"ckvT{i}")
                    for i in range(NNL)]
            krraw = pers.tile([RHD, T], BF16, tag="krraw", name="krraw")

            _ecnt = [0]

            def ecopy(dst, src, pin=None):
                """PSUM->SBUF evacuation copy, alternating Act/DVE."""
                _ecnt[0] += 1
                eng = pin if pin else ("act" if _ecnt[0] % 2 else "dve")
                if eng == "act":
                    nc.scalar.copy(dst, src)
                else:
                    nc.vector.tensor_copy(dst, src)

            def transpose_pair_into(dst_ap, srcA, srcB, pin=None):
                tp2 = ptp.tile([P, 2 * P], F32R, tag="tp", name="tp")
                nc.tensor.transpose(tp2[:, 0:P], srcA, ident[:])
                nc.tensor.transpose(tp2[:, P:2 * P], srcB, ident[:])
                ecopy(dst_ap, tp2[:], pin=pin)

            # ============ phase A: x chunks + all weight prep, interleaved ==
            with tc.tile_pool(name="pb", bufs=1) as pb:
                # -- persistent-ish weight destinations (pb outlives phase A)
                wuqT = [pb.tile([P, HLOC * HS], BF16, tag=f"wuqT{i}",
                                name=f"wuqT{i}") for i in range(NNL)]
                wukT = [pb.tile([P, HLOC * HS], BF16, tag=f"wukT{i}",
                                name=f"wukT{i}") for i in range(NNL)]
                wuvT = [pb.tile([P, HLOC * HS], BF16, tag=f"wuvT{i}",
                                name=f"wuvT{i}") for i in range(NNL)]
                wqrT = [[pb.tile([P, P], BF16, tag=f"wqrT{g}{i}",
                                 name=f"wqrT{g}{i}") for i in range(NNL)]
                        for g in range(2)]
                woT = [pb.tile([P, C], BF16, tag=f"woT{i}", name=f"woT{i}")
                       for i in range(HLOC)]

                pa_ctx = (
                    tc.tile_pool(name="pa", bufs=1),
                    tc.tile_pool(name="pacc", bufs=1, space="PSUM"),
                )
                pa = pa_ctx[0].__enter__()
                pacc = pa_ctx[1].__enter__()

                wdqT = [pa.tile([P, NL], BF16, tag=f"wdqT{i}", name=f"wdqT{i}")
                        for i in range(NCT)]
                wdkvT = [pa.tile([P, NL], BF16, tag=f"wdkvT{i}",
                                 name=f"wdkvT{i}") for i in range(NCT)]
                wkrT = [pa.tile([P, RHD], BF16, tag=f"wkrT{i}", name=f"wkrT{i}")
                        for i in range(NCT)]
                xT = [pa.tile([P, TCH], BF16, tag=f"xT{i}", name=f"xT{i}")
                      for i in range(NCT)]

                def x_chunk_transpose(tch):
                    t0 = tch * TCH
                    for sp in range(2):
                        rA = slice(t0 + 2 * sp * P, t0 + (2 * sp + 1) * P)
                        rB = slice(t0 + (2 * sp + 1) * P, t0 + (2 * sp + 2) * P)
                        for hf in range(2):
                            cf = slice(hf * (C // 2), (hf + 1) * (C // 2))
                            xA = pa.tile([P, C // 2], F32R, tag="xA", bufs=2,
                                         name="xA")
                            xB = pa.tile([P, C // 2], F32R, tag="xB", bufs=2,
                                         name="xB")
                            nc.sync.dma_start(xA[:], x_ext.ap()[rA, cf])
                            nc.sync.dma_start(xB[:], x_ext.ap()[rB, cf])
                            for ci in range(NCT // 2):
                                transpose_pair_into(
                                    xT[hf * 8 + ci][:, 2 * sp * P:(2 * sp + 2) * P],
                                    xA[:, ci * P:(ci + 1) * P],
                                    xB[:, ci * P:(ci + 1) * P],
                                )

                def x_chunk_matmuls(tch):
                    t0 = tch * TCH
                    for wTs, dstT in ((wdqT, cqT), (wdkvT, ckvT)):
                        for j in range(NNL):
                            acc = pacc.tile([P, TCH], F32, tag=f"acc{j}",
                                            name=f"acc{j}")
                            for ci in range(NCT):
                                nc.tensor.matmul(
                                    acc[:],
                                    wTs[ci][:, j * P:(j + 1) * P],
                                    xT[ci][:],
                                    start=(ci == 0),
                                    stop=(ci == NCT - 1),
                                )
                            ecopy(dstT[j][:, t0:t0 + TCH], acc[:])
                    acck = pacc.tile([RHD, TCH], F32, tag="acck", name="acck")
                    for ci in range(NCT):
                        nc.tensor.matmul(
                            acck[:],
                            wkrT[ci][:],
                            xT[ci][:],
                            start=(ci == 0),
                            stop=(ci == NCT - 1),
                        )
                    ecopy(krraw[:, t0:t0 + TCH], acck[:], pin="act")

                def wd_prep():
                    for w_ext, wTs in ((wdq_ext, wdqT), (wdkv_ext, wdkvT)):
                        for rp in range(NL // P // 2):
                            rA = slice(2 * rp * P, (2 * rp + 1) * P)
                            rB = slice((2 * rp + 1) * P, (2 * rp + 2) * P)
                            for hf in range(2):
                                cf = slice(hf * (C // 2), (hf + 1) * (C // 2))
                                sA = pa.tile([P, C // 2], F32R, tag="wsA",
                                             bufs=2, name="wsA")
                                sB = pa.tile([P, C // 2], F32R, tag="wsB",
                                             bufs=2, name="wsB")
                                nc.sync.dma_start(sA[:], w_ext.ap()[rA, cf])
                                nc.sync.dma_start(sB[:], w_ext.ap()[rB, cf])
                                for ci in range(NCT // 2):
                                    transpose_pair_into(
                                        wTs[hf * 8 + ci][:, 2 * rp * P:(2 * rp + 2) * P],
                                        sA[:, ci * P:(ci + 1) * P],
                                        sB[:, ci * P:(ci + 1) * P],
                                    )
                    kstrip = pa.tile([RHD, C], F32R, tag="kstrip", name="kstrip")
                    nc.sync.dma_start(kstrip[:], wkr_ext.ap())
                    for ci in range(NCT):
                        tp = ptp.tile([P, 2 * P], F32R, tag="tp", name="tp")
                        nc.tensor.transpose(
                            tp[:, :RHD], kstrip[:, ci * P:(ci + 1) * P],
                            ident[:RHD, :RHD])
                        ev, od = _deint(tp[:, :RHD])
                        nc.scalar.copy(wkrT[ci][:, 0:32], ev)
                        nc.scalar.copy(wkrT[ci][:, 32:64], od)

                def table_prep():
                    for s in range(T // P):
                        cst = pa.tile([P, RHD // 2], F32R, tag="cst", bufs=2,
                                      name="cst")
                        sst = pa.tile([P, RHD // 2], F32R, tag="sst", bufs=2,
                                      name="sst")
                        nc.sync.dma_start(cst[:],
                                          cos_ext.ap()[s * P:(s + 1) * P, :])
                        nc.sync.dma_start(sst[:],
                                          sin_ext.ap()[s * P:(s + 1) * P, :])
                        tp = ptp.tile([P, 2 * P], F32R, tag="tp", name="tp")
                        nc.tensor.transpose(tp[:32, 0:P], cst[:], ident[:])
                        nc.tensor.transpose(tp[:32, P:2 * P], sst[:], ident[:])
                        nc.vector.tensor_copy(ca4[0:32, s * P:(s + 1) * P],
                                              tp[:32, 0:P])
                        nc.vector.tensor_copy(sa4[0:32, s * P:(s + 1) * P],
                                              tp[:32, P:2 * P])
                    for d in range(1, 4):
                        nc.vector.tensor_copy(ca4[32 * d:32 * (d + 1), :],
                                              ca4[0:32, :])
                        nc.vector.tensor_copy(sa4[32 * d:32 * (d + 1), :],
                                              sa4[0:32, :])

                def wu_prep():
                    for w_ext, wT in ((wuq_ext, wuqT), (wuk_ext, wukT),
                                      (wuv_ext, wuvT)):
                        for rp in range(HLOC * HS // P // 2):
                            sA = pa.tile([P, NL], F32R, tag="usA", bufs=2,
                                         name="usA")
                            sB = pa.tile([P, NL], F32R, tag="usB", bufs=2,
                                         name="usB")
                            nc.sync.dma_start(
                                sA[:], w_ext.ap()[2 * rp * P:(2 * rp + 1) * P, :])
                            nc.sync.dma_start(
                                sB[:], w_ext.ap()[(2 * rp + 1) * P:(2 * rp + 2) * P, :])
                            for cs in range(NNL):
                                transpose_pair_into(
                                    wT[cs][:, 2 * rp * P:(2 * rp + 2) * P],
                                    sA[:, cs * P:(cs + 1) * P],
                                    sB[:, cs * P:(cs + 1) * P],
                                )

                def wo_wqr_prep():
                    for g in range(2):
                        strip = pa.tile([P, NL], F32R, tag="qrs", bufs=2,
                                        name="qrs")
                        nc.sync.dma_start(
                            strip[:], wqr_ext.ap()[g * P:(g + 1) * P, :])
                        for cs in range(NNL):
                            tp = ptp.tile([P, 2 * P], F32R, tag="tp", name="tp")
                            nc.tensor.transpose(
                                tp[:, 0:P], strip[:, cs * P:(cs + 1) * P],
                                ident[:])
                            evA, odA = _deint(tp[:, 0:RHD])
                            evB, odB = _deint(tp[:, RHD:2 * RHD])
                            nc.scalar.copy(wqrT[g][cs][:, 0:32], evA)
                            nc.scalar.copy(wqrT[g][cs][:, 32:64], evB)
                            nc.scalar.copy(wqrT[g][cs][:, 64:96], odA)
                            nc.scalar.copy(wqrT[g][cs][:, 96:128], odB)
                    for sp in range(C // P // 2):
                        oA = pa.tile([P, HLOC * HS], F32R, tag="osA", bufs=2,
                                     name="osA")
                        oB = pa.tile([P, HLOC * HS], F32R, tag="osB", bufs=2,
                                     name="osB")
                        nc.sync.dma_start(
                            oA[:], wo_ext.ap()[2 * sp * P:(2 * sp + 1) * P, :])
                        nc.sync.dma_start(
                            oB[:], wo_ext.ap()[(2 * sp + 1) * P:(2 * sp + 2) * P, :])
                        for fs in range(HLOC):
                            transpose_pair_into(
                                woT[fs][:, 2 * sp * P:(2 * sp + 2) * P],
                                oA[:, fs * P:(fs + 1) * P],
                                oB[:, fs * P:(fs + 1) * P],
                            )

                # interleave: x transposes first so PE starts immediately,
                # weight preps slot between chunks while x DMA streams.
                x_chunk_transpose(0)
                wd_prep()
                x_chunk_matmuls(0)
                x_chunk_transpose(1)
                table_prep()
                x_chunk_matmuls(1)
                x_chunk_transpose(2)
                wu_prep()
                x_chunk_matmuls(2)
                x_chunk_transpose(3)
                wo_wqr_prep()
                x_chunk_matmuls(3)

                pa_ctx[1].__exit__(None, None, None)
                pa_ctx[0].__exit__(None, None, None)

                # ============ phase B: rope, up-projections, fp8 packs ======
                pb2_ctx = tc.tile_pool(name="pb2", bufs=1)
                pb2 = pb2_ctx.__enter__()
                q8 = [pb2.tile([P, 2, T], F8, tag=f"q8{h}", name=f"q8{h}")
                      for h in range(HLOC)]
                k8 = [pb2.tile([P, 2, T], F8, tag=f"k8{h}", name=f"k8{h}")
                      for h in range(HLOC)]
                vv = [pb2.tile([P, T // P, P], BF16, tag=f"vv{h}", name=f"vv{h}")
                      for h in range(HLOC)]
                for h in range(HLOC):
                    nc.vector.memset(q8[h][64:128, 1, :], 0.0)
                    nc.gpsimd.memset(k8[h][64:128, 1, :], 0.0)

                with (
                    tc.tile_pool(name="pmm", bufs=2, space="PSUM") as pmm,
                    tc.tile_pool(name="pou", bufs=2, space="PSUM") as pou,
                    tc.tile_pool(name="pde", bufs=2, space="PSUM") as pde,
                    tc.tile_pool(name="pat", bufs=1) as pat,
                ):
                    # k_r rope -> krf8 (planar re/im), shared across heads
                    krf8 = pb2.tile([RHD, T], F8, tag="krf8", name="krf8")
                    rtmp = pb2.tile([P, T], BF16, tag="rtmp", name="rtmp")
                    rro = pb2.tile([P, T], BF16, tag="rro", name="rro")
                    nc.vector.tensor_mul(rtmp[0:32, :], krraw[32:64, :], sa4[32:64, :])
                    nc.vector.tensor_mul(rtmp[32:64, :], krraw[32:64, :], ca4[32:64, :])
                    nc.vector.tensor_mul(rro[0:32, :], krraw[0:32, :], ca4[0:32, :])
                    nc.vector.tensor_mul(rro[32:64, :], krraw[0:32, :], sa4[0:32, :])
                    nc.vector.tensor_sub(krf8[0:32, :], rro[0:32, :], rtmp[0:32, :])
                    nc.vector.tensor_add(krf8[32:64, :], rro[32:64, :], rtmp[32:64, :])
                    for h in range(HLOC):
                        nc.gpsimd.dma_start(out=k8[h][0:RHD, 1, :], in_=krf8[:])

                    # up-projections, head-pair at a time
                    for g in range(2):
                        hA, hB = 2 * g, 2 * g + 1
                        qraw = pb2.tile([P, T], BF16, tag="qraw", name="qraw")
                        for ch in range(NCH):
                            sl = slice(ch * TCH, (ch + 1) * TCH)
                            for hh in (hA, hB):
                                for wT, src, dst in ((wuqT, cqT, q8),
                                                     (wukT, ckvT, k8)):
                                    acc = pmm.tile([P, TCH], F32, tag="mm",
                                                   name="mm")
                                    for nl in range(NNL):
                                        nc.tensor.matmul(
                                            acc[:],
                                            wT[nl][:, hh * P:(hh + 1) * P],
                                            src[nl][:, sl],
                                            start=(nl == 0),
                                            stop=(nl == NNL - 1),
                                        )
                                    ecopy(dst[hh][:, 0, sl], acc[:], pin="act")
                            qacc = pmm.tile([P, TCH], F32, tag="mm", name="mm")
                            for nl in range(NNL):
                                nc.tensor.matmul(
                                    qacc[:],
                                    wqrT[g][nl][:],
                                    cqT[nl][:, sl],
                                    start=(nl == 0),
                                    stop=(nl == NNL - 1),
                                )
                            ecopy(qraw[:, sl], qacc[:], pin="act")
                        # natural-layout V for both heads
                        for hh in (hA, hB):
                            for ts4 in range(T // TCH):
                                vps = pmm.tile([P, TCH], F32, tag="mm", name="mm")
                                for j in range(4):
                                    kt = ts4 * 4 + j
                                    for nl in range(NNL):
                                        nc.tensor.matmul(
                                            vps[:, j * P:(j + 1) * P],
                                            ckvT[nl][:, kt * P:(kt + 1) * P],
                                            wuvT[nl][:, hh * P:(hh + 1) * P],
                                            start=(nl == 0),
                                            stop=(nl == NNL - 1),
                                            skip_group_check=True,
                                        )
                                ecopy(
                                    vv[hh][:, ts4 * 4:(ts4 + 1) * 4, :],
                                    vps[:].rearrange("p (a b) -> p a b", a=4),
                                    pin="act",
                                )
                        # rope for the pair: rows [Are, Bre, Aim, Bim]
                        roq = pb2.tile([P, T], F8, tag="roq", bufs=2, name="roq")
                        nc.vector.tensor_mul(rtmp[0:64, :], qraw[64:128, :],
                                             sa4[64:128, :])
                        nc.vector.tensor_mul(rtmp[64:128, :], qraw[64:128, :],
                                             ca4[64:128, :])
                        nc.vector.tensor_mul(rro[0:64, :], qraw[0:64, :],
                                             ca4[0:64, :])
                        nc.vector.tensor_mul(rro[64:128, :], qraw[0:64, :],
                                             sa4[0:64, :])
                        nc.vector.tensor_sub(roq[0:64, :], rro[0:64, :],
                                             rtmp[0:64, :])
                        nc.vector.tensor_add(roq[64:128, :], rro[64:128, :],
                                             rtmp[64:128, :])
                        nc.gpsimd.dma_start(out=q8[hA][0:32, 1, :], in_=roq[0:32, :])
                        nc.gpsimd.dma_start(out=q8[hA][32:64, 1, :], in_=roq[64:96, :])
                        nc.gpsimd.dma_start(out=q8[hB][0:32, 1, :], in_=roq[32:64, :])
                        nc.gpsimd.dma_start(out=q8[hB][32:64, 1, :], in_=roq[96:128, :])

                    # ============ attention + deferred norm/output proj =====
                    deferred_b = deque()   # normalization tails
                    outproj_q = deque()    # (ohs, tq, cs_start) groups

                    def emit_outproj_group():
                        g_ohs, g_tq, cs0 = outproj_q.popleft()
                        g_qsl = slice(g_tq * TCH, (g_tq + 1) * TCH)
                        for cs in range(cs0, cs0 + 4):
                            acc = pmm.tile([P, TCH], F32, tag="mm", name="mm")
                            for h2 in range(HLOC):
                                nc.tensor.matmul(
                                    acc[:],
                                    woT[h2][:, cs * P:(cs + 1) * P],
                                    g_ohs[h2][:],
                                    start=(h2 == 0),
                                    stop=(h2 == HLOC - 1),
                                )
                            ot = pat.tile([P, TCH], F32, tag="ot", bufs=3,
                                          name="ot")
                            ecopy(ot[:], acc[:], pin="dve")
                            nc.sync.dma_start(
                                out_ext.ap()[cs * P:(cs + 1) * P, g_qsl], ot[:])

                    ohs_by_tq = {}
                    for tq in range(NCH):
                        qsl = slice(tq * TCH, (tq + 1) * TCH)
                        ohs = []
                        ohs_by_tq[tq] = ohs
                        for h in range(HLOC):
                            outU = pou.tile([P, TCH], F32, tag="ou", name="ou")
                            den = pde.tile([P, 4], F32, tag="de", name="de")
                            nc.vector.memset(den[:], 0.0)
                            nsub = (tq + 1) * 4
                            pend = deque()

                            def flush_one(outU=outU, den=den, nsub=nsub, h=h,
                                          tq=tq, pend=pend):
                                Pt, kt = pend.popleft()
                                kc, ks = kt // 4, kt % 4
                                off = ks * P if kc == tq else 0
                                for qq in range(off // P, 4):
                                    nc.tensor.matmul(
                                        den[:, qq:qq + 1],
                                        Pt[:, qq * P:(qq + 1) * P],
                                        onescol[:],
                                        start=False,
                                        stop=(kt == nsub - 1),
                                        skip_group_check=True,
                                    )
                                nc.tensor.matmul(
                                    outU[:, off:TCH],
                                    vv[h][:, kt, :],
                                    Pt[:, off:TCH],
                                    start=(kt == 0),
                                    stop=(kt == nsub - 1),
                                    skip_group_check=True,
                                )

                            for kt in range(nsub):
                                kc, ks = kt // 4, kt % 4
                                diag = kc == tq
                                off = ks * P if diag else 0
                                npr = TCH - off
                                ST = pmm.tile([P, TCH], F32, tag="mm", name="mm")
                                nc.tensor.matmul(
                                    ST[:, 0:npr],
                                    k8[h][:, :, kt * P:(kt + 1) * P],
                                    q8[h][:, :, qsl.start + off:qsl.stop],
                                    perf_mode=DR,
                                    start=True,
                                    stop=True,
                                )
                                if diag:
                                    nc.vector.tensor_add(
                                        ST[:, 0:P], ST[:, 0:P],
                                        maskbuf[:, 384:384 + P],
                                    )
                                Pt = pat.tile([P, TCH], BF16, tag="pt", bufs=6,
                                              name="pt")
                                if off:
                                    nc.vector.memset(Pt[:, 0:off], 0.0)
                                nc.scalar.activation(Pt[:, off:TCH], ST[:, 0:npr],
                                                     Exp, scale=SCALE)
                                pend.append((Pt, kt))
                                if len(pend) > 2:
                                    flush_one()
                                if kt == 2:
                                    # emit previous head's deferred tail and
                                    # one output-projection group here, where
                                    # PE has score work queued to hide them
                                    if deferred_b:
                                        deferred_b.popleft()()
                                    if outproj_q:
                                        emit_outproj_group()
                            while pend:
                                flush_one()

                            # normalization head: transpose+recip now (DVE),
                            # broadcast+apply deferred under the next head
                            den_sb = pat.tile([P, 4], F32R, tag="dsb", bufs=2,
                                              name="dsb")
                            nc.vector.tensor_copy(den_sb[:], den[:])
                            tpd = ptp.tile([P, 2 * P], F32R, tag="tp", name="tp")
                            nc.tensor.transpose(tpd[0:4, 0:P], den_sb[:], ident[:])
                            rec = pat.tile([4, P], F32, tag="rec", bufs=2,
                                           name="rec")
                            nc.vector.reciprocal(rec[:], tpd[0:4, 0:P])
                            recb = pat.tile([4, P], BF16, tag="recb", bufs=2,
                                            name="recb")
                            nc.vector.tensor_copy(recb[:], rec[:])
                            oh = pat.tile([P, TCH], BF16, tag=f"oh{h}", bufs=2,
                                          name=f"oh{h}")
                            ohs.append(oh)

                            def norm_tail(recb=recb, outU=outU, oh=oh, h=h,
                                          tq=tq):
                                bc = pmm.tile([P, TCH], F32, tag="mm", name="mm")
                                for qq in range(4):
                                    nc.tensor.matmul(
                                        bc[:, qq * P:(qq + 1) * P],
                                        sel4[:, qq * P:(qq + 1) * P],
                                        recb[:],
                                        start=True, stop=True,
                                        skip_group_check=True,
                                    )
                                bcs = pat.tile([P, TCH], BF16, tag="bcs",
                                               bufs=2, name="bcs")
                                nc.vector.tensor_copy(bcs[:], bc[:])
                                nc.vector.tensor_mul(oh[:], outU[:], bcs[:])
                                if h == HLOC - 1:
                                    for cs0 in range(0, NCT, 4):
                                        outproj_q.append(
                                            (ohs_by_tq[tq], tq, cs0))

                            deferred_b.append(norm_tail)

                    # drain deferred work
                    while deferred_b:
                        deferred_b.popleft()()
                    while outproj_q:
                        emit_outproj_group()

                pb2_ctx.__exit__(None, None, None)

    nc.compile()
    return nc


def _get_nc():
    if "nc" not in _NC_CACHE:
        _NC_CACHE["nc"] = build()
    return _NC_CACHE["nc"]


def kernel(x, freqs_cos, freqs_sin, W_dq, W_uq, W_dkv, W_uk, W_uv, W_qr, W_kr,
           W_o, trace=False, **trace_kwargs):
    nc = _get_nc()
    f32 = lambda a: np.ascontiguousarray(np.asarray(a, dtype=np.float32))
    x = f32(x); W_dq = f32(W_dq); W_uq = f32(W_uq); W_dkv = f32(W_dkv)
    W_uk = f32(W_uk); W_uv = f32(W_uv); W_qr = f32(W_qr); W_kr = f32(W_kr)
    W_o = f32(W_o)
    cos = f32(freqs_cos); sin = f32(freqs_sin)

    in_maps = []
    for c in range(8):
        b, r = divmod(c, 4)
        in_maps.append({
            "x": x[b],
            "wdq": W_dq, "wdkv": W_dkv, "wkr": W_kr,
            "wuq": W_uq[r * HLOC * HS:(r + 1) * HLOC * HS],
            "wuk": W_uk[r * HLOC * HS:(r + 1) * HLOC * HS],
            "wuv": W_uv[r * HLOC * HS:(r + 1) * HLOC * HS],
            "wqr": W_qr[r * HLOC * RHD:(r + 1) * HLOC * RHD],
            "wo": W_o[:, r * HLOC * HS:(r + 1) * HLOC * HS],
            "cos": cos, "sin": sin,
        })
    res = run_bass_kernel_spmd(nc, in_maps, core_ids=list(range(8)),
                               trace=trace, **trace_kwargs)
    out = np.zeros((B, T, C), dtype=np.float32)
    for c in range(8):
        b = c // 4
        out[b] += res.results[c]["out"].T
    kernel.last_result = res
    return out


# revision 41
# speedup vs baseline: 1.4919x; 1.0778x over previous
"""MLA-style attention kernel for 8 TRN2 NeuronCores, v3.

Sharding: core c -> batch b = c//4, heads r*4..r*4+3 where r = c%4.
The latent down-projections are REPLICATED within each 4-core batch
group (no collective, no cross-core dependency): each core computes the
full-T latents c_q/c_kv/k_r from the full x[b], then its 4 heads'
attention and a partial output projection summed on the host.

All activations stay SBUF-resident in a transposed [feature, T] layout.
Down/up-projections run in bf16 (PSUM fp32 accumulate).  Scores use
fp8e4 with DoubleRow perf mode: q/k packed as [128, 2, T] fp8 where
slot 0 holds the 128 content dims and slot 1 rows 0:64 hold the roped
rope dims (planar re/im), rows 64:128 zero.  One DoubleRow matmul per
512x128 score subtile (4x fewer PE cycles than two f32r matmuls).
Softmax denominators use N=1 ones-column matmuls accumulating into a
[128q, 4] PSUM tile (start=False onto memset zeros -- a start=True
would wipe sibling columns through the 2KB zero-region), then
transpose + reciprocal + selector broadcast matmul for normalization.
Diagonal causal blocks shrink the score matmul to the valid q-range,
memset the dead Pt columns, and add a fixed 128-wide triangular mask.
exp() pipelines two subtiles deep; the normalization tail of head h and
the output projection of chunk tq-1 are emitted under the NEXT head's
score loop so PE never drains while Act works.  V is produced directly
in natural [t, hs] layout (lhsT = ckv^T): no transposes after phase A.
Both hardware DMA queues are used: SP for W_dq/W_dkv/x/output, Act for
cos/sin, W_u/W_qr/W_o, mask, and the SBUF-to-SBUF fp8 slot copies.
"""
import math
from collections import deque
import numpy as np

import concourse.bass as bass
import concourse.bacc as bacc
import concourse.mybir as mybir
import concourse.tile as tile
from concourse.bass_utils import run_bass_kernel_spmd

F32 = mybir.dt.float32
F32R = mybir.dt.float32r
BF16 = mybir.dt.bfloat16
F8 = mybir.dt.float8e4
Exp = mybir.ActivationFunctionType.Exp
DR = mybir.MatmulPerfMode.DoubleRow

B, T, C = 2, 2048, 2048
H = 16
HS = 128
NL = 512
RHD = 64
HLOC = 4              # heads per core
P = 128
NNL = NL // P         # 4
NCT = C // P          # 16
TCH = 512
NCH = T // TCH        # 4
SCALE = 1.0 / math.sqrt(HS + RHD)
NEG = -1.0e30

_NC_CACHE = {}


def _deint(ap2d):
    # [p, 2d] -> (evens [p, d], odds [p, d]) along the free dim
    rr = ap2d.rearrange("p (d two) -> p two d", two=2)
    return rr[:, 0, :], rr[:, 1, :]


def build():
    nc = bacc.Bacc("TRN2", target_bir_lowering=False, debug=False, num_devices=8)

    x_ext = nc.dram_tensor("x", [T, C], F32R, kind="ExternalInput")
    wdq_ext = nc.dram_tensor("wdq", [NL, C], F32R, kind="ExternalInput")
    wdkv_ext = nc.dram_tensor("wdkv", [NL, C], F32R, kind="ExternalInput")
    wkr_ext = nc.dram_tensor("wkr", [RHD, C], F32R, kind="ExternalInput")
    wuq_ext = nc.dram_tensor("wuq", [HLOC * HS, NL], F32R, kind="ExternalInput")
    wuk_ext = nc.dram_tensor("wuk", [HLOC * HS, NL], F32R, kind="ExternalInput")
    wuv_ext = nc.dram_tensor("wuv", [HLOC * HS, NL], F32R, kind="ExternalInput")
    wqr_ext = nc.dram_tensor("wqr", [HLOC * RHD, NL], F32R, kind="ExternalInput")
    wo_ext = nc.dram_tensor("wo", [C, HLOC * HS], F32R, kind="ExternalInput")
    cos_ext = nc.dram_tensor("cos", [T, RHD // 2], F32R, kind="ExternalInput")
    sin_ext = nc.dram_tensor("sin", [T, RHD // 2], F32R, kind="ExternalInput")
    out_ext = nc.dram_tensor("out", [C, T], F32, kind="ExternalOutput")

    ident_dram = nc.inline_tensor(np.eye(P, dtype=np.float32), name="identc")
    # triangular mask for the 128-wide diagonal band of shrunk S^T tiles
    m = np.full((P, 896), NEG, dtype=np.float32)
    for jj in range(P):
        m[jj, 384 + jj:] = 0.0
    masks_dram = nc.inline_tensor(m, name="maskc")
    # row selector for the 1/den broadcast: sel4[k, qq*128+j] = (k == qq)
    sel = np.zeros((4, 512), dtype=np.float32)
    for qq in range(4):
        sel[qq, qq * P:(qq + 1) * P] = 1.0
    sel4_dram = nc.inline_tensor(sel, name="sel4c")

    with tile.TileContext(nc) as tc:
        with tc.tile_pool(name="pers", bufs=1) as pers:
            ptp_cell = [None]
            ident = pers.tile([P, P], F32R, tag="ident", name="ident")
            nc.sync.dma_start(ident[:], ident_dram.ap().bitcast(F32R))
            maskbuf = pers.tile([P, 896], BF16, tag="maskbuf", name="maskbuf")
            nc.gpsimd.dma_start(out=maskbuf[:], in_=masks_dram.ap())
            onescol = pers.tile([P, 1], BF16, tag="onescol", name="onescol")
            nc.vector.memset(onescol[:], 1.0)
            sel4 = pers.tile([4, TCH], BF16, tag="sel4", name="sel4")
            nc.gpsimd.dma_start(out=sel4[:], in_=sel4_dram.ap())

            # rope tables, cos/sin duplicated on all four 32-row groups
            ca4 = pers.tile([P, T], BF16, tag="ca4", name="ca4")
            sa4 = pers.tile([P, T], BF16, tag="sa4", name="sa4")

            # full-T latents (bf16, [feat, T])
            cqT = [pers.tile([P, T], BF16, tag=f"cqT{i}", name=f"cqT{i}")
                   for i in range(NNL)]
            ckvT = [pers.tile([P, T], BF16, tag=f"ckvT{i}", name=f"ckvT{i}")
                    for i in range(NNL)]
            krraw = pers.tile([RHD, T], BF16, tag="krraw", name="krraw")

            _ecnt = [0]

            def ecopy(dst, src, pin=None):
                """PSUM->SBUF evacuation copy, alternating Act/DVE."""
                _ecnt[0] += 1
                eng = pin if pin else ("act" if _ecnt[0] % 2 else "dve")
                if eng == "act":
                    nc.scalar.copy(dst, src)
                else:
                    nc.vector.tensor_copy(dst, src)

            def transpose_pair_into(dst_ap, srcA, srcB, pin=None):
                tp2 = ptp_cell[0].tile([P, 2 * P], F32R, tag="tp", name="tp")
                nc.tensor.transpose(tp2[:, 0:P], srcA, ident[:])
                nc.tensor.transpose(tp2[:, P:2 * P], srcB, ident[:])
                ecopy(dst_ap, tp2[:], pin=pin)

            # ============ phase A: x chunks + all weight prep, interleaved ==
            with tc.tile_pool(name="pb", bufs=1) as pb:
                # -- persistent-ish weight destinations (pb outlives phase A)
                wuqT = [pb.tile([P, HLOC * HS], BF16, tag=f"wuqT{i}",
                                name=f"wuqT{i}") for i in range(NNL)]
                wukT = [pb.tile([P, HLOC * HS], BF16, tag=f"wukT{i}",
                                name=f"wukT{i}") for i in range(NNL)]
                wuvT = [pb.tile([P, HLOC * HS], BF16, tag=f"wuvT{i}",
                                name=f"wuvT{i}") for i in range(NNL)]
                wqrT = [[pb.tile([P, P], BF16, tag=f"wqrT{g}{i}",
                                 name=f"wqrT{g}{i}") for i in range(NNL)]
                        for g in range(2)]
                woT = [pb.tile([P, C], BF16, tag=f"woT{i}", name=f"woT{i}")
                       for i in range(HLOC)]

                pa_ctx = (
                    tc.tile_pool(name="pa", bufs=1),
                    tc.tile_pool(name="pacc", bufs=1, space="PSUM"),
                    tc.tile_pool(name="ptpA", bufs=3, space="PSUM"),
                )
                pa = pa_ctx[0].__enter__()
                pacc = pa_ctx[1].__enter__()
                ptp_cell[0] = pa_ctx[2].__enter__()

                wdqT = [pa.tile([P, NL], BF16, tag=f"wdqT{i}", name=f"wdqT{i}")
                        for i in range(NCT)]
                wdkvT = [pa.tile([P, NL], BF16, tag=f"wdkvT{i}",
                                 name=f"wdkvT{i}") for i in range(NCT)]
                wkrT = [pa.tile([P, RHD], BF16, tag=f"wkrT{i}", name=f"wkrT{i}")
                        for i in range(NCT)]
                xT = [pa.tile([P, TCH], BF16, tag=f"xT{i}", name=f"xT{i}")
                      for i in range(NCT)]

                def x_chunk_transpose(tch):
                    t0 = tch * TCH
                    for sp in range(2):
                        rA = slice(t0 + 2 * sp * P, t0 + (2 * sp + 1) * P)
                        rB = slice(t0 + (2 * sp + 1) * P, t0 + (2 * sp + 2) * P)
                        for hf in range(2):
                            cf = slice(hf * (C // 2), (hf + 1) * (C // 2))
                            xA = pa.tile([P, C // 2], F32R, tag="xA", bufs=2,
                                         name="xA")
                            xB = pa.tile([P, C // 2], F32R, tag="xB", bufs=2,
                                         name="xB")
                            nc.sync.dma_start(xA[:], x_ext.ap()[rA, cf])
                            nc.sync.dma_start(xB[:], x_ext.ap()[rB, cf])
                            for ci in range(NCT // 2):
                                transpose_pair_into(
                                    xT[hf * 8 + ci][:, 2 * sp * P:(2 * sp + 2) * P],
                                    xA[:, ci * P:(ci + 1) * P],
                                    xB[:, ci * P:(ci + 1) * P],
                                )

                def x_chunk_matmuls(tch):
                    t0 = tch * TCH
                    for wTs, dstT in ((wdqT, cqT), (wdkvT, ckvT)):
                        for j in range(NNL):
                            acc = pacc.tile([P, TCH], F32, tag=f"acc{j}",
                                            name=f"acc{j}")
                            for ci in range(NCT):
                                nc.tensor.matmul(
                                    acc[:],
                                    wTs[ci][:, j * P:(j + 1) * P],
                                    xT[ci][:],
                                    start=(ci == 0),
                                    stop=(ci == NCT - 1),
                                )
                            ecopy(dstT[j][:, t0:t0 + TCH], acc[:])
                    acck = pacc.tile([RHD, TCH], F32, tag="acck", name="acck")
                    for ci in range(NCT):
                        nc.tensor.matmul(
                            acck[:],
                            wkrT[ci][:],
                            xT[ci][:],
                            start=(ci == 0),
                            stop=(ci == NCT - 1),
                        )
                    ecopy(krraw[:, t0:t0 + TCH], acck[:], pin="act")

                def wd_prep():
                    for w_ext, wTs in ((wdq_ext, wdqT), (wdkv_ext, wdkvT)):
                        for rp in range(NL // P // 2):
                            rA = slice(2 * rp * P, (2 * rp + 1) * P)
                            rB = slice((2 * rp + 1) * P, (2 * rp + 2) * P)
                            for hf in range(2):
                                cf = slice(hf * (C // 2), (hf + 1) * (C // 2))
                                sA = pa.tile([P, C // 2], F32R, tag="wsA",
                                             bufs=2, name="wsA")
                                sB = pa.tile([P, C // 2], F32R, tag="wsB",
                                             bufs=2, name="wsB")
                                nc.sync.dma_start(sA[:], w_ext.ap()[rA, cf])
                                nc.sync.dma_start(sB[:], w_ext.ap()[rB, cf])
                                for ci in range(NCT // 2):
                                    transpose_pair_into(
                                        wTs[hf * 8 + ci][:, 2 * rp * P:(2 * rp + 2) * P],
                                        sA[:, ci * P:(ci + 1) * P],
                                        sB[:, ci * P:(ci + 1) * P],
                                    )
                    kstrip = pa.tile([RHD, C], F32R, tag="kstrip", name="kstrip")
                    nc.sync.dma_start(kstrip[:], wkr_ext.ap())
                    for ci in range(NCT):
                        tp = ptp_cell[0].tile([P, 2 * P], F32R, tag="tp",
                                              name="tp")
                        nc.tensor.transpose(
                            tp[:, :RHD], kstrip[:, ci * P:(ci + 1) * P],
                            ident[:RHD, :RHD])
                        ev, od = _deint(tp[:, :RHD])
                        nc.scalar.copy(wkrT[ci][:, 0:32], ev)
                        nc.scalar.copy(wkrT[ci][:, 32:64], od)

                def table_prep():
                    for s in range(T // P):
                        cst = pa.tile([P, RHD // 2], F32R, tag="cst", bufs=2,
                                      name="cst")
                        sst = pa.tile([P, RHD // 2], F32R, tag="sst", bufs=2,
                                      name="sst")
                        nc.sync.dma_start(cst[:],
                                          cos_ext.ap()[s * P:(s + 1) * P, :])
                        nc.sync.dma_start(sst[:],
                                          sin_ext.ap()[s * P:(s + 1) * P, :])
                        tp = ptp_cell[0].tile([P, 2 * P], F32R, tag="tp",
                                              name="tp")
                        nc.tensor.transpose(tp[:32, 0:P], cst[:], ident[:])
                        nc.tensor.transpose(tp[:32, P:2 * P], sst[:], ident[:])
                        nc.vector.tensor_copy(ca4[0:32, s * P:(s + 1) * P],
                                              tp[:32, 0:P])
                        nc.vector.tensor_copy(sa4[0:32, s * P:(s + 1) * P],
                                              tp[:32, P:2 * P])
                    for d in range(1, 4):
                        nc.vector.tensor_copy(ca4[32 * d:32 * (d + 1), :],
                                              ca4[0:32, :])
                        nc.vector.tensor_copy(sa4[32 * d:32 * (d + 1), :],
                                              sa4[0:32, :])

                def wu_prep():
                    for w_ext, wT in ((wuq_ext, wuqT), (wuk_ext, wukT),
                                      (wuv_ext, wuvT)):
                        for rp in range(HLOC * HS // P // 2):
                            sA = pa.tile([P, NL], F32R, tag="usA", bufs=2,
                                         name="usA")
                            sB = pa.tile([P, NL], F32R, tag="usB", bufs=2,
                                         name="usB")
                            nc.sync.dma_start(
                                sA[:], w_ext.ap()[2 * rp * P:(2 * rp + 1) * P, :])
                            nc.sync.dma_start(
                                sB[:], w_ext.ap()[(2 * rp + 1) * P:(2 * rp + 2) * P, :])
                            for cs in range(NNL):
                                transpose_pair_into(
                                    wT[cs][:, 2 * rp * P:(2 * rp + 2) * P],
                                    sA[:, cs * P:(cs + 1) * P],
                                    sB[:, cs * P:(cs + 1) * P],
                                )

                def wo_wqr_prep():
                    for g in range(2):
                        strip = pa.tile([P, NL], F32R, tag="qrs", bufs=2,
                                        name="qrs")
                        nc.sync.dma_start(
                            strip[:], wqr_ext.ap()[g * P:(g + 1) * P, :])
                        for cs in range(NNL):
                            tp = ptp_cell[0].tile([P, 2 * P], F32R, tag="tp",
                                                  name="tp")
                            nc.tensor.transpose(
                                tp[:, 0:P], strip[:, cs * P:(cs + 1) * P],
                                ident[:])
                            evA, odA = _deint(tp[:, 0:RHD])
                            evB, odB = _deint(tp[:, RHD:2 * RHD])
                            nc.scalar.copy(wqrT[g][cs][:, 0:32], evA)
                            nc.scalar.copy(wqrT[g][cs][:, 32:64], evB)
                            nc.scalar.copy(wqrT[g][cs][:, 64:96], odA)
                            nc.scalar.copy(wqrT[g][cs][:, 96:128], odB)
                    for sp in range(C // P // 2):
                        oA = pa.tile([P, HLOC * HS], F32R, tag="osA", bufs=2,
                                     name="osA")
                        oB = pa.tile([P, HLOC * HS], F32R, tag="osB", bufs=2,
                                     name="osB")
                        nc.sync.dma_start(
                            oA[:], wo_ext.ap()[2 * sp * P:(2 * sp + 1) * P, :])
                        nc.sync.dma_start(
                            oB[:], wo_ext.ap()[(2 * sp + 1) * P:(2 * sp + 2) * P, :])
                        for fs in range(HLOC):
                            transpose_pair_into(
                                woT[fs][:, 2 * sp * P:(2 * sp + 2) * P],
                                oA[:, fs * P:(fs + 1) * P],
                                oB[:, fs * P:(fs + 1) * P],
                            )

                # interleave: x transposes first so PE starts immediately,
                # weight preps slot between chunks while x DMA streams.
                x_chunk_transpose(0)
                wd_prep()
                x_chunk_matmuls(0)
                x_chunk_transpose(1)
                table_prep()
                x_chunk_matmuls(1)
                x_chunk_transpose(2)
                wu_prep()
                x_chunk_matmuls(2)
                x_chunk_transpose(3)
                wo_wqr_prep()
                x_chunk_matmuls(3)

                pa_ctx[2].__exit__(None, None, None)
                pa_ctx[1].__exit__(None, None, None)
                pa_ctx[0].__exit__(None, None, None)

                # ============ phase B: rope, up-projections, fp8 packs ======
                pb2_ctx = tc.tile_pool(name="pb2", bufs=1)
                pb2 = pb2_ctx.__enter__()
                q8 = [pb2.tile([P, 2, T], F8, tag=f"q8{h}", name=f"q8{h}")
                      for h in range(HLOC)]
                k8 = [pb2.tile([P, 2, T], F8, tag=f"k8{h}", name=f"k8{h}")
                      for h in range(HLOC)]
                vv = [pb2.tile([P, T // P, P], BF16, tag=f"vv{h}", name=f"vv{h}")
                      for h in range(HLOC)]
                for h in range(HLOC):
                    nc.vector.memset(q8[h][64:128, 1, :], 0.0)
                    nc.gpsimd.memset(k8[h][64:128, 1, :], 0.0)

                with (
                    tc.tile_pool(name="pmm", bufs=3, space="PSUM") as pmm,
                    tc.tile_pool(name="pou", bufs=2, space="PSUM") as pou,
                    tc.tile_pool(name="pde", bufs=2, space="PSUM") as pde,
                    tc.tile_pool(name="pat", bufs=1) as pat,
                ):
                    # k_r rope -> krf8 (planar re/im), shared across heads
                    krf8 = pb2.tile([RHD, T], F8, tag="krf8", name="krf8")
                    rtmp = pb2.tile([P, T], BF16, tag="rtmp", name="rtmp")
                    rro = pb2.tile([P, T], BF16, tag="rro", name="rro")
                    nc.vector.tensor_mul(rtmp[0:32, :], krraw[32:64, :], sa4[32:64, :])
                    nc.vector.tensor_mul(rtmp[32:64, :], krraw[32:64, :], ca4[32:64, :])
                    nc.vector.tensor_mul(rro[0:32, :], krraw[0:32, :], ca4[0:32, :])
                    nc.vector.tensor_mul(rro[32:64, :], krraw[0:32, :], sa4[0:32, :])
                    nc.vector.tensor_sub(krf8[0:32, :], rro[0:32, :], rtmp[0:32, :])
                    nc.vector.tensor_add(krf8[32:64, :], rro[32:64, :], rtmp[32:64, :])
                    for h in range(HLOC):
                        nc.sync.dma_start(k8[h][0:RHD, 1, :], krf8[:])

                    # up-projections, head-pair at a time
                    for g in range(2):
                        hA, hB = 2 * g, 2 * g + 1
                        qraw = pb2.tile([P, T], BF16, tag="qraw", name="qraw")
                        for ch in range(NCH):
                            sl = slice(ch * TCH, (ch + 1) * TCH)
                            for hh in (hA, hB):
                                for wT, src, dst in ((wuqT, cqT, q8),
                                                     (wukT, ckvT, k8)):
                                    acc = pmm.tile([P, TCH], F32, tag="mm",
                                                   name="mm")
                                    for nl in range(NNL):
                                        nc.tensor.matmul(
                                            acc[:],
                                            wT[nl][:, hh * P:(hh + 1) * P],
                                            src[nl][:, sl],
                                            start=(nl == 0),
                                            stop=(nl == NNL - 1),
                                        )
                                    ecopy(dst[hh][:, 0, sl], acc[:], pin="act")
                            qacc = pmm.tile([P, TCH], F32, tag="mm", name="mm")
                            for nl in range(NNL):
                                nc.tensor.matmul(
                                    qacc[:],
                                    wqrT[g][nl][:],
                                    cqT[nl][:, sl],
                                    start=(nl == 0),
                                    stop=(nl == NNL - 1),
                                )
                            ecopy(qraw[:, sl], qacc[:], pin="act")
                        # natural-layout V for both heads
                        for hh in (hA, hB):
                            for ts4 in range(T // TCH):
                                vps = pmm.tile([P, TCH], F32, tag="mm", name="mm")
                                for j in range(4):
                                    kt = ts4 * 4 + j
                                    for nl in range(NNL):
                                        nc.tensor.matmul(
                                            vps[:, j * P:(j + 1) * P],
                                            ckvT[nl][:, kt * P:(kt + 1) * P],
                                            wuvT[nl][:, hh * P:(hh + 1) * P],
                                            start=(nl == 0),
                                            stop=(nl == NNL - 1),
                                            skip_group_check=True,
                                        )
                                ecopy(
                                    vv[hh][:, ts4 * 4:(ts4 + 1) * 4, :],
                                    vps[:].rearrange("p (a b) -> p a b", a=4),
                                    pin="act",
                                )
                        # rope for the pair: rows [Are, Bre, Aim, Bim]
                        roq = pb2.tile([P, T], F8, tag="roq", bufs=2, name="roq")
                        nc.vector.tensor_mul(rtmp[0:64, :], qraw[64:128, :],
                                             sa4[64:128, :])
                        nc.vector.tensor_mul(rtmp[64:128, :], qraw[64:128, :],
                                             ca4[64:128, :])
                        nc.vector.tensor_mul(rro[0:64, :], qraw[0:64, :],
                                             ca4[0:64, :])
                        nc.vector.tensor_mul(rro[64:128, :], qraw[0:64, :],
                                             sa4[0:64, :])
                        nc.vector.tensor_sub(roq[0:64, :], rro[0:64, :],
                                             rtmp[0:64, :])
                        nc.vector.tensor_add(roq[64:128, :], rro[64:128, :],
                                             rtmp[64:128, :])
                        nc.sync.dma_start(q8[hA][0:32, 1, :], roq[0:32, :])
                        nc.sync.dma_start(q8[hA][32:64, 1, :], roq[64:96, :])
                        nc.sync.dma_start(q8[hB][0:32, 1, :], roq[32:64, :])
                        nc.sync.dma_start(q8[hB][32:64, 1, :], roq[96:128, :])

                    # ============ attention + deferred norm/output proj =====
                    deferred_b = deque()   # normalization tails
                    outproj_q = deque()    # (ohs, tq, cs_start) groups

                    def emit_outproj_group():
                        g_ohs, g_tq, cs0 = outproj_q.popleft()
                        g_qsl = slice(g_tq * TCH, (g_tq + 1) * TCH)
                        for cs in range(cs0, cs0 + 4):
                            acc = pmm.tile([P, TCH], F32, tag="mm", name="mm")
                            for h2 in range(HLOC):
                                nc.tensor.matmul(
                                    acc[:],
                                    woT[h2][:, cs * P:(cs + 1) * P],
                                    g_ohs[h2][:],
                                    start=(h2 == 0),
                                    stop=(h2 == HLOC - 1),
                                )
                            ot = pat.tile([P, TCH], F32, tag="ot", bufs=3,
                                          name="ot")
                            ecopy(ot[:], acc[:], pin="dve")
                            nc.sync.dma_start(
                                out_ext.ap()[cs * P:(cs + 1) * P, g_qsl], ot[:])

                    ohs_by_tq = {}
                    for tq in range(NCH):
                        qsl = slice(tq * TCH, (tq + 1) * TCH)
                        ohs = []
                        ohs_by_tq[tq] = ohs
                        for h in range(HLOC):
                            outU = pou.tile([P, TCH], F32, tag="ou", name="ou")
                            den = pde.tile([P, 4], F32, tag="de", name="de")
                            nc.vector.memset(den[:], 0.0)
                            nsub = (tq + 1) * 4
                            pend = deque()

                            def flush_one(outU=outU, den=den, nsub=nsub, h=h,
                                          tq=tq, pend=pend):
                                Pt, kt = pend.popleft()
                                kc, ks = kt // 4, kt % 4
                                off = ks * P if kc == tq else 0
                                for qq in range(off // P, 4):
                                    nc.tensor.matmul(
                                        den[:, qq:qq + 1],
                                        Pt[:, qq * P:(qq + 1) * P],
                                        onescol[:],
                                        start=False,
                                        stop=(kt == nsub - 1),
                                        skip_group_check=True,
                                    )
                                nc.tensor.matmul(
                                    outU[:, off:TCH],
                                    vv[h][:, kt, :],
                                    Pt[:, off:TCH],
                                    start=(kt == 0),
                                    stop=(kt == nsub - 1),
                                    skip_group_check=True,
                                )

                            for kt in range(nsub):
                                kc, ks = kt // 4, kt % 4
                                diag = kc == tq
                                off = ks * P if diag else 0
                                npr = TCH - off
                                ST = pmm.tile([P, TCH], F32, tag="mm", name="mm")
                                nc.tensor.matmul(
                                    ST[:, 0:npr],
                                    k8[h][:, :, kt * P:(kt + 1) * P],
                                    q8[h][:, :, qsl.start + off:qsl.stop],
                                    perf_mode=DR,
                                    start=True,
                                    stop=True,
                                )
                                if diag:
                                    nc.vector.tensor_add(
                                        ST[:, 0:P], ST[:, 0:P],
                                        maskbuf[:, 384:384 + P],
                                    )
                                Pt = pat.tile([P, TCH], BF16, tag="pt", bufs=6,
                                              name="pt")
                                if off:
                                    nc.gpsimd.memset(Pt[:, 0:off], 0.0)
                                nc.scalar.activation(Pt[:, off:TCH], ST[:, 0:npr],
                                                     Exp, scale=SCALE)
                                pend.append((Pt, kt))
                                if len(pend) > 2:
                                    flush_one()
                                if kt == 2:
                                    # emit previous head's deferred tail and
                                    # one output-projection group here, where
                                    # PE has score work queued to hide them
                                    if deferred_b:
                                        deferred_b.popleft()()
                                    if outproj_q:
                                        emit_outproj_group()
                            while pend:
                                flush_one()

                            # normalization head: transpose+recip now (DVE),
                            # broadcast+apply deferred under the next head
                            den_sb = pat.tile([P, 4], F32R, tag="dsb", bufs=2,
                                              name="dsb")
                            nc.vector.tensor_copy(den_sb[:], den[:])
                            tpd = pde.tile([4, P], F32R, tag="tpd", bufs=1,
                                           name="tpd")
                            nc.tensor.transpose(tpd[0:4, 0:P], den_sb[:], ident[:])
                            rec = pat.tile([4, P], F32, tag="rec", bufs=2,
                                           name="rec")
                            nc.vector.reciprocal(rec[:], tpd[:])
                            recb = pat.tile([4, P], BF16, tag="recb", bufs=2,
                                            name="recb")
                            nc.vector.tensor_copy(recb[:], rec[:])
                            oh = pat.tile([P, TCH], BF16, tag=f"oh{h}", bufs=2,
                                          name=f"oh{h}")
                            ohs.append(oh)

                            def norm_tail(recb=recb, outU=outU, oh=oh, h=h,
                                          tq=tq):
                                bc = pmm.tile([P, TCH], F32, tag="mm", name="mm")
                                for qq in range(4):
                                    nc.tensor.matmul(
                                        bc[:, qq * P:(qq + 1) * P],
                                        sel4[:, qq * P:(qq + 1) * P],
                                        recb[:],
                                        start=True, stop=True,
                                        skip_group_check=True,
                                    )
                                bcs = pat.tile([P, TCH], BF16, tag="bcs",
                                               bufs=2, name="bcs")
                                nc.vector.tensor_copy(bcs[:], bc[:])
                                nc.vector.tensor_mul(oh[:], outU[:], bcs[:])
                                if h == HLOC - 1:
                                    for cs0 in range(0, NCT, 4):
                                        outproj_q.append(
                                            (ohs_by_tq[tq], tq, cs0))

                            deferred_b.append(norm_tail)

                    # drain deferred work
                    while deferred_b:
                        deferred_b.popleft()()
                    while outproj_q:
                        emit_outproj_group()

                pb2_ctx.__exit__(None, None, None)

    nc.compile()
    return nc


def _get_nc():
    if "nc" not in _NC_CACHE:
        _NC_CACHE["nc"] = build()
    return _NC_CACHE["nc"]


def kernel(x, freqs_cos, freqs_sin, W_dq, W_uq, W_dkv, W_uk, W_uv, W_qr, W_kr,
           W_o, trace=False, **trace_kwargs):
    nc = _get_nc()
    f32 = lambda a: np.ascontiguousarray(np.asarray(a, dtype=np.float32))
    x = f32(x); W_dq = f32(W_dq); W_uq = f32(W_uq); W_dkv = f32(W_dkv)
    W_uk = f32(W_uk); W_uv = f32(W_uv); W_qr = f32(W_qr); W_kr = f32(W_kr)
    W_o = f32(W_o)
    cos = f32(freqs_cos); sin = f32(freqs_sin)

    in_maps = []
    for c in range(8):
        b, r = divmod(c, 4)
        in_maps.append({
            "x": x[b],
            "wdq": W_dq, "wdkv": W_dkv, "wkr": W_kr,
            "wuq": W_uq[r * HLOC * HS:(r + 1) * HLOC * HS],
            "wuk": W_uk[r * HLOC * HS:(r + 1) * HLOC * HS],
            "wuv": W_uv[r * HLOC * HS:(r + 1) * HLOC * HS],
            "wqr": W_qr[r * HLOC * RHD:(r + 1) * HLOC * RHD],
            "wo": W_o[:, r * HLOC * HS:(r + 1) * HLOC * HS],
            "cos": cos, "sin": sin,
        })
    res = run_bass_kernel_spmd(nc, in_maps, core_ids=list(range(8)),
                               trace=trace, **trace_kwargs)
    out = np.zeros((B, T, C), dtype=np.float32)
    for c in range(8):
        b = c // 4
        out[b] += res.results[c]["out"].T
    kernel.last_result = res
    return out


# revision 42
# speedup vs baseline: 1.5404x; 1.0325x over previous
"""MLA-style attention kernel for 8 TRN2 NeuronCores, v3.

Sharding: core c -> batch b = c//4, heads r*4..r*4+3 where r = c%4.
The latent down-projections are REPLICATED within each 4-core batch
group (no collective, no cross-core dependency): each core computes the
full-T latents c_q/c_kv/k_r from the full x[b], then its 4 heads'
attention and a partial output projection summed on the host.

All activations stay SBUF-resident in a transposed [feature, T] layout.
Down/up-projections run in bf16 (PSUM fp32 accumulate).  Scores use
fp8e4 with DoubleRow perf mode: q/k packed as [128, 2, T] fp8 where
slot 0 holds the 128 content dims and slot 1 rows 0:64 hold the roped
rope dims (planar re/im), rows 64:128 zero.  One DoubleRow matmul per
512x128 score subtile (4x fewer PE cycles than two f32r matmuls).
Softmax denominators use N=1 ones-column matmuls accumulating into a
[128q, 4] PSUM tile (start=False onto memset zeros -- a start=True
would wipe sibling columns through the 2KB zero-region), then
transpose + reciprocal + selector broadcast matmul for normalization.
Diagonal causal blocks shrink the score matmul to the valid q-range,
memset the dead Pt columns, and add a fixed 128-wide triangular mask.
exp() pipelines two subtiles deep; the normalization tail of head h and
the output projection of chunk tq-1 are emitted under the NEXT head's
score loop so PE never drains while Act works.  V is produced directly
in natural [t, hs] layout (lhsT = ckv^T): no transposes after phase A.
Both hardware DMA queues are used: SP for W_dq/W_dkv/x/output, Act for
cos/sin, W_u/W_qr/W_o, mask, and the SBUF-to-SBUF fp8 slot copies.
"""
import math
from collections import deque
import numpy as np

import concourse.bass as bass
import concourse.bacc as bacc
import concourse.mybir as mybir
import concourse.tile as tile
from concourse.bass_utils import run_bass_kernel_spmd

F32 = mybir.dt.float32
F32R = mybir.dt.float32r
BF16 = mybir.dt.bfloat16
F8 = mybir.dt.float8e4
Exp = mybir.ActivationFunctionType.Exp
DR = mybir.MatmulPerfMode.DoubleRow

B, T, C = 2, 2048, 2048
H = 16
HS = 128
NL = 512
RHD = 64
HLOC = 4              # heads per core
P = 128
NNL = NL // P         # 4
NCT = C // P          # 16
TCH = 512
NCH = T // TCH        # 4
SCALE = 1.0 / math.sqrt(HS + RHD)
NEG = -1.0e30

_NC_CACHE = {}


def _deint(ap2d):
    # [p, 2d] -> (evens [p, d], odds [p, d]) along the free dim
    rr = ap2d.rearrange("p (d two) -> p two d", two=2)
    return rr[:, 0, :], rr[:, 1, :]


def build():
    nc = bacc.Bacc("TRN2", target_bir_lowering=False, debug=False, num_devices=8)

    x_ext = nc.dram_tensor("x", [T, C], F32R, kind="ExternalInput")
    wdq_ext = nc.dram_tensor("wdq", [NL, C], F32R, kind="ExternalInput")
    wdkv_ext = nc.dram_tensor("wdkv", [NL, C], F32R, kind="ExternalInput")
    wkr_ext = nc.dram_tensor("wkr", [RHD, C], F32R, kind="ExternalInput")
    wuq_ext = nc.dram_tensor("wuq", [HLOC * HS, NL], F32R, kind="ExternalInput")
    wuk_ext = nc.dram_tensor("wuk", [HLOC * HS, NL], F32R, kind="ExternalInput")
    wuv_ext = nc.dram_tensor("wuv", [HLOC * HS, NL], F32R, kind="ExternalInput")
    wqr_ext = nc.dram_tensor("wqr", [HLOC * RHD, NL], F32R, kind="ExternalInput")
    wo_ext = nc.dram_tensor("wo", [C, HLOC * HS], F32R, kind="ExternalInput")
    cos_ext = nc.dram_tensor("cos", [T, RHD // 2], F32R, kind="ExternalInput")
    sin_ext = nc.dram_tensor("sin", [T, RHD // 2], F32R, kind="ExternalInput")
    out_ext = nc.dram_tensor("out", [C, T], F32, kind="ExternalOutput")

    ident_dram = nc.inline_tensor(np.eye(P, dtype=np.float32), name="identc")
    # triangular mask for the 128-wide diagonal band of shrunk S^T tiles
    m = np.full((P, 896), NEG, dtype=np.float32)
    for jj in range(P):
        m[jj, 384 + jj:] = 0.0
    masks_dram = nc.inline_tensor(m, name="maskc")
    # row selector for the 1/den broadcast: sel4[k, qq*128+j] = (k == qq)
    sel = np.zeros((4, 512), dtype=np.float32)
    for qq in range(4):
        sel[qq, qq * P:(qq + 1) * P] = 1.0
    sel4_dram = nc.inline_tensor(sel, name="sel4c")

    with tile.TileContext(nc) as tc:
        with tc.tile_pool(name="pers", bufs=1) as pers:
            ptp_cell = [None]
            ident = pers.tile([P, P], F32R, tag="ident", name="ident")
            nc.sync.dma_start(ident[:], ident_dram.ap().bitcast(F32R))
            maskbuf = pers.tile([P, 896], BF16, tag="maskbuf", name="maskbuf")
            nc.gpsimd.dma_start(out=maskbuf[:], in_=masks_dram.ap())
            onescol = pers.tile([P, 1], BF16, tag="onescol", name="onescol")
            nc.vector.memset(onescol[:], 1.0)
            identb = pers.tile([P, P], BF16, tag="identb", name="identb")
            nc.vector.tensor_copy(identb[:], ident[:])
            sel4 = pers.tile([4, TCH], BF16, tag="sel4", name="sel4")
            nc.gpsimd.dma_start(out=sel4[:], in_=sel4_dram.ap())

            # rope tables, cos/sin duplicated on all four 32-row groups
            ca4 = pers.tile([P, T], BF16, tag="ca4", name="ca4")
            sa4 = pers.tile([P, T], BF16, tag="sa4", name="sa4")

            # full-T latents (bf16, [feat, T])
            cqT = [pers.tile([P, T], BF16, tag=f"cqT{i}", name=f"cqT{i}")
                   for i in range(NNL)]
            ckvT = [pers.tile([P, T], BF16, tag=f"ckvT{i}", name=f"ckvT{i}")
                    for i in range(NNL)]
            krraw = pers.tile([RHD, T], BF16, tag="krraw", name="krraw")

            _ecnt = [0]

            def ecopy(dst, src, pin=None):
                """PSUM->SBUF evacuation copy, alternating Act/DVE."""
                _ecnt[0] += 1
                eng = pin if pin else ("act" if _ecnt[0] % 2 else "dve")
                if eng == "act":
                    nc.scalar.copy(dst, src)
                else:
                    nc.vector.tensor_copy(dst, src)

            def transpose_pair_into(dst_ap, srcA, srcB, pin=None):
                tp2 = ptp_cell[0].tile([P, 2 * P], F32R, tag="tp", name="tp")
                nc.tensor.transpose(tp2[:, 0:P], srcA, ident[:])
                nc.tensor.transpose(tp2[:, P:2 * P], srcB, ident[:])
                ecopy(dst_ap, tp2[:], pin=pin)

            # ============ phase A: x chunks + all weight prep, interleaved ==
            with tc.tile_pool(name="pb", bufs=1) as pb:
                # -- persistent-ish weight destinations (pb outlives phase A)
                wuqT = [pb.tile([P, HLOC * HS], BF16, tag=f"wuqT{i}",
                                name=f"wuqT{i}") for i in range(NNL)]
                wukT = [pb.tile([P, HLOC * HS], BF16, tag=f"wukT{i}",
                                name=f"wukT{i}") for i in range(NNL)]
                wuvT = [pb.tile([P, HLOC * HS], BF16, tag=f"wuvT{i}",
                                name=f"wuvT{i}") for i in range(NNL)]
                wqrT = [[pb.tile([P, P], BF16, tag=f"wqrT{g}{i}",
                                 name=f"wqrT{g}{i}") for i in range(NNL)]
                        for g in range(2)]
                woT = [pb.tile([P, C], BF16, tag=f"woT{i}", name=f"woT{i}")
                       for i in range(HLOC)]

                pa_ctx = (
                    tc.tile_pool(name="pa", bufs=1),
                    tc.tile_pool(name="pacc", bufs=1, space="PSUM"),
                    tc.tile_pool(name="ptpA", bufs=3, space="PSUM"),
                )
                pa = pa_ctx[0].__enter__()
                pacc = pa_ctx[1].__enter__()
                ptp_cell[0] = pa_ctx[2].__enter__()

                wdqT = [pa.tile([P, NL], BF16, tag=f"wdqT{i}", name=f"wdqT{i}")
                        for i in range(NCT)]
                wdkvT = [pa.tile([P, NL], BF16, tag=f"wdkvT{i}",
                                 name=f"wdkvT{i}") for i in range(NCT)]
                wkrT = [pa.tile([P, RHD], BF16, tag=f"wkrT{i}", name=f"wkrT{i}")
                        for i in range(NCT)]
                xT = [pa.tile([P, TCH], BF16, tag=f"xT{i}", name=f"xT{i}")
                      for i in range(NCT)]

                def x_chunk_transpose(tch):
                    t0 = tch * TCH
                    for sp in range(2):
                        rA = slice(t0 + 2 * sp * P, t0 + (2 * sp + 1) * P)
                        rB = slice(t0 + (2 * sp + 1) * P, t0 + (2 * sp + 2) * P)
                        for hf in range(2):
                            cf = slice(hf * (C // 2), (hf + 1) * (C // 2))
                            xA = pa.tile([P, C // 2], F32R, tag="xA", bufs=2,
                                         name="xA")
                            xB = pa.tile([P, C // 2], F32R, tag="xB", bufs=2,
                                         name="xB")
                            nc.sync.dma_start(xA[:], x_ext.ap()[rA, cf])
                            nc.sync.dma_start(xB[:], x_ext.ap()[rB, cf])
                            for ci in range(NCT // 2):
                                transpose_pair_into(
                                    xT[hf * 8 + ci][:, 2 * sp * P:(2 * sp + 2) * P],
                                    xA[:, ci * P:(ci + 1) * P],
                                    xB[:, ci * P:(ci + 1) * P],
                                )

                def x_chunk_matmuls(tch):
                    t0 = tch * TCH
                    for wTs, dstT in ((wdqT, cqT), (wdkvT, ckvT)):
                        for j in range(NNL):
                            acc = pacc.tile([P, TCH], F32, tag=f"acc{j}",
                                            name=f"acc{j}")
                            for ci in range(NCT):
                                nc.tensor.matmul(
                                    acc[:],
                                    wTs[ci][:, j * P:(j + 1) * P],
                                    xT[ci][:],
                                    start=(ci == 0),
                                    stop=(ci == NCT - 1),
                                )
                            ecopy(dstT[j][:, t0:t0 + TCH], acc[:])
                    acck = pacc.tile([RHD, TCH], F32, tag="acck", name="acck")
                    for ci in range(NCT):
                        nc.tensor.matmul(
                            acck[:],
                            wkrT[ci][:],
                            xT[ci][:],
                            start=(ci == 0),
                            stop=(ci == NCT - 1),
                        )
                    ecopy(krraw[:, t0:t0 + TCH], acck[:], pin="act")

                def wd_prep():
                    for w_ext, wTs in ((wdq_ext, wdqT), (wdkv_ext, wdkvT)):
                        for rp in range(NL // P // 2):
                            rA = slice(2 * rp * P, (2 * rp + 1) * P)
                            rB = slice((2 * rp + 1) * P, (2 * rp + 2) * P)
                            for hf in range(2):
                                cf = slice(hf * (C // 2), (hf + 1) * (C // 2))
                                sA = pa.tile([P, C // 2], F32R, tag="wsA",
                                             bufs=2, name="wsA")
                                sB = pa.tile([P, C // 2], F32R, tag="wsB",
                                             bufs=2, name="wsB")
                                nc.sync.dma_start(sA[:], w_ext.ap()[rA, cf])
                                nc.sync.dma_start(sB[:], w_ext.ap()[rB, cf])
                                for ci in range(NCT // 2):
                                    transpose_pair_into(
                                        wTs[hf * 8 + ci][:, 2 * rp * P:(2 * rp + 2) * P],
                                        sA[:, ci * P:(ci + 1) * P],
                                        sB[:, ci * P:(ci + 1) * P],
                                    )
                    kstrip = pa.tile([RHD, C], F32R, tag="kstrip", name="kstrip")
                    nc.sync.dma_start(kstrip[:], wkr_ext.ap())
                    for ci in range(NCT):
                        tp = ptp_cell[0].tile([P, 2 * P], F32R, tag="tp",
                                              name="tp")
                        nc.tensor.transpose(
                            tp[:, :RHD], kstrip[:, ci * P:(ci + 1) * P],
                            ident[:RHD, :RHD])
                        ev, od = _deint(tp[:, :RHD])
                        nc.scalar.copy(wkrT[ci][:, 0:32], ev)
                        nc.scalar.copy(wkrT[ci][:, 32:64], od)

                def table_prep():
                    for s in range(T // P):
                        cst = pa.tile([P, RHD // 2], F32R, tag="cst", bufs=2,
                                      name="cst")
                        sst = pa.tile([P, RHD // 2], F32R, tag="sst", bufs=2,
                                      name="sst")
                        nc.sync.dma_start(cst[:],
                                          cos_ext.ap()[s * P:(s + 1) * P, :])
                        nc.sync.dma_start(sst[:],
                                          sin_ext.ap()[s * P:(s + 1) * P, :])
                        tp = ptp_cell[0].tile([P, 2 * P], F32R, tag="tp",
                                              name="tp")
                        nc.tensor.transpose(tp[:32, 0:P], cst[:], ident[:])
                        nc.tensor.transpose(tp[:32, P:2 * P], sst[:], ident[:])
                        nc.vector.tensor_copy(ca4[0:32, s * P:(s + 1) * P],
                                              tp[:32, 0:P])
                        nc.vector.tensor_copy(sa4[0:32, s * P:(s + 1) * P],
                                              tp[:32, P:2 * P])
                    for d in range(1, 4):
                        nc.vector.tensor_copy(ca4[32 * d:32 * (d + 1), :],
                                              ca4[0:32, :])
                        nc.vector.tensor_copy(sa4[32 * d:32 * (d + 1), :],
                                              sa4[0:32, :])

                def wu_prep():
                    for w_ext, wT in ((wuq_ext, wuqT), (wuk_ext, wukT),
                                      (wuv_ext, wuvT)):
                        for rp in range(HLOC * HS // P // 2):
                            sA = pa.tile([P, NL], F32R, tag="usA", bufs=2,
                                         name="usA")
                            sB = pa.tile([P, NL], F32R, tag="usB", bufs=2,
                                         name="usB")
                            nc.sync.dma_start(
                                sA[:], w_ext.ap()[2 * rp * P:(2 * rp + 1) * P, :])
                            nc.sync.dma_start(
                                sB[:], w_ext.ap()[(2 * rp + 1) * P:(2 * rp + 2) * P, :])
                            for cs in range(NNL):
                                transpose_pair_into(
                                    wT[cs][:, 2 * rp * P:(2 * rp + 2) * P],
                                    sA[:, cs * P:(cs + 1) * P],
                                    sB[:, cs * P:(cs + 1) * P],
                                )

                def wo_wqr_prep():
                    for g in range(2):
                        strip = pa.tile([P, NL], F32R, tag="qrs", bufs=2,
                                        name="qrs")
                        nc.sync.dma_start(
                            strip[:], wqr_ext.ap()[g * P:(g + 1) * P, :])
                        for cs in range(NNL):
                            tp = ptp_cell[0].tile([P, 2 * P], F32R, tag="tp",
                                                  name="tp")
                            nc.tensor.transpose(
                                tp[:, 0:P], strip[:, cs * P:(cs + 1) * P],
                                ident[:])
                            evA, odA = _deint(tp[:, 0:RHD])
                            evB, odB = _deint(tp[:, RHD:2 * RHD])
                            nc.scalar.copy(wqrT[g][cs][:, 0:32], evA)
                            nc.scalar.copy(wqrT[g][cs][:, 32:64], evB)
                            nc.scalar.copy(wqrT[g][cs][:, 64:96], odA)
                            nc.scalar.copy(wqrT[g][cs][:, 96:128], odB)
                    for sp in range(C // P // 2):
                        oA = pa.tile([P, HLOC * HS], F32R, tag="osA", bufs=2,
                                     name="osA")
                        oB = pa.tile([P, HLOC * HS], F32R, tag="osB", bufs=2,
                                     name="osB")
                        nc.sync.dma_start(
                            oA[:], wo_ext.ap()[2 * sp * P:(2 * sp + 1) * P, :])
                        nc.sync.dma_start(
                            oB[:], wo_ext.ap()[(2 * sp + 1) * P:(2 * sp + 2) * P, :])
                        for fs in range(HLOC):
                            transpose_pair_into(
                                woT[fs][:, 2 * sp * P:(2 * sp + 2) * P],
                                oA[:, fs * P:(fs + 1) * P],
                                oB[:, fs * P:(fs + 1) * P],
                            )

                # interleave: x transposes first so PE starts immediately,
                # weight preps slot between chunks while x DMA streams.
                x_chunk_transpose(0)
                wd_prep()
                x_chunk_matmuls(0)
                x_chunk_transpose(1)
                table_prep()
                x_chunk_matmuls(1)
                x_chunk_transpose(2)
                wu_prep()
                x_chunk_matmuls(2)
                x_chunk_transpose(3)
                wo_wqr_prep()
                x_chunk_matmuls(3)

                pa_ctx[2].__exit__(None, None, None)
                pa_ctx[1].__exit__(None, None, None)
                pa_ctx[0].__exit__(None, None, None)

                # ============ phase B: rope, up-projections, fp8 packs ======
                pb2_ctx = tc.tile_pool(name="pb2", bufs=1)
                pb2 = pb2_ctx.__enter__()
                q8 = [pb2.tile([P, 2, T], F8, tag=f"q8{h}", name=f"q8{h}")
                      for h in range(HLOC)]
                k8 = [pb2.tile([P, 2, T], F8, tag=f"k8{h}", name=f"k8{h}")
                      for h in range(HLOC)]
                vv = [pb2.tile([P, T // P, P], BF16, tag=f"vv{h}", name=f"vv{h}")
                      for h in range(HLOC)]
                for h in range(HLOC):
                    nc.vector.memset(q8[h][64:128, 1, :], 0.0)
                    nc.gpsimd.memset(k8[h][64:128, 1, :], 0.0)

                with (
                    tc.tile_pool(name="pmm", bufs=3, space="PSUM") as pmm,
                    tc.tile_pool(name="pou", bufs=2, space="PSUM") as pou,
                    tc.tile_pool(name="pde", bufs=2, space="PSUM") as pde,
                    tc.tile_pool(name="pat", bufs=1) as pat,
                ):
                    # k_r rope -> krf8 (planar re/im), shared across heads
                    krf8 = pb2.tile([RHD, T], F8, tag="krf8", name="krf8")
                    rtmp = pb2.tile([P, T], BF16, tag="rtmp", name="rtmp")
                    rro = pb2.tile([P, T], BF16, tag="rro", name="rro")
                    nc.vector.tensor_mul(rtmp[0:32, :], krraw[32:64, :], sa4[32:64, :])
                    nc.vector.tensor_mul(rtmp[32:64, :], krraw[32:64, :], ca4[32:64, :])
                    nc.vector.tensor_mul(rro[0:32, :], krraw[0:32, :], ca4[0:32, :])
                    nc.vector.tensor_mul(rro[32:64, :], krraw[0:32, :], sa4[0:32, :])
                    nc.vector.tensor_sub(krf8[0:32, :], rro[0:32, :], rtmp[0:32, :])
                    nc.vector.tensor_add(krf8[32:64, :], rro[32:64, :], rtmp[32:64, :])
                    for h in range(HLOC):
                        nc.sync.dma_start(k8[h][0:RHD, 1, :], krf8[:])

                    # up-projections, head-pair at a time
                    for g in range(2):
                        hA, hB = 2 * g, 2 * g + 1
                        qraw = pb2.tile([P, T], BF16, tag="qraw", name="qraw")
                        for ch in range(NCH):
                            sl = slice(ch * TCH, (ch + 1) * TCH)
                            for hh in (hA, hB):
                                for wT, src, dst in ((wuqT, cqT, q8),
                                                     (wukT, ckvT, k8)):
                                    acc = pmm.tile([P, TCH], F32, tag="mm",
                                                   name="mm")
                                    for nl in range(NNL):
                                        nc.tensor.matmul(
                                            acc[:],
                                            wT[nl][:, hh * P:(hh + 1) * P],
                                            src[nl][:, sl],
                                            start=(nl == 0),
                                            stop=(nl == NNL - 1),
                                        )
                                    ecopy(dst[hh][:, 0, sl], acc[:], pin="act")
                            qacc = pmm.tile([P, TCH], F32, tag="mm", name="mm")
                            for nl in range(NNL):
                                nc.tensor.matmul(
                                    qacc[:],
                                    wqrT[g][nl][:],
                                    cqT[nl][:, sl],
                                    start=(nl == 0),
                                    stop=(nl == NNL - 1),
                                )
                            ecopy(qraw[:, sl], qacc[:], pin="act")
                        # natural-layout V for both heads
                        for hh in (hA, hB):
                            for ts4 in range(T // TCH):
                                vps = pmm.tile([P, TCH], F32, tag="mm", name="mm")
                                for j in range(4):
                                    kt = ts4 * 4 + j
                                    for nl in range(NNL):
                                        nc.tensor.matmul(
                                            vps[:, j * P:(j + 1) * P],
                                            ckvT[nl][:, kt * P:(kt + 1) * P],
                                            wuvT[nl][:, hh * P:(hh + 1) * P],
                                            start=(nl == 0),
                                            stop=(nl == NNL - 1),
                                            skip_group_check=True,
                                        )
                                ecopy(
                                    vv[hh][:, ts4 * 4:(ts4 + 1) * 4, :],
                                    vps[:].rearrange("p (a b) -> p a b", a=4),
                                    pin="act",
                                )
                        # rope for the pair: rows [Are, Bre, Aim, Bim]
                        roq = pb2.tile([P, T], F8, tag="roq", bufs=2, name="roq")
                        nc.vector.tensor_mul(rtmp[0:64, :], qraw[64:128, :],
                                             sa4[64:128, :])
                        nc.vector.tensor_mul(rtmp[64:128, :], qraw[64:128, :],
                                             ca4[64:128, :])
                        nc.vector.tensor_mul(rro[0:64, :], qraw[0:64, :],
                                             ca4[0:64, :])
                        nc.vector.tensor_mul(rro[64:128, :], qraw[0:64, :],
                                             sa4[0:64, :])
                        nc.vector.tensor_sub(roq[0:64, :], rro[0:64, :],
                                             rtmp[0:64, :])
                        nc.vector.tensor_add(roq[64:128, :], rro[64:128, :],
                                             rtmp[64:128, :])
                        nc.sync.dma_start(q8[hA][0:32, 1, :], roq[0:32, :])
                        nc.sync.dma_start(q8[hA][32:64, 1, :], roq[64:96, :])
                        nc.sync.dma_start(q8[hB][0:32, 1, :], roq[32:64, :])
                        nc.sync.dma_start(q8[hB][32:64, 1, :], roq[96:128, :])

                    # ============ attention + deferred norm/output proj =====
                    deferred_b = deque()   # normalization tails
                    outproj_q = deque()    # (ohs, tq, cs_start) groups

                    def emit_outproj_group():
                        g_ohs, g_tq, cs0 = outproj_q.popleft()
                        g_qsl = slice(g_tq * TCH, (g_tq + 1) * TCH)
                        for cs in range(cs0, cs0 + 4):
                            acc = pmm.tile([P, TCH], F32, tag="mm", name="mm")
                            for h2 in range(HLOC):
                                nc.tensor.matmul(
                                    acc[:],
                                    woT[h2][:, cs * P:(cs + 1) * P],
                                    g_ohs[h2][:],
                                    start=(h2 == 0),
                                    stop=(h2 == HLOC - 1),
                                )
                            ot = pat.tile([P, TCH], F32, tag="ot", bufs=3,
                                          name="ot")
                            ecopy(ot[:], acc[:], pin="dve")
                            nc.sync.dma_start(
                                out_ext.ap()[cs * P:(cs + 1) * P, g_qsl], ot[:])

                    ohs_by_tq = {}
                    next_den = pde.tile([P, 4], F32, tag="de", name="de")
                    nc.vector.memset(next_den[:], 0.0)
                    for tq in range(NCH):
                        qsl = slice(tq * TCH, (tq + 1) * TCH)
                        ohs = []
                        ohs_by_tq[tq] = ohs
                        for h in range(HLOC):
                            outU = pou.tile([P, TCH], F32, tag="ou", name="ou")
                            den = next_den
                            nsub = (tq + 1) * 4
                            pend = deque()

                            def flush_one(outU=outU, den=den, nsub=nsub, h=h,
                                          tq=tq, pend=pend):
                                Pt, kt = pend.popleft()
                                kc, ks = kt // 4, kt % 4
                                off = ks * P if kc == tq else 0
                                for qq in range(off // P, 4):
                                    nc.tensor.matmul(
                                        den[:, qq:qq + 1],
                                        Pt[:, qq * P:(qq + 1) * P],
                                        onescol[:],
                                        start=False,
                                        stop=(kt == nsub - 1),
                                        skip_group_check=True,
                                    )
                                nc.tensor.matmul(
                                    outU[:, off:TCH],
                                    vv[h][:, kt, :],
                                    Pt[:, off:TCH],
                                    start=(kt == 0),
                                    stop=(kt == nsub - 1),
                                    skip_group_check=True,
                                )

                            for kt in range(nsub):
                                kc, ks = kt // 4, kt % 4
                                diag = kc == tq
                                off = ks * P if diag else 0
                                npr = TCH - off
                                ST = pmm.tile([P, TCH], F32, tag="mm", name="mm")
                                nc.tensor.matmul(
                                    ST[:, 0:npr],
                                    k8[h][:, :, kt * P:(kt + 1) * P],
                                    q8[h][:, :, qsl.start + off:qsl.stop],
                                    perf_mode=DR,
                                    start=True,
                                    stop=not diag,
                                )
                                if diag:
                                    # causal band mask folded in on PE:
                                    # ST[:, 0:128] += I^T @ mask
                                    nc.tensor.matmul(
                                        ST[:, 0:P],
                                        identb[:],
                                        maskbuf[:, 384:384 + P],
                                        start=False,
                                        stop=True,
                                        skip_group_check=True,
                                    )
                                Pt = pat.tile([P, TCH], BF16, tag="pt", bufs=6,
                                              name="pt")
                                nc.scalar.activation(Pt[:, off:TCH], ST[:, 0:npr],
                                                     Exp, scale=SCALE)
                                pend.append((Pt, kt))
                                if len(pend) > 2:
                                    flush_one()
                                if kt == 2:
                                    # emit previous head's deferred tail and
                                    # one output-projection group here, where
                                    # PE has score work queued to hide them
                                    if deferred_b:
                                        deferred_b.popleft()()
                                    if outproj_q:
                                        emit_outproj_group()
                            while pend:
                                flush_one()

                            # normalization head: transpose+recip now (DVE),
                            # broadcast+apply deferred under the next head
                            den_sb = pat.tile([P, 4], F32R, tag="dsb", bufs=2,
                                              name="dsb")
                            nc.vector.tensor_copy(den_sb[:], den[:])
                            next_den = pde.tile([P, 4], F32, tag="de", name="de")
                            nc.vector.memset(next_den[:], 0.0)
                            tpd = pde.tile([4, P], F32R, tag="tpd", bufs=1,
                                           name="tpd")
                            nc.tensor.transpose(tpd[0:4, 0:P], den_sb[:], ident[:])
                            rec = pat.tile([4, P], F32, tag="rec", bufs=2,
                                           name="rec")
                            nc.vector.reciprocal(rec[:], tpd[:])
                            recb = pat.tile([4, P], BF16, tag="recb", bufs=2,
                                            name="recb")
                            nc.vector.tensor_copy(recb[:], rec[:])
                            oh = pat.tile([P, TCH], BF16, tag=f"oh{h}", bufs=2,
                                          name=f"oh{h}")
                            ohs.append(oh)

                            def norm_tail(recb=recb, outU=outU, oh=oh, h=h,
                                          tq=tq):
                                bc = pmm.tile([P, TCH], F32, tag="mm", name="mm")
                                for qq in range(4):
                                    nc.tensor.matmul(
                                        bc[:, qq * P:(qq + 1) * P],
                                        sel4[:, qq * P:(qq + 1) * P],
                                        recb[:],
                                        start=True, stop=True,
                                        skip_group_check=True,
                                    )
                                bcs = pat.tile([P, TCH], BF16, tag="bcs",
                                               bufs=2, name="bcs")
                                nc.vector.tensor_copy(bcs[:], bc[:])
                                nc.vector.tensor_mul(oh[:], outU[:], bcs[:])
                                if h == HLOC - 1:
                                    for cs0 in range(0, NCT, 4):
                                        outproj_q.append(
                                            (ohs_by_tq[tq], tq, cs0))

                            deferred_b.append(norm_tail)

                    # drain deferred work
                    while deferred_b:
                        deferred_b.popleft()()
                    while outproj_q:
                        emit_outproj_group()

                pb2_ctx.__exit__(None, None, None)

    nc.compile()
    return nc


def _get_nc():
    if "nc" not in _NC_CACHE:
        _NC_CACHE["nc"] = build()
    return _NC_CACHE["nc"]


def kernel(x, freqs_cos, freqs_sin, W_dq, W_uq, W_dkv, W_uk, W_uv, W_qr, W_kr,
           W_o, trace=False, **trace_kwargs):
    nc = _get_nc()
    f32 = lambda a: np.ascontiguousarray(np.asarray(a, dtype=np.float32))
    x = f32(x); W_dq = f32(W_dq); W_uq = f32(W_uq); W_dkv = f32(W_dkv)
    W_uk = f32(W_uk); W_uv = f32(W_uv); W_qr = f32(W_qr); W_kr = f32(W_kr)
    W_o = f32(W_o)
    cos = f32(freqs_cos); sin = f32(freqs_sin)

    in_maps = []
    for c in range(8):
        b, r = divmod(c, 4)
        in_maps.append({
            "x": x[b],
            "wdq": W_dq, "wdkv": W_dkv, "wkr": W_kr,
            "wuq": W_uq[r * HLOC * HS:(r + 1) * HLOC * HS],
            "wuk": W_uk[r * HLOC * HS:(r + 1) * HLOC * HS],
            "wuv": W_uv[r * HLOC * HS:(r + 1) * HLOC * HS],
            "wqr": W_qr[r * HLOC * RHD:(r + 1) * HLOC * RHD],
            "wo": W_o[:, r * HLOC * HS:(r + 1) * HLOC * HS],
            "cos": cos, "sin": sin,
        })
    res = run_bass_kernel_spmd(nc, in_maps, core_ids=list(range(8)),
                               trace=trace, **trace_kwargs)
    out = np.zeros((B, T, C), dtype=np.float32)
    for c in range(8):
        b = c // 4
        out[b] += res.results[c]["out"].T
    kernel.last_result = res
    return out


# revision 43
# speedup vs baseline: 1.6089x; 1.0445x over previous
"""MLA-style attention kernel for 8 TRN2 NeuronCores, v3.

Sharding: core c -> batch b = c//4, heads r*4..r*4+3 where r = c%4.
The latent down-projections are REPLICATED within each 4-core batch
group (no collective, no cross-core dependency): each core computes the
full-T latents c_q/c_kv/k_r from the full x[b], then its 4 heads'
attention and a partial output projection summed on the host.

All activations stay SBUF-resident in a transposed [feature, T] layout.
Down/up-projections run in bf16 (PSUM fp32 accumulate).  Scores use
fp8e4 with DoubleRow perf mode: q/k packed as [128, 2, T] fp8 where
slot 0 holds the 128 content dims and slot 1 rows 0:64 hold the roped
rope dims (planar re/im), rows 64:128 zero.  One DoubleRow matmul per
512x128 score subtile (4x fewer PE cycles than two f32r matmuls).
Softmax denominators use N=1 ones-column matmuls accumulating into a
[128q, 4] PSUM tile (start=False onto memset zeros -- a start=True
would wipe sibling columns through the 2KB zero-region), then
transpose + reciprocal + selector broadcast matmul for normalization.
Diagonal causal blocks shrink the score matmul to the valid q-range,
memset the dead Pt columns, and add a fixed 128-wide triangular mask.
exp() pipelines two subtiles deep; the normalization tail of head h and
the output projection of chunk tq-1 are emitted under the NEXT head's
score loop so PE never drains while Act works.  V is produced directly
in natural [t, hs] layout (lhsT = ckv^T): no transposes after phase A.
Both hardware DMA queues are used: SP for W_dq/W_dkv/x/output, Act for
cos/sin, W_u/W_qr/W_o, mask, and the SBUF-to-SBUF fp8 slot copies.
"""
import math
from collections import deque
import numpy as np

import concourse.bass as bass
import concourse.bacc as bacc
import concourse.mybir as mybir
import concourse.tile as tile
from concourse.bass_utils import run_bass_kernel_spmd

F32 = mybir.dt.float32
F32R = mybir.dt.float32r
BF16 = mybir.dt.bfloat16
F8 = mybir.dt.float8e4
Exp = mybir.ActivationFunctionType.Exp
DR = mybir.MatmulPerfMode.DoubleRow

B, T, C = 2, 2048, 2048
H = 16
HS = 128
NL = 512
RHD = 64
HLOC = 4              # heads per core
P = 128
NNL = NL // P         # 4
NCT = C // P          # 16
TCH = 512
NCH = T // TCH        # 4
SCALE = 1.0 / math.sqrt(HS + RHD)
NEG = -1.0e30

_NC_CACHE = {}


def _deint(ap2d):
    # [p, 2d] -> (evens [p, d], odds [p, d]) along the free dim
    rr = ap2d.rearrange("p (d two) -> p two d", two=2)
    return rr[:, 0, :], rr[:, 1, :]


def build():
    nc = bacc.Bacc("TRN2", target_bir_lowering=False, debug=False, num_devices=8)

    x_ext = nc.dram_tensor("x", [T, C], F32R, kind="ExternalInput")
    wdq_ext = nc.dram_tensor("wdq", [NL, C], F32R, kind="ExternalInput")
    wdkv_ext = nc.dram_tensor("wdkv", [NL, C], F32R, kind="ExternalInput")
    wkr_ext = nc.dram_tensor("wkr", [RHD, C], F32R, kind="ExternalInput")
    wuq_ext = nc.dram_tensor("wuq", [HLOC * HS, NL], F32R, kind="ExternalInput")
    wuk_ext = nc.dram_tensor("wuk", [HLOC * HS, NL], F32R, kind="ExternalInput")
    wuv_ext = nc.dram_tensor("wuv", [HLOC * HS, NL], F32R, kind="ExternalInput")
    wqr_ext = nc.dram_tensor("wqr", [HLOC * RHD, NL], F32R, kind="ExternalInput")
    wo_ext = nc.dram_tensor("wo", [C, HLOC * HS], F32R, kind="ExternalInput")
    cos_ext = nc.dram_tensor("cos", [T, RHD // 2], F32R, kind="ExternalInput")
    sin_ext = nc.dram_tensor("sin", [T, RHD // 2], F32R, kind="ExternalInput")
    out_ext = nc.dram_tensor("out", [C, T], F32, kind="ExternalOutput")

    ident_dram = nc.inline_tensor(np.eye(P, dtype=np.float32), name="identc")
    # triangular mask for the 128-wide diagonal band of shrunk S^T tiles
    m = np.full((P, 896), NEG, dtype=np.float32)
    for jj in range(P):
        m[jj, 384 + jj:] = 0.0
    masks_dram = nc.inline_tensor(m, name="maskc")
    # row selector for the 1/den broadcast: sel4[k, qq*128+j] = (k == qq)
    sel = np.zeros((4, 512), dtype=np.float32)
    for qq in range(4):
        sel[qq, qq * P:(qq + 1) * P] = 1.0
    sel4_dram = nc.inline_tensor(sel, name="sel4c")

    with tile.TileContext(nc) as tc:
        with tc.tile_pool(name="pers", bufs=1) as pers:
            ptp_cell = [None]
            ident = pers.tile([P, P], F32R, tag="ident", name="ident")
            nc.sync.dma_start(ident[:], ident_dram.ap().bitcast(F32R))
            maskbuf = pers.tile([P, 896], BF16, tag="maskbuf", name="maskbuf")
            nc.gpsimd.dma_start(out=maskbuf[:], in_=masks_dram.ap())
            onescol = pers.tile([P, 1], BF16, tag="onescol", name="onescol")
            nc.vector.memset(onescol[:], 1.0)
            identb = pers.tile([P, P], BF16, tag="identb", name="identb")
            nc.vector.tensor_copy(identb[:], ident[:])
            sel4 = pers.tile([4, TCH], BF16, tag="sel4", name="sel4")
            nc.gpsimd.dma_start(out=sel4[:], in_=sel4_dram.ap())

            # rope tables, cos/sin duplicated on all four 32-row groups
            ca4 = pers.tile([P, T], BF16, tag="ca4", name="ca4")
            sa4 = pers.tile([P, T], BF16, tag="sa4", name="sa4")

            # full-T latents (bf16, [feat, T])
            cqT = [pers.tile([P, T], BF16, tag=f"cqT{i}", name=f"cqT{i}")
                   for i in range(NNL)]
            ckvT = [pers.tile([P, T], BF16, tag=f"ckvT{i}", name=f"ckvT{i}")
                    for i in range(NNL)]
            krraw = pers.tile([RHD, T], BF16, tag="krraw", name="krraw")

            _ecnt = [0]

            def ecopy(dst, src, pin=None):
                """PSUM->SBUF evacuation copy, alternating Act/DVE."""
                _ecnt[0] += 1
                eng = pin if pin else ("act" if _ecnt[0] % 2 else "dve")
                if eng == "act":
                    nc.scalar.copy(dst, src)
                else:
                    nc.vector.tensor_copy(dst, src)

            def transpose_pair_into(dst_ap, srcA, srcB, pin=None):
                tp2 = ptp_cell[0].tile([P, 2 * P], F32R, tag="tp", name="tp")
                nc.tensor.transpose(tp2[:, 0:P], srcA, ident[:])
                nc.tensor.transpose(tp2[:, P:2 * P], srcB, ident[:])
                ecopy(dst_ap, tp2[:], pin=pin)

            # ============ phase A: x chunks + all weight prep, interleaved ==
            with tc.tile_pool(name="pb", bufs=1) as pb:
                # -- persistent-ish weight destinations (pb outlives phase A)
                wuqT = [pb.tile([P, HLOC * HS], BF16, tag=f"wuqT{i}",
                                name=f"wuqT{i}") for i in range(NNL)]
                wukT = [pb.tile([P, HLOC * HS], BF16, tag=f"wukT{i}",
                                name=f"wukT{i}") for i in range(NNL)]
                wuvT = [pb.tile([P, HLOC * HS], BF16, tag=f"wuvT{i}",
                                name=f"wuvT{i}") for i in range(NNL)]
                wqrT = [[pb.tile([P, P], BF16, tag=f"wqrT{g}{i}",
                                 name=f"wqrT{g}{i}") for i in range(NNL)]
                        for g in range(2)]
                woT = [pb.tile([P, C], BF16, tag=f"woT{i}", name=f"woT{i}")
                       for i in range(HLOC)]

                pa_ctx = (
                    tc.tile_pool(name="pa", bufs=1),
                    tc.tile_pool(name="pacc", bufs=1, space="PSUM"),
                    tc.tile_pool(name="ptpA", bufs=3, space="PSUM"),
                )
                pa = pa_ctx[0].__enter__()
                pacc = pa_ctx[1].__enter__()
                ptp_cell[0] = pa_ctx[2].__enter__()

                wdqT = [pa.tile([P, NL], BF16, tag=f"wdqT{i}", name=f"wdqT{i}")
                        for i in range(NCT)]
                wdkvT = [pa.tile([P, NL], BF16, tag=f"wdkvT{i}",
                                 name=f"wdkvT{i}") for i in range(NCT)]
                wkrT = [pa.tile([P, RHD], BF16, tag=f"wkrT{i}", name=f"wkrT{i}")
                        for i in range(NCT)]
                xT = [pa.tile([P, TCH], BF16, tag=f"xT{i}", name=f"xT{i}")
                      for i in range(NCT)]

                def x_chunk_transpose(tch):
                    t0 = tch * TCH
                    for sp in range(2):
                        rA = slice(t0 + 2 * sp * P, t0 + (2 * sp + 1) * P)
                        rB = slice(t0 + (2 * sp + 1) * P, t0 + (2 * sp + 2) * P)
                        for hf in range(2):
                            cf = slice(hf * (C // 2), (hf + 1) * (C // 2))
                            xA = pa.tile([P, C // 2], F32R, tag="xA", bufs=3,
                                         name="xA")
                            xB = pa.tile([P, C // 2], F32R, tag="xB", bufs=3,
                                         name="xB")
                            nc.sync.dma_start(xA[:], x_ext.ap()[rA, cf])
                            nc.sync.dma_start(xB[:], x_ext.ap()[rB, cf])
                            for ci in range(NCT // 2):
                                transpose_pair_into(
                                    xT[hf * 8 + ci][:, 2 * sp * P:(2 * sp + 2) * P],
                                    xA[:, ci * P:(ci + 1) * P],
                                    xB[:, ci * P:(ci + 1) * P],
                                )

                def x_chunk_matmuls(tch):
                    t0 = tch * TCH
                    for wTs, dstT in ((wdqT, cqT), (wdkvT, ckvT)):
                        for j in range(NNL):
                            acc = pacc.tile([P, TCH], F32, tag=f"acc{j}",
                                            name=f"acc{j}")
                            for ci in range(NCT):
                                nc.tensor.matmul(
                                    acc[:],
                                    wTs[ci][:, j * P:(j + 1) * P],
                                    xT[ci][:],
                                    start=(ci == 0),
                                    stop=(ci == NCT - 1),
                                )
                            ecopy(dstT[j][:, t0:t0 + TCH], acc[:])
                    acck = pacc.tile([RHD, TCH], F32, tag="acck", name="acck")
                    for ci in range(NCT):
                        nc.tensor.matmul(
                            acck[:],
                            wkrT[ci][:],
                            xT[ci][:],
                            start=(ci == 0),
                            stop=(ci == NCT - 1),
                        )
                    ecopy(krraw[:, t0:t0 + TCH], acck[:], pin="act")

                def wd_prep():
                    for w_ext, wTs in ((wdq_ext, wdqT), (wdkv_ext, wdkvT)):
                        for rp in range(NL // P // 2):
                            rA = slice(2 * rp * P, (2 * rp + 1) * P)
                            rB = slice((2 * rp + 1) * P, (2 * rp + 2) * P)
                            for hf in range(2):
                                cf = slice(hf * (C // 2), (hf + 1) * (C // 2))
                                sA = pa.tile([P, C // 2], F32R, tag="wsA",
                                             bufs=2, name="wsA")
                                sB = pa.tile([P, C // 2], F32R, tag="wsB",
                                             bufs=2, name="wsB")
                                nc.sync.dma_start(sA[:], w_ext.ap()[rA, cf])
                                nc.sync.dma_start(sB[:], w_ext.ap()[rB, cf])
                                for ci in range(NCT // 2):
                                    transpose_pair_into(
                                        wTs[hf * 8 + ci][:, 2 * rp * P:(2 * rp + 2) * P],
                                        sA[:, ci * P:(ci + 1) * P],
                                        sB[:, ci * P:(ci + 1) * P],
                                    )
                    kstrip = pa.tile([RHD, C], F32R, tag="kstrip", name="kstrip")
                    nc.sync.dma_start(kstrip[:], wkr_ext.ap())
                    for ci in range(NCT):
                        tp = ptp_cell[0].tile([P, 2 * P], F32R, tag="tp",
                                              name="tp")
                        nc.tensor.transpose(
                            tp[:, :RHD], kstrip[:, ci * P:(ci + 1) * P],
                            ident[:RHD, :RHD])
                        ev, od = _deint(tp[:, :RHD])
                        nc.scalar.copy(wkrT[ci][:, 0:32], ev)
                        nc.scalar.copy(wkrT[ci][:, 32:64], od)

                def table_prep():
                    for s in range(T // P):
                        cst = pa.tile([P, RHD // 2], F32R, tag="cst", bufs=2,
                                      name="cst")
                        sst = pa.tile([P, RHD // 2], F32R, tag="sst", bufs=2,
                                      name="sst")
                        nc.sync.dma_start(cst[:],
                                          cos_ext.ap()[s * P:(s + 1) * P, :])
                        nc.sync.dma_start(sst[:],
                                          sin_ext.ap()[s * P:(s + 1) * P, :])
                        tp = ptp_cell[0].tile([P, 2 * P], F32R, tag="tp",
                                              name="tp")
                        nc.tensor.transpose(tp[:32, 0:P], cst[:], ident[:])
                        nc.tensor.transpose(tp[:32, P:2 * P], sst[:], ident[:])
                        nc.vector.tensor_copy(ca4[0:32, s * P:(s + 1) * P],
                                              tp[:32, 0:P])
                        nc.vector.tensor_copy(sa4[0:32, s * P:(s + 1) * P],
                                              tp[:32, P:2 * P])
                    for d in range(1, 4):
                        nc.vector.tensor_copy(ca4[32 * d:32 * (d + 1), :],
                                              ca4[0:32, :])
                        nc.vector.tensor_copy(sa4[32 * d:32 * (d + 1), :],
                                              sa4[0:32, :])

                def wu_prep():
                    for w_ext, wT in ((wuq_ext, wuqT), (wuk_ext, wukT),
                                      (wuv_ext, wuvT)):
                        for rp in range(HLOC * HS // P // 2):
                            sA = pa.tile([P, NL], F32R, tag="usA", bufs=2,
                                         name="usA")
                            sB = pa.tile([P, NL], F32R, tag="usB", bufs=2,
                                         name="usB")
                            nc.sync.dma_start(
                                sA[:], w_ext.ap()[2 * rp * P:(2 * rp + 1) * P, :])
                            nc.sync.dma_start(
                                sB[:], w_ext.ap()[(2 * rp + 1) * P:(2 * rp + 2) * P, :])
                            for cs in range(NNL):
                                transpose_pair_into(
                                    wT[cs][:, 2 * rp * P:(2 * rp + 2) * P],
                                    sA[:, cs * P:(cs + 1) * P],
                                    sB[:, cs * P:(cs + 1) * P],
                                )

                def wo_wqr_prep():
                    for g in range(2):
                        strip = pa.tile([P, NL], F32R, tag="qrs", bufs=2,
                                        name="qrs")
                        nc.sync.dma_start(
                            strip[:], wqr_ext.ap()[g * P:(g + 1) * P, :])
                        for cs in range(NNL):
                            tp = ptp_cell[0].tile([P, 2 * P], F32R, tag="tp",
                                                  name="tp")
                            nc.tensor.transpose(
                                tp[:, 0:P], strip[:, cs * P:(cs + 1) * P],
                                ident[:])
                            evA, odA = _deint(tp[:, 0:RHD])
                            evB, odB = _deint(tp[:, RHD:2 * RHD])
                            nc.scalar.copy(wqrT[g][cs][:, 0:32], evA)
                            nc.scalar.copy(wqrT[g][cs][:, 32:64], evB)
                            nc.scalar.copy(wqrT[g][cs][:, 64:96], odA)
                            nc.scalar.copy(wqrT[g][cs][:, 96:128], odB)
                    for sp in range(C // P // 2):
                        oA = pa.tile([P, HLOC * HS], F32R, tag="osA", bufs=2,
                                     name="osA")
                        oB = pa.tile([P, HLOC * HS], F32R, tag="osB", bufs=2,
                                     name="osB")
                        nc.sync.dma_start(
                            oA[:], wo_ext.ap()[2 * sp * P:(2 * sp + 1) * P, :])
                        nc.sync.dma_start(
                            oB[:], wo_ext.ap()[(2 * sp + 1) * P:(2 * sp + 2) * P, :])
                        for fs in range(HLOC):
                            transpose_pair_into(
                                woT[fs][:, 2 * sp * P:(2 * sp + 2) * P],
                                oA[:, fs * P:(fs + 1) * P],
                                oB[:, fs * P:(fs + 1) * P],
                            )

                # interleave: x transposes first so PE starts immediately,
                # weight preps slot between chunks while x DMA streams.
                x_chunk_transpose(0)
                wd_prep()
                x_chunk_matmuls(0)
                x_chunk_transpose(1)
                table_prep()
                x_chunk_matmuls(1)
                x_chunk_transpose(2)
                wu_prep()
                x_chunk_matmuls(2)
                x_chunk_transpose(3)
                wo_wqr_prep()
                x_chunk_matmuls(3)

                pa_ctx[2].__exit__(None, None, None)
                pa_ctx[1].__exit__(None, None, None)
                pa_ctx[0].__exit__(None, None, None)

                # ============ phase B: rope, up-projections, fp8 packs ======
                pb2_ctx = tc.tile_pool(name="pb2", bufs=1)
                pb2 = pb2_ctx.__enter__()
                q8 = [pb2.tile([P, 2, T], F8, tag=f"q8{h}", name=f"q8{h}")
                      for h in range(HLOC)]
                k8 = [pb2.tile([P, 2, T], F8, tag=f"k8{h}", name=f"k8{h}")
                      for h in range(HLOC)]
                vv = [pb2.tile([P, T // P, P], BF16, tag=f"vv{h}", name=f"vv{h}")
                      for h in range(HLOC)]
                for h in range(HLOC):
                    nc.vector.memset(q8[h][64:128, 1, :], 0.0)
                    nc.gpsimd.memset(k8[h][64:128, 1, :], 0.0)

                with (
                    tc.tile_pool(name="pmm", bufs=3, space="PSUM") as pmm,
                    tc.tile_pool(name="pou", bufs=2, space="PSUM") as pou,
                    tc.tile_pool(name="pde", bufs=2, space="PSUM") as pde,
                    tc.tile_pool(name="pat", bufs=1) as pat,
                ):
                    # k_r rope -> krf8 (planar re/im), shared across heads
                    krf8 = pb2.tile([RHD, T], F8, tag="krf8", name="krf8")
                    rtmp = pb2.tile([P, T], BF16, tag="rtmp", name="rtmp")
                    rro = pb2.tile([P, T], BF16, tag="rro", name="rro")
                    nc.vector.tensor_mul(rtmp[0:32, :], krraw[32:64, :], sa4[32:64, :])
                    nc.vector.tensor_mul(rtmp[32:64, :], krraw[32:64, :], ca4[32:64, :])
                    nc.vector.tensor_mul(rro[0:32, :], krraw[0:32, :], ca4[0:32, :])
                    nc.vector.tensor_mul(rro[32:64, :], krraw[0:32, :], sa4[0:32, :])
                    nc.vector.tensor_sub(krf8[0:32, :], rro[0:32, :], rtmp[0:32, :])
                    nc.vector.tensor_add(krf8[32:64, :], rro[32:64, :], rtmp[32:64, :])
                    for h in range(HLOC):
                        nc.sync.dma_start(k8[h][0:RHD, 1, :], krf8[:])

                    # up-projections, head-pair at a time
                    for g in range(2):
                        hA, hB = 2 * g, 2 * g + 1
                        qraw = pb2.tile([P, T], BF16, tag="qraw", name="qraw")
                        for ch in range(NCH):
                            sl = slice(ch * TCH, (ch + 1) * TCH)
                            for hh in (hA, hB):
                                for wT, src, dst in ((wuqT, cqT, q8),
                                                     (wukT, ckvT, k8)):
                                    acc = pmm.tile([P, TCH], F32, tag="mm",
                                                   name="mm")
                                    for nl in range(NNL):
                                        nc.tensor.matmul(
                                            acc[:],
                                            wT[nl][:, hh * P:(hh + 1) * P],
                                            src[nl][:, sl],
                                            start=(nl == 0),
                                            stop=(nl == NNL - 1),
                                        )
                                    ecopy(dst[hh][:, 0, sl], acc[:], pin="act")
                            qacc = pmm.tile([P, TCH], F32, tag="mm", name="mm")
                            for nl in range(NNL):
                                nc.tensor.matmul(
                                    qacc[:],
                                    wqrT[g][nl][:],
                                    cqT[nl][:, sl],
                                    start=(nl == 0),
                                    stop=(nl == NNL - 1),
                                )
                            ecopy(qraw[:, sl], qacc[:], pin="act")
                        # natural-layout V for both heads
                        for hh in (hA, hB):
                            for ts4 in range(T // TCH):
                                vps = pmm.tile([P, TCH], F32, tag="mm", name="mm")
                                for j in range(4):
                                    kt = ts4 * 4 + j
                                    for nl in range(NNL):
                                        nc.tensor.matmul(
                                            vps[:, j * P:(j + 1) * P],
                                            ckvT[nl][:, kt * P:(kt + 1) * P],
                                            wuvT[nl][:, hh * P:(hh + 1) * P],
                                            start=(nl == 0),
                                            stop=(nl == NNL - 1),
                                            skip_group_check=True,
                                        )
                                ecopy(
                                    vv[hh][:, ts4 * 4:(ts4 + 1) * 4, :],
                                    vps[:].rearrange("p (a b) -> p a b", a=4),
                                    pin="act",
                                )
                        # rope for the pair: rows [Are, Bre, Aim, Bim]
                        roq = pb2.tile([P, T], F8, tag="roq", bufs=2, name="roq")
                        nc.vector.tensor_mul(rtmp[0:64, :], qraw[64:128, :],
                                             sa4[64:128, :])
                        nc.vector.tensor_mul(rtmp[64:128, :], qraw[64:128, :],
                                             ca4[64:128, :])
                        nc.vector.tensor_mul(rro[0:64, :], qraw[0:64, :],
                                             ca4[0:64, :])
                        nc.vector.tensor_mul(rro[64:128, :], qraw[0:64, :],
                                             sa4[0:64, :])
                        nc.vector.tensor_sub(roq[0:64, :], rro[0:64, :],
                                             rtmp[0:64, :])
                        nc.vector.tensor_add(roq[64:128, :], rro[64:128, :],
                                             rtmp[64:128, :])
                        nc.sync.dma_start(q8[hA][0:32, 1, :], roq[0:32, :])
                        nc.sync.dma_start(q8[hA][32:64, 1, :], roq[64:96, :])
                        nc.sync.dma_start(q8[hB][0:32, 1, :], roq[32:64, :])
                        nc.sync.dma_start(q8[hB][32:64, 1, :], roq[96:128, :])

                    # ============ attention + deferred norm/output proj =====
                    deferred_b = deque()   # normalization tails
                    outproj_q = deque()    # (ohs, tq, cs_start) groups

                    def emit_outproj_group():
                        g_ohs, g_tq, cs = outproj_q.popleft()
                        g_qsl = slice(g_tq * TCH, (g_tq + 1) * TCH)
                        acc = pmm.tile([P, TCH], F32, tag="mm", name="mm")
                        for h2 in range(HLOC):
                            nc.tensor.matmul(
                                acc[:],
                                woT[h2][:, cs * P:(cs + 1) * P],
                                g_ohs[h2][:],
                                start=(h2 == 0),
                                stop=(h2 == HLOC - 1),
                            )
                        ot = pat.tile([P, TCH], F32, tag="ot", bufs=3,
                                      name="ot")
                        ecopy(ot[:], acc[:], pin="dve")
                        nc.sync.dma_start(
                            out_ext.ap()[cs * P:(cs + 1) * P, g_qsl], ot[:])

                    ohs_by_tq = {}
                    next_den = pde.tile([P, 4], F32, tag="de", name="de")
                    nc.vector.memset(next_den[:], 0.0)
                    for tq in range(NCH):
                        qsl = slice(tq * TCH, (tq + 1) * TCH)
                        ohs = []
                        ohs_by_tq[tq] = ohs
                        for h in range(HLOC):
                            outU = pou.tile([P, TCH], F32, tag="ou", name="ou")
                            den = next_den
                            nsub = (tq + 1) * 4
                            pend = deque()

                            def flush_one(outU=outU, den=den, nsub=nsub, h=h,
                                          tq=tq, pend=pend):
                                Pt, kt = pend.popleft()
                                kc, ks = kt // 4, kt % 4
                                off = ks * P if kc == tq else 0
                                for qq in range(off // P, 4):
                                    nc.tensor.matmul(
                                        den[:, qq:qq + 1],
                                        Pt[:, qq * P:(qq + 1) * P],
                                        onescol[:],
                                        start=False,
                                        stop=(kt == nsub - 1),
                                        skip_group_check=True,
                                    )
                                nc.tensor.matmul(
                                    outU[:, off:TCH],
                                    vv[h][:, kt, :],
                                    Pt[:, off:TCH],
                                    start=(kt == 0),
                                    stop=(kt == nsub - 1),
                                    skip_group_check=True,
                                )

                            for kt in range(nsub):
                                kc, ks = kt // 4, kt % 4
                                diag = kc == tq
                                off = ks * P if diag else 0
                                npr = TCH - off
                                ST = pmm.tile([P, TCH], F32, tag="mm", name="mm")
                                nc.tensor.matmul(
                                    ST[:, 0:npr],
                                    k8[h][:, :, kt * P:(kt + 1) * P],
                                    q8[h][:, :, qsl.start + off:qsl.stop],
                                    perf_mode=DR,
                                    start=True,
                                    stop=not diag,
                                )
                                if diag:
                                    # causal band mask folded in on PE:
                                    # ST[:, 0:128] += I^T @ mask
                                    nc.tensor.matmul(
                                        ST[:, 0:P],
                                        identb[:],
                                        maskbuf[:, 384:384 + P],
                                        start=False,
                                        stop=True,
                                        skip_group_check=True,
                                    )
                                Pt = pat.tile([P, TCH], BF16, tag="pt", bufs=6,
                                              name="pt")
                                nc.scalar.activation(Pt[:, off:TCH], ST[:, 0:npr],
                                                     Exp, scale=SCALE)
                                pend.append((Pt, kt))
                                if len(pend) > 2:
                                    flush_one()
                                if kt == 2 and deferred_b:
                                    # previous head's deferred normalization
                                    deferred_b.popleft()()
                                if kt >= 2 and kt % 2 == 0 and outproj_q:
                                    # one 128-row output-projection column
                                    emit_outproj_group()
                            while pend:
                                flush_one()

                            # normalization head: transpose+recip now (DVE),
                            # broadcast+apply deferred under the next head
                            den_sb = pat.tile([P, 4], F32R, tag="dsb", bufs=2,
                                              name="dsb")
                            nc.vector.tensor_copy(den_sb[:], den[:])
                            next_den = pde.tile([P, 4], F32, tag="de", name="de")
                            nc.vector.memset(next_den[:], 0.0)
                            tpd = pde.tile([4, P], F32R, tag="tpd", bufs=1,
                                           name="tpd")
                            nc.tensor.transpose(tpd[0:4, 0:P], den_sb[:], ident[:])
                            rec = pat.tile([4, P], F32, tag="rec", bufs=2,
                                           name="rec")
                            nc.vector.reciprocal(rec[:], tpd[:])
                            recb = pat.tile([4, P], BF16, tag="recb", bufs=2,
                                            name="recb")
                            nc.vector.tensor_copy(recb[:], rec[:])
                            oh = pat.tile([P, TCH], BF16, tag=f"oh{h}", bufs=2,
                                          name=f"oh{h}")
                            ohs.append(oh)

                            def norm_tail(recb=recb, outU=outU, oh=oh, h=h,
                                          tq=tq):
                                bc = pmm.tile([P, TCH], F32, tag="mm", name="mm")
                                for qq in range(4):
                                    nc.tensor.matmul(
                                        bc[:, qq * P:(qq + 1) * P],
                                        sel4[:, qq * P:(qq + 1) * P],
                                        recb[:],
                                        start=True, stop=True,
                                        skip_group_check=True,
                                    )
                                bcs = pat.tile([P, TCH], BF16, tag="bcs",
                                               bufs=2, name="bcs")
                                nc.vector.tensor_copy(bcs[:], bc[:])
                                nc.vector.tensor_mul(oh[:], outU[:], bcs[:])
                                if h == HLOC - 1:
                                    for cs0 in range(NCT):
                                        outproj_q.append(
                                            (ohs_by_tq[tq], tq, cs0))

                            deferred_b.append(norm_tail)

                    # drain deferred work
                    while deferred_b:
                        deferred_b.popleft()()
                    while outproj_q:
                        emit_outproj_group()

                pb2_ctx.__exit__(None, None, None)

    nc.compile()
    return nc


def _get_nc():
    if "nc" not in _NC_CACHE:
        _NC_CACHE["nc"] = build()
    return _NC_CACHE["nc"]


def kernel(x, freqs_cos, freqs_sin, W_dq, W_uq, W_dkv, W_uk, W_uv, W_qr, W_kr,
           W_o, trace=False, **trace_kwargs):
    nc = _get_nc()
    f32 = lambda a: np.ascontiguousarray(np.asarray(a, dtype=np.float32))
    x = f32(x); W_dq = f32(W_dq); W_uq = f32(W_uq); W_dkv = f32(W_dkv)
    W_uk = f32(W_uk); W_uv = f32(W_uv); W_qr = f32(W_qr); W_kr = f32(W_kr)
    W_o = f32(W_o)
    cos = f32(freqs_cos); sin = f32(freqs_sin)

    in_maps = []
    for c in range(8):
        b, r = divmod(c, 4)
        in_maps.append({
            "x": x[b],
            "wdq": W_dq, "wdkv": W_dkv, "wkr": W_kr,
            "wuq": W_uq[r * HLOC * HS:(r + 1) * HLOC * HS],
            "wuk": W_uk[r * HLOC * HS:(r + 1) * HLOC * HS],
            "wuv": W_uv[r * HLOC * HS:(r + 1) * HLOC * HS],
            "wqr": W_qr[r * HLOC * RHD:(r + 1) * HLOC * RHD],
            "wo": W_o[:, r * HLOC * HS:(r + 1) * HLOC * HS],
            "cos": cos, "sin": sin,
        })
    res = run_bass_kernel_spmd(nc, in_maps, core_ids=list(range(8)),
                               trace=trace, **trace_kwargs)
    out = np.zeros((B, T, C), dtype=np.float32)
    for c in range(8):
        b = c // 4
        out[b] += res.results[c]["out"].T
    kernel.last_result = res
    return out
